# revision 1
# baseline (speedup 1.0000x reference)
"""Trainium2 Bass kernel for nn_BertAdapterCapsuleMask.

Strategy (8 NeuronCores, SPMD — identical program, per-core data):

The reference computes, per example b:
  sem   = squash_n(x @ sem_w + sem_b)                      (capsule layer)
  priors[c,n,:] = sem[n,:] @ route_weights[c,n]            (routing priors)
  vote  = 3-iter masked dynamic routing over (c,b) pairs
  h_out = reshape(vote,(B,S,C)) @ larger_w' + larger_b'    (NB: the reshape
          mixes examples: h_out[b] reads vote rows 3b..3b+2 of the
          row-major [C*B, S] vote matrix)
  out   = x + adapter(x + h_out)                           (768->2048->768 MLP)

Sharding: the routing problem is independent per (c,b) pair (384 pairs).
Core k owns pairs t in [48k, 48k+48) AND examples b in [16k, 16k+16).
Because vote row index used by h_out[b] is exactly 3b..3b+2, core k's own
pairs produce precisely the vote rows its own examples need — zero
cross-core communication.  Each core computes sem for the 48 examples
b' = t mod 128 its pairs reference (sem is cheap), then routing, then the
adapter for its 16 own examples.

Weight folds (host side, exact):
  gfc1 folded into fc2_w rows;  glarger into larger_w;  (larger_b*glarger)
  and larger_b's path folded into fc1_b;  h_out@fc1_w folded to
  V @ M1 with M1 = (larger_w*glarger) @ fc1_w, so h_out never materializes.

Precision: matmuls in bf16 (fp32 PSUM accumulation), routing arithmetic in
fp32, final residual adds the untouched fp32 x.
"""

import numpy as np
import ml_dtypes

import concourse.bass as bass
import concourse.bacc as bacc
import concourse.mybir as mybir
import concourse.tile as tile
from concourse import bass_utils

BF16 = ml_dtypes.bfloat16
F32 = mybir.dt.float32
BF = mybir.dt.bfloat16
AF = mybir.ActivationFunctionType
ALU = mybir.AluOpType

B, S, H, A, C, N = 128, 128, 768, 2048, 3, 10
NUM_ITER = 3
NCORES = 8
BL = B // NCORES          # 16 own examples / core
NPAIR = 3 * B // NCORES   # 48 routing pairs / core
HK = H // 128             # 6
AK = A // 128             # 16
TOK = BL * S              # 2048 tokens / core
HALF = TOK // 2           # 1024
NC30 = N * C              # 30
NSEM = 18                 # sem examples per core: [16k, 16k+18) mod 128


def _sigmoid_f32(z):
    z = np.asarray(z, np.float32)
    out = np.empty_like(z)
    pos = z >= 0
    out[pos] = 1.0 / (1.0 + np.exp(-z[pos], dtype=np.float32))
    ez = np.exp(z[~pos], dtype=np.float32)
    out[~pos] = ez / (1.0 + ez)
    return out.astype(np.float32)


def _bf(x):
    return np.ascontiguousarray(np.asarray(x, np.float32).astype(BF16))


# ---------------------------------------------------------------------------
# device program
# ---------------------------------------------------------------------------

def _build_program(act_n, variant="full"):
    """variant: 'full' | 'capsule' (skip adapter, copy x->out) |
    'adapter' (skip capsule phases, memset vt) | 'p1'/'p2'/'p3' (capsule
    prefixes: sem only / +squash / +priors)."""
    level = {"p1": 1, "p2": 2, "p3": 3, "capsule": 4, "full": 4, "adapter": 0}[variant]
    nc = bacc.Bacc("TRN2", target_bir_lowering=False, debug=False,
                   num_devices=NCORES)

    d_xtsem = nc.dram_tensor("xt_sem", [6, HK, 128, 3 * S], F32, kind="ExternalInput")
    d_xtown = nc.dram_tensor("xt_own", [HK, 128, TOK], BF, kind="ExternalInput")
    d_xown = nc.dram_tensor("x_own", [BL, S, H], F32, kind="ExternalInput")
    d_rw = nc.dram_tensor("rw_pack", [C, 128, act_n * C * S], F32, kind="ExternalInput")
    d_sw = nc.dram_tensor("sw", [HK, 128, NC30], F32, kind="ExternalInput")
    d_semb = nc.dram_tensor("semb", [1, NC30], F32, kind="ExternalInput")
    d_m1 = nc.dram_tensor("m1", [C, A], BF, kind="ExternalInput")
    d_fc1w = nc.dram_tensor("fc1w", [HK, 128, A], BF, kind="ExternalInput")
    d_fc1b = nc.dram_tensor("fc1b", [128, AK], F32, kind="ExternalInput")
    d_fc2w = nc.dram_tensor("fc2w", [AK, 128, H], BF, kind="ExternalInput")
    d_b2 = nc.dram_tensor("b2row", [1, H], BF, kind="ExternalInput")
    d_g2 = nc.dram_tensor("g2", [1, H], F32, kind="ExternalInput")
    d_masks = nc.dram_tensor("masks", [NPAIR, C], F32, kind="ExternalInput")
    d_vcb = nc.dram_tensor("votecb", [NPAIR * S], BF, kind="Internal")
    d_out = nc.dram_tensor("out", [BL, S, H], F32, kind="ExternalOutput")

    with tile.TileContext(nc) as tc:
        with (
            tc.tile_pool(name="w", bufs=1) as wp,
            tc.tile_pool(name="semx", bufs=2) as sxp,
            tc.tile_pool(name="sem", bufs=1) as smp,
            tc.tile_pool(name="rt", bufs=1) as rp,
            tc.tile_pool(name="ad", bufs=1) as ap_,
            tc.tile_pool(name="st", bufs=2) as sp,
            tc.tile_pool(name="ps", bufs=8, space="PSUM") as pp,
            tc.tile_pool(name="dram", bufs=1, space="DRAM") as dp,
        ):
            # ---------------- persistent weights -----------------
            # (adapter-prepass inputs first: PE can start on fc1 immediately)
            xo_sb = wp.tile([128, HK * TOK], BF, tag="bigx", bufs=1)
            for hk in range(HK):
                nc.scalar.dma_start(xo_sb[:, hk * TOK:(hk + 1) * TOK], d_xtown[hk])
            fc1w_sb = wp.tile([128, HK * A], BF)
            for hk in range(HK):
                nc.scalar.dma_start(fc1w_sb[:, hk * A:(hk + 1) * A], d_fc1w[hk])
            fc1b_sb = wp.tile([128, AK], F32)
            nc.scalar.dma_start(fc1b_sb[:], d_fc1b[:])
            sw_sb = wp.tile([128, HK * NC30], F32)
            for hk in range(HK):
                nc.sync.dma_start(sw_sb[:, hk * NC30:(hk + 1) * NC30], d_sw[hk])
            semb_sb = wp.tile([1, NC30], F32)
            nc.sync.dma_start(semb_sb[:], d_semb[:])
            ones_sb = wp.tile([1, 128], BF)
            nc.gpsimd.memset(ones_sb[:], 1.0)
            ones_f = wp.tile([1, 128], F32)
            nc.gpsimd.memset(ones_f[:], 1.0)
            masks_sb = wp.tile([NPAIR, C], F32)
            nc.sync.dma_start(masks_sb[:], d_masks[:])
            m1_sb = wp.tile([C, A], BF)
            nc.sync.dma_start(m1_sb[:], d_m1[:])
            fc2w_sb = wp.tile([128, AK * H], BF, tag="bigx", bufs=1)
            for ak in range(AK):
                nc.scalar.dma_start(fc2w_sb[:, ak * H:(ak + 1) * H], d_fc2w[ak])
            b2_sb = wp.tile([1, H], BF)
            nc.sync.dma_start(b2_sb[:], d_b2[:])
            g2rep = wp.tile([128, H], F32)
            g2_src = d_g2.ap()  # [1, H] dram -> broadcast to 128 partitions
            g2_b = bass.AP(g2_src.tensor, g2_src.offset, [[0, 128], [1, H]])
            nc.sync.dma_start(g2rep[:], g2_b)

            # ---------------- fc1 pass 1 (x-only part; no routing dep) -----
            # z1p accumulates fc1_w.T @ xT; the capsule term M1.T@VT, bias and
            # relu are applied in pass 2 once routing is done.  Half B is
            # emitted after fc2-A (its z1p slot reuses half A's).
            z1ps = {}

            def emit_fc1_pass1(hf):
                z1p = ap_.tile([128, AK * HALF], BF, tag="z1p", bufs=2,
                               name=f"z1p_{hf}")
                z1ps[hf] = z1p
                for ak in range(AK):
                    pss = [pp.tile([128, 512], F32, tag="mm",
                                   name=f"ps_p1_{hf}_{ak}_{i}") for i in range(2)]
                    for hk in range(HK):
                        lhsT = fc1w_sb[:, hk * A + ak * 128: hk * A + (ak + 1) * 128]
                        for i in range(2):
                            col = hf * HALF + i * 512
                            nc.tensor.matmul(
                                pss[i][:], lhsT,
                                xo_sb[:, hk * TOK + col: hk * TOK + col + 512],
                                start=(hk == 0), stop=(hk == HK - 1))
                    for i in range(2):
                        nc.scalar.copy(
                            z1p[:, ak * HALF + i * 512: ak * HALF + (i + 1) * 512],
                            pss[i][:])


            if variant != "adapter":
                # ------- phase 1: sem + squash (18 examples [16k,16k+18)) -----
                sem_own = smp.tile([128, NSEM * NC30], F32)
                for g in range(6):
                    xt_g = sxp.tile([128, HK * 3 * S], F32, tag="xtg")
                    src_ = d_xtsem.ap()[g]  # [HK, 128, 384]
                    nc.sync.dma_start(
                        xt_g[:].rearrange("p (hk c) -> p hk c", hk=HK),
                        src_.rearrange("hk p c -> p hk c"))
                    for el in range(3):
                        slot = g * 3 + el
                        ps = pp.tile([128, NC30], F32, tag="mm", name=f"ps_sem_{slot}")
                        for hk in range(HK):
                            nc.tensor.matmul(
                                ps[:],
                                xt_g[:, hk * (3 * S) + el * S: hk * (3 * S) + (el + 1) * S],
                                sw_sb[:, hk * NC30:(hk + 1) * NC30],
                                start=(hk == 0), stop=False)
                        nc.tensor.matmul(ps[:], ones_f[:], semb_sb[:],
                                         start=False, stop=True)
                        nc.scalar.copy(sem_own[:, slot * NC30:(slot + 1) * NC30], ps[:])

                # squash over n:  f = sqrt(sq)/(1+sq) via exp(0.5*ln(sq))
                sem2 = smp.tile([128, NSEM * NC30], F32)
                nc.vector.tensor_tensor(sem2[:], sem_own[:], sem_own[:], op=ALU.mult)
                sqt = smp.tile([128, NSEM * C], F32)
                nc.vector.tensor_reduce(
                    sqt[:].rearrange("p (slot cc) -> p slot cc", cc=C),
                    sem2[:].rearrange("p (slot n cc) -> p slot cc n", n=N, cc=C),
                    axis=mybir.AxisListType.X, op=ALU.add)
                lnq = smp.tile([128, NSEM * C], F32)
                nc.scalar.activation(lnq[:], sqt[:], AF.Ln)
                sqq = smp.tile([128, NSEM * C], F32)
                nc.scalar.activation(sqq[:], lnq[:], AF.Exp, scale=0.5)  # sqrt(sq)
                up = smp.tile([128, NSEM * C], F32)
                nc.vector.tensor_scalar_add(up[:], sqt[:], 1.0)
                ru = smp.tile([128, NSEM * C], F32)
                nc.vector.reciprocal(ru[:], up[:])
                fq = smp.tile([128, NSEM * C], F32)
                nc.vector.tensor_tensor(fq[:], sqq[:], ru[:], op=ALU.mult)
                # sem_sq = sem_own * f  (f broadcast over n), fp32
                sem_sq = sem2  # reuse scratch
                f_ap = fq[:]
                f_b = bass.AP(f_ap.tensor, f_ap.offset,
                              [f_ap.ap[0], [C, NSEM], [0, N], [1, C]])
                nc.vector.tensor_tensor(
                    sem_sq[:].rearrange("p (slot n cc) -> p slot n cc", n=N, cc=C),
                    sem_own[:].rearrange("p (slot n cc) -> p slot n cc", n=N, cc=C),
                    f_b, op=ALU.mult)
                # materialize pair-ordered copy: block p=3i+u <- slot i+u
                # (matmul weight APs allow only one free dim, so gather here)
                sem_pair = smp.tile([128, NPAIR * NC30], F32)
                sq_ap = sem_sq[:]
                gather = bass.AP(sq_ap.tensor, sq_ap.offset,
                                 [sq_ap.ap[0], [NC30, BL], [NC30, C], [1, NC30]])
                nc.vector.tensor_copy(
                    sem_pair[:].rearrange("p (i u nc) -> p i u nc", i=BL, u=C),
                    gather)

                if level >= 3:
                    # ---------------- phase 2: priors -----------------
                    # lhsT rows (pair p = 3i+u) read sem slot i+u:
                    # AP dims [(30,16)@i, (30,3)@u] both stride 30 (overlapping)
                    sem_v = sem_pair[:].rearrange("p (pair nc) -> p nc pair", nc=NC30)
                    priors = rp.tile([NPAIR, act_n * S], F32)
                    for g in range(C):
                        for n in range(act_n):
                            ps = pp.tile([NPAIR, S], F32, tag="mm", name=f"ps_pr_{g}_{n}")
                            rwt = sxp.tile([128, C * S], F32, tag="rwt", bufs=4,
                                           name=f"rw_{g}_{n}")
                            nc.scalar.dma_start(
                                rwt[:], d_rw.ap()[g][:, (n * C) * S:(n * C + C) * S])
                            for cc in range(C):
                                nc.tensor.matmul(
                                    ps[:], sem_v[:, n * C + cc, :],
                                    rwt[:, cc * S:(cc + 1) * S],
                                    start=(cc == 0), stop=(cc == C - 1))
                            dst = priors[:, n * S:(n + 1) * S]
                            for g2 in range(1):
                                pass
                            if g == 0:
                                nc.vector.tensor_scalar_mul(dst, ps[:], masks_sb[:, 0:1])
                            else:
                                nc.vector.scalar_tensor_tensor(
                                    dst, ps[:], masks_sb[:, g:g + 1], dst,
                                    op0=ALU.mult, op1=ALU.add)

                if level >= 4:
                    # ---------------- phase 3: routing -----------------
                    vote = rp.tile([NPAIR, S], F32)
                    scr = rp.tile([NPAIR, S], F32)
                    La = rp.tile([NPAIR, act_n], F32)
                    Lb = rp.tile([NPAIR, act_n], F32)
                    sqv = rp.tile([NPAIR, 1], F32)
                    lv = rp.tile([NPAIR, 1], F32)
                    sv = rp.tile([NPAIR, 1], F32)
                    uv = rp.tile([NPAIR, 1], F32)
                    rv = rp.tile([NPAIR, 1], F32)
                    fv = rp.tile([NPAIR, 1], F32)
                    outv = rp.tile([NPAIR, S], F32)
                    mx = rp.tile([NPAIR, 1], F32)
                    mneg = rp.tile([NPAIR, 1], F32)
                    ex = rp.tile([NPAIR, act_n], F32)
                    es = rp.tile([NPAIR, 1], F32)
                    ers = rp.tile([NPAIR, 1], F32)
                    probs = rp.tile([NPAIR, act_n], F32)

                    def vote_from(pr_scalar_ap_or_const, first_const=None):
                        """vote = sum_n probs_n * priors_n."""
                        for n in range(act_n):
                            blk = priors[:, n * S:(n + 1) * S]
                            sc = (first_const if first_const is not None
                                  else pr_scalar_ap_or_const[:, n:n + 1])
                            if n == 0:
                                nc.vector.tensor_scalar_mul(vote[:], blk, sc)
                            else:
                                nc.vector.scalar_tensor_tensor(
                                    vote[:], blk, sc, vote[:], op0=ALU.mult, op1=ALU.add)

                    def squash_vote():
                        nc.vector.tensor_tensor(scr[:], vote[:], vote[:], op=ALU.mult)
                        nc.vector.tensor_reduce(sqv[:], scr[:],
                                                axis=mybir.AxisListType.X, op=ALU.add)
                        nc.scalar.activation(lv[:], sqv[:], AF.Ln)
                        nc.scalar.activation(sv[:], lv[:], AF.Exp, scale=0.5)
                        nc.vector.tensor_scalar_add(uv[:], sqv[:], 1.0)
                        nc.vector.reciprocal(rv[:], uv[:])
                        nc.vector.tensor_tensor(fv[:], sv[:], rv[:], op=ALU.mult)
                        nc.vector.tensor_scalar_mul(outv[:], vote[:], fv[:])

                    def deltas(Lprev, Lnew, first):
                        for n in range(act_n):
                            nc.vector.tensor_tensor(
                                scr[:], priors[:, n * S:(n + 1) * S], outv[:],
                                op=ALU.mult)
                            nc.vector.tensor_reduce(
                                Lnew[:, n:n + 1], scr[:],
                                axis=mybir.AxisListType.X, op=ALU.add)
                        if not first:
                            nc.vector.tensor_tensor(Lnew[:], Lnew[:], Lprev[:],
                                                    op=ALU.add)

                    def softmax(L):
                        nc.vector.tensor_reduce(mx[:], L[:], axis=mybir.AxisListType.X,
                                                op=ALU.max)
                        nc.vector.tensor_scalar_mul(mneg[:], mx[:], -1.0)
                        nc.scalar.activation(ex[:], L[:], AF.Exp, bias=mneg[:],
                                             accum_out=es[:])
                        nc.vector.reciprocal(ers[:], es[:])
                        nc.vector.tensor_scalar_mul(probs[:], ex[:], ers[:])

                    # iter 0
                    vote_from(None, first_const=1.0 / act_n)
                    squash_vote()
                    deltas(None, La, first=True)
                    # iter 1
                    softmax(La)
                    vote_from(probs)
                    squash_vote()
                    deltas(La, Lb, first=False)
                    # iter 2 (final)
                    softmax(Lb)
                    vote_from(probs)

                    vb = rp.tile([NPAIR, S], BF)
                    nc.vector.tensor_copy(vb[:], vote[:])
                    nc.sync.dma_start(
                        d_vcb.ap().rearrange("(p s) -> p s", p=NPAIR), vb[:])

                    # VT[c, e*128+s] = votecb_flat[3*e*128 + 3*s + c]
                    vt_sb = ap_.tile([C, TOK], BF)
                    vflat = d_vcb.ap()
                    for e in range(BL):
                        src = bass.AP(vflat.tensor, vflat.offset + 3 * e * S,
                                      [[1, C], [C, S]])
                        nc.sync.dma_start(vt_sb[:, e * S:(e + 1) * S], src)

            else:
                vt_sb = ap_.tile([C, TOK], BF)
                nc.gpsimd.memset(vt_sb[:], 0.0)

            if variant in ("full", "adapter"):
                emit_fc1_pass1(0)
                emit_fc1_pass1(1)
                # -------- phase 4: fc1 pass 2 (capsule term) + fc2 --------
                def emit_fc1_pass2_and_fc2(hf):
                    z1 = z1ps[hf]
                    for ak in range(AK):
                        ps2 = [pp.tile([128, 512], F32, tag="mm",
                                       name=f"ps_p2_{hf}_{ak}_{i}") for i in range(2)]
                        m1l = m1_sb[:, ak * 128:(ak + 1) * 128]
                        for i in range(2):
                            col = hf * HALF + i * 512
                            nc.tensor.matmul(ps2[i][:], m1l,
                                             vt_sb[:, col:col + 512],
                                             start=True, stop=True)
                        tmp = sp.tile([128, HALF], F32, tag="tmp",
                                      name=f"tmp_{hf}_{ak}", bufs=2)
                        for i in range(2):
                            # tmp = (ps2 + fc1b) + z1p
                            nc.vector.scalar_tensor_tensor(
                                tmp[:, i * 512:(i + 1) * 512], ps2[i][:],
                                fc1b_sb[:, ak:ak + 1],
                                z1[:, ak * HALF + i * 512: ak * HALF + (i + 1) * 512],
                                op0=ALU.add, op1=ALU.add)
                        nc.scalar.activation(
                            z1[:, ak * HALF:(ak + 1) * HALF], tmp[:], AF.Relu)
                    for tt in range(8):
                        e = hf * 8 + tt
                        psa = pp.tile([128, 512], F32, tag="mm", name=f"ps_f2a_{e}")
                        psb = pp.tile([128, 256], F32, tag="mm", name=f"ps_f2b_{e}")
                        for ak in range(AK):
                            lhsT = z1[:, ak * HALF + tt * 128: ak * HALF + (tt + 1) * 128]
                            nc.tensor.matmul(psa[:], lhsT,
                                             fc2w_sb[:, ak * H: ak * H + 512],
                                             start=(ak == 0), stop=False)
                            nc.tensor.matmul(psb[:], lhsT,
                                             fc2w_sb[:, ak * H + 512: ak * H + H],
                                             start=(ak == 0), stop=False)
                        nc.tensor.matmul(psa[:], ones_sb[:], b2_sb[:, 0:512],
                                         start=False, stop=True)
                        nc.tensor.matmul(psb[:], ones_sb[:], b2_sb[:, 512:H],
                                         start=False, stop=True)
                        xt = sp.tile([128, H], F32, tag="x", name=f"x_{e}")
                        nc.sync.dma_start(xt[:], d_xown[e])
                        ot = sp.tile([128, H], F32, tag="o", name=f"o_{e}")
                        nc.scalar.activation(ot[:, 0:512], psa[:], AF.Relu)
                        nc.scalar.activation(ot[:, 512:H], psb[:], AF.Relu)
                        nc.vector.tensor_tensor(ot[:], ot[:], g2rep[:], op=ALU.mult)
                        nc.vector.tensor_tensor(ot[:], ot[:], xt[:], op=ALU.add)
                        nc.sync.dma_start(d_out[e], ot[:])

                emit_fc1_pass2_and_fc2(0)
                emit_fc1_pass2_and_fc2(1)
            else:
                for e in range(BL):
                    xt = sp.tile([128, H], F32, tag="x", name=f"xc_{e}")
                    nc.sync.dma_start(xt[:], d_xown[e])
                    nc.sync.dma_start(d_out[e], xt[:])

    nc.compile()
    return nc


# ---------------------------------------------------------------------------
# host marshaling
# ---------------------------------------------------------------------------

def _prep_core_inputs(k, x, shared, act_n):
    # own (output) examples: b_i = 48k + 43 i (mod 128).  Because
    # 3*43 = 129 = 1 (mod 128), the 48 routing pairs t = 3 b_i + u map to
    # sem examples b' = t mod 128 = 16k + (i + u) mod 128 — just the 18
    # consecutive examples [16k, 16k+18).  Pair (i,u) sits at row 3i+u and
    # reads sem slot i+u; votecb rows 3e..3e+2 are exactly what h_out of
    # own example e needs, so no cross-core traffic anywhere.
    own = np.array([(48 * k + 43 * i) % B for i in range(BL)])
    sem_ex = np.array([(16 * k + j) % B for j in range(NSEM)])

    # xt_sem: [6, hk, 128, 3*S] fp32, groups of 3 sem examples
    xs = np.transpose(x[sem_ex], (2, 0, 1)).reshape(H, NSEM * S).astype(np.float32)
    xt_sem = np.empty((6, HK, 128, 3 * S), np.float32)
    for g in range(6):
        for hk in range(HK):
            xt_sem[g, hk] = xs[hk * 128:(hk + 1) * 128,
                               g * 3 * S:(g + 1) * 3 * S]

    xo = np.transpose(x[own], (2, 0, 1)).reshape(H, TOK).astype(BF16)
    xt_own = np.ascontiguousarray(xo.reshape(HK, 128, TOK))
    x_own = np.ascontiguousarray(x[own].astype(np.float32))

    # group g == c' directly; mask[p, g] = (c' of pair p == g)
    masks = np.zeros((NPAIR, C), np.float32)
    for i in range(BL):
        for u in range(C):
            t = 3 * int(own[i]) + u
            masks[3 * i + u, t // B] = 1.0

    return {
        "xt_sem": xt_sem,
        "xt_own": xt_own,
        "x_own": x_own,
        "rw_pack": shared["rw_pack"],
        "masks": masks,
        **{n: shared[n] for n in ("sw", "semb", "m1", "fc1w", "fc1b",
                                  "fc2w", "b2row", "g2")},
    }


_CACHE = {}


def _make_shared(inputs):
    fc1_w = np.asarray(inputs["fc1_w"], np.float32)
    fc1_b = np.asarray(inputs["fc1_b"], np.float32)
    fc2_w = np.asarray(inputs["fc2_w"], np.float32)
    fc2_b = np.asarray(inputs["fc2_b"], np.float32)
    efc1 = np.asarray(inputs["efc1"], np.float32)
    efc2 = np.asarray(inputs["efc2"], np.float32)
    sem_w = np.asarray(inputs["sem_w"], np.float32)
    sem_b = np.asarray(inputs["sem_b"], np.float32)
    route_weights = np.asarray(inputs["route_weights"], np.float32)
    larger_w = np.asarray(inputs["larger_w"], np.float32)
    larger_b = np.asarray(inputs["larger_b"], np.float32)
    elarger = np.asarray(inputs["elarger"], np.float32)
    t = int(np.asarray(inputs["t"]))
    sf = np.float32(int(np.asarray(inputs["s"])))
    act_n = t + 1

    gfc1 = _sigmoid_f32(sf * efc1[t])
    gfc2 = _sigmoid_f32(sf * efc2[t])
    glarger = _sigmoid_f32(sf * elarger[t])

    lwg = (larger_w * glarger[None, :]).astype(np.float32)
    lb_eff = (larger_b * glarger).astype(np.float32)
    rw4 = route_weights.reshape(C, N, S, C, S)

    return {
        "sw": np.ascontiguousarray(np.transpose(sem_w, (1, 0, 2))
                                   .reshape(H, NC30).astype(np.float32)
                                   ).reshape(HK, 128, NC30),
        "semb": np.ascontiguousarray(sem_b.reshape(1, NC30).astype(np.float32)),
        "m1": _bf(lwg @ fc1_w),
        "fc1w": _bf(fc1_w).reshape(HK, 128, A),
        "fc1b": np.ascontiguousarray(
            (fc1_b + lb_eff @ fc1_w).astype(np.float32).reshape(AK, 128).T),
        "fc2w": _bf(fc2_w * gfc1[:, None]).reshape(AK, 128, H),
        "b2row": _bf(fc2_b.reshape(1, H)),
        "g2": np.ascontiguousarray(gfc2.reshape(1, H)),
        "rw_pack": np.stack([
            np.ascontiguousarray(np.transpose(rw4[c, :act_n], (1, 0, 2, 3))
                                 .reshape(S, act_n * C * S).astype(np.float32))
            for c in range(C)]),
    }


def kernel(**inputs):
    x = np.asarray(inputs["x"], np.float32)
    t = int(np.asarray(inputs["t"]))
    act_n = t + 1
    shared = _make_shared(inputs)

    if act_n not in _CACHE:
        _CACHE[act_n] = _build_program(act_n)
    nc = _CACHE[act_n]

    in_maps = [_prep_core_inputs(k, x, shared, act_n) for k in range(NCORES)]
    res = bass_utils.run_bass_kernel_spmd(nc, in_maps, core_ids=list(range(NCORES)))
    out = np.empty((B, S, H), np.float32)
    for k in range(NCORES):
        own = [(48 * k + 43 * i) % B for i in range(BL)]
        out[own] = res.results[k]["out"]
    return out



# revision 5
# speedup vs baseline: 1.3068x; 1.3068x over previous
"""Trainium2 Bass kernel for nn_BertAdapterCapsuleMask (fp8 DoubleRow version).

Strategy (8 NeuronCores, SPMD — identical program, per-core data):

Sharding (same as proven baseline): core k owns routing pairs
t in [48k, 48k+48) and output examples b_i = (48k + 43 i) mod 128, so the
pairs' sem examples are exactly the 18 consecutive [16k, 16k+18) and the
vote rows each core produces are exactly the ones its own examples'
adapter needs — zero cross-core traffic.

Speed: the adapter GEMMs (fc1 768x2048, fc2 2048x768 per 2048 tokens)
run in fp8-e4m3 with MatmulPerfMode.DoubleRow (256-deep contraction,
half-cycle per output column).  Accuracy is restored with residual
("lo") streams quantized at the SAME dequant scale as the hi streams —
fp8's relative precision is scale-invariant, so all streams of one GEMM
accumulate into a single PSUM group with zero combine cost:

  fc1 psum  = xh*w1h + xl*w1h + xh*w1l + vt8*m1q      (all at 2^15)
  fc2 psum  = h8*w2h + h8*w2l                          (at 2^15)

The capsule path keeps near-fp32 accuracy (routing softmax amplifies
sem errors ~10x): sem runs as four fp8 streams with fine residual
scales (separate psums, 3 tiny combines), squash/routing stay fp32,
priors run bf16.  The capsule term enters fc1 via the vt8*m1q rank-3
DoubleRow update, so it never materializes as h_out.

Weight folds (host, exact): gfc1 into fc2_w; glarger into larger_w;
(larger_b*glarger + fc1_b) into the fc1 Act bias; gfc2*2^-15 into the
fc2 output gate.
"""

import numpy as np
import ml_dtypes

import concourse.bass as bass
import concourse.bacc as bacc
import concourse.mybir as mybir
import concourse.tile as tile
from concourse import bass_utils

E4 = ml_dtypes.float8_e4m3
BF16 = ml_dtypes.bfloat16
F32 = mybir.dt.float32
BF = mybir.dt.bfloat16
F8 = mybir.dt.float8e4
AF = mybir.ActivationFunctionType
ALU = mybir.AluOpType
PM = mybir.MatmulPerfMode

B, S, H, A, C, N = 128, 128, 768, 2048, 3, 10
NUM_ITER = 3
NCORES = 8
BL = B // NCORES          # 16 own examples / core
NPAIR = 3 * B // NCORES   # 48 routing pairs / core
HK = H // 128             # 6
AK = A // 128             # 16
TOK = BL * S              # 2048 tokens / core
NC30 = N * C              # 30
NSEM = 18                 # sem examples per core: [16k, 16k+18) mod 128
NPRE = 2                  # fc1 col-chunks (of 512 tokens) via z1p prepass

# fp8 scales.  All adapter streams share dequant 2^-15; sem streams use
# fine residual scales (routing is sensitive) and combine explicitly.
SXH = 16.0
SW = 2048.0
SVT, SM1 = 8.0, 4096.0         # SVT*SM1 == SXH*SW == 2^15
SH8 = 16.0
SXSL = 512.0                   # xsem lo
SSWL = 32768.0                 # sem_w lo
Z1INV = 1.0 / 32768.0          # 2^-15
H8SC = SH8 * Z1INV             # 2^-11


def _sigmoid_f32(z):
    z = np.asarray(z, np.float64)
    return (1.0 / (1.0 + np.exp(-z))).astype(np.float32)


def _q8(a, scale):
    """fp8-e4m3 quantize: stores clip(a*scale, +-240); dequant is 1/scale."""
    z = np.clip(np.asarray(a, np.float32) * np.float32(scale), -240.0, 240.0)
    return np.ascontiguousarray(z.astype(E4))


def _bf(x):
    return np.ascontiguousarray(np.asarray(x, np.float32).astype(BF16))


# ---------------------------------------------------------------------------
# device program
# ---------------------------------------------------------------------------

def _build_program(act_n, has_semb, has_b2):
    nc = bacc.Bacc("TRN2", target_bir_lowering=False, debug=False,
                   num_devices=NCORES)

    d_xsh = nc.dram_tensor("xsem_h", [6, HK, 128, 3 * S], F8, kind="ExternalInput")
    d_xsl = nc.dram_tensor("xsem_l", [6, HK, 128, 3 * S], F8, kind="ExternalInput")
    d_swh = nc.dram_tensor("swh", [HK, 128, NC30], F8, kind="ExternalInput")
    d_swl = nc.dram_tensor("swl", [HK, 128, NC30], F8, kind="ExternalInput")
    d_semb = nc.dram_tensor("semb", [1, NC30], F32, kind="ExternalInput")
    d_rw = nc.dram_tensor("rw_pack", [C, 128, act_n * C * S], BF, kind="ExternalInput")
    d_masks = nc.dram_tensor("masks", [NPAIR, C], F32, kind="ExternalInput")
    d_xh = nc.dram_tensor("xq_h", [HK, 128, TOK], F8, kind="ExternalInput")
    d_xl = nc.dram_tensor("xq_l", [HK, 128, TOK], F8, kind="ExternalInput")
    d_w1h = nc.dram_tensor("w1h", [HK, 128, A], F8, kind="ExternalInput")
    d_w1l = nc.dram_tensor("w1l", [HK, 128, A], F8, kind="ExternalInput")
    d_w2h = nc.dram_tensor("w2h", [AK, 128, H], F8, kind="ExternalInput")
    d_w2l = nc.dram_tensor("w2l", [AK, 128, H], F8, kind="ExternalInput")
    d_m1q = nc.dram_tensor("m1q", [C, A], F8, kind="ExternalInput")
    d_b1s = nc.dram_tensor("b1s", [128, AK], F32, kind="ExternalInput")
    d_g2s = nc.dram_tensor("g2s", [1, H], F32, kind="ExternalInput")
    d_b2 = nc.dram_tensor("b2row", [1, H], BF, kind="ExternalInput")
    d_xown = nc.dram_tensor("x_own", [BL, S, H], F32, kind="ExternalInput")
    d_vcb = nc.dram_tensor("votecb", [NPAIR * S], F8, kind="Internal")
    d_out = nc.dram_tensor("out", [BL, S, H], F32, kind="ExternalOutput")

    def pair_ap(ap, off, stride, n):
        """[[part], [stride, 2], [1, n]] AP at ap.offset+off (DoubleRow pair)."""
        return bass.AP(ap.tensor, ap.offset + off, [ap.ap[0], [stride, 2], [1, n]])

    with tile.TileContext(nc) as tc:
        with (
            tc.tile_pool(name="w", bufs=1) as wp,
            tc.tile_pool(name="semx", bufs=2) as sxp,
            tc.tile_pool(name="sem", bufs=1) as smp,
            tc.tile_pool(name="rt", bufs=1) as rp,
            tc.tile_pool(name="ad", bufs=1) as ap_,
            tc.tile_pool(name="st", bufs=2) as sp,
            tc.tile_pool(name="ps", bufs=8, space="PSUM") as pp,
        ):
            # ---------------- persistent loads -----------------
            # sync queue: capsule-critical data first
            swh_sb = wp.tile([128, HK * NC30], F8)
            swl_sb = wp.tile([128, HK * NC30], F8)
            for hk in range(HK):
                nc.sync.dma_start(swh_sb[:, hk * NC30:(hk + 1) * NC30], d_swh[hk])
                nc.sync.dma_start(swl_sb[:, hk * NC30:(hk + 1) * NC30], d_swl[hk])
            semb_sb = wp.tile([1, NC30], F32)
            nc.sync.dma_start(semb_sb[:], d_semb[:])
            masks_sb = wp.tile([NPAIR, C], F32)
            nc.sync.dma_start(masks_sb[:], d_masks[:])
            # scalar queue: fc1 prepass data first
            xh_sb = wp.tile([128, HK * TOK], F8, tag="bigx", bufs=1)
            w1h_sb = wp.tile([128, HK * A], F8)
            for hk in range(HK):
                nc.scalar.dma_start(xh_sb[:, hk * TOK:(hk + 1) * TOK], d_xh[hk])
            for hk in range(HK):
                nc.scalar.dma_start(w1h_sb[:, hk * A:(hk + 1) * A], d_w1h[hk])
            xl_sb = wp.tile([128, HK * TOK], F8, tag="bigx2", bufs=1)
            w1l_sb = wp.tile([128, HK * A], F8)
            for hk in range(HK):
                nc.scalar.dma_start(xl_sb[:, hk * TOK:(hk + 1) * TOK], d_xl[hk])
            for hk in range(HK):
                nc.scalar.dma_start(w1l_sb[:, hk * A:(hk + 1) * A], d_w1l[hk])
            w2h_sb = wp.tile([128, AK * H], F8, tag="bigw2", bufs=1)
            w2l_sb = wp.tile([128, AK * H], F8, tag="bigw2l", bufs=1)
            for ak in range(AK):
                nc.scalar.dma_start(w2h_sb[:, ak * H:(ak + 1) * H], d_w2h[ak])
            for ak in range(AK):
                nc.scalar.dma_start(w2l_sb[:, ak * H:(ak + 1) * H], d_w2l[ak])

            # m1 lhsT [128, 2*A]: rows 0-2 of k-tile0 hold m1q, rest zero
            m1_sb = wp.tile([128, 2 * A], F8)
            nc.gpsimd.memset(m1_sb[:], 0.0)
            m1_dst = bass.AP(m1_sb[:].tensor, m1_sb[:].offset,
                             [[m1_sb[:].ap[0][0], C], [1, A]])
            nc.scalar.dma_start(m1_dst, d_m1q.ap())
            # vt rhs [128, 2*TOK]: rows 0-2 of k-tile0 get the vote gather;
            # everything else zero (multiplied by zero m1 rows anyway, but
            # the simulator wants initialized reads).
            vt_sb = ap_.tile([128, 2 * TOK], F8)
            nc.gpsimd.memset(vt_sb[:], 0.0)

            b1_sb = wp.tile([128, AK], F32)
            nc.sync.dma_start(b1_sb[:], d_b1s[:])
            g2rep = wp.tile([128, H], F32)
            g2_src = d_g2s.ap()
            nc.sync.dma_start(
                g2rep[:], bass.AP(g2_src.tensor, g2_src.offset, [[0, 128], [1, H]]))
            ones_f = wp.tile([1, 128], F32)
            nc.gpsimd.memset(ones_f[:], 1.0)
            if has_b2:
                ones_bf = wp.tile([1, 128], BF)
                nc.gpsimd.memset(ones_bf[:], 1.0)
                b2_sb = wp.tile([1, H], BF)
                nc.sync.dma_start(b2_sb[:], d_b2[:])

            # ---------------- phase 1: sem (4 fp8 streams) -----------------
            sem_own = smp.tile([128, NSEM * NC30], F32)
            for g in range(6):
                xtg_h = sxp.tile([128, HK * 3 * S], F8, tag="xtgh")
                xtg_l = sxp.tile([128, HK * 3 * S], F8, tag="xtgl")
                nc.sync.dma_start(
                    xtg_h[:].rearrange("p (hk c) -> p hk c", hk=HK),
                    d_xsh.ap()[g].rearrange("hk p c -> p hk c"))
                nc.sync.dma_start(
                    xtg_l[:].rearrange("p (hk c) -> p hk c", hk=HK),
                    d_xsl.ap()[g].rearrange("hk p c -> p hk c"))
                for el in range(3):
                    slot = g * 3 + el
                    ps_a = pp.tile([128, NC30], F32, tag="mm", name=f"ps_sa_{slot}")
                    ps_b = pp.tile([128, NC30], F32, tag="mm", name=f"ps_sb_{slot}")
                    ps_c = pp.tile([128, NC30], F32, tag="mm", name=f"ps_sc_{slot}")
                    ps_d = pp.tile([128, NC30], F32, tag="mm", name=f"ps_sd_{slot}")
                    for hp in range(HK // 2):
                        off = (2 * hp) * (3 * S) + el * S
                        lhT_h = pair_ap(xtg_h[:], off, 3 * S, 128)
                        lhT_l = pair_ap(xtg_l[:], off, 3 * S, 128)
                        woff = (2 * hp) * NC30
                        rw_h = pair_ap(swh_sb[:], woff, NC30, NC30)
                        rw_l = pair_ap(swl_sb[:], woff, NC30, NC30)
                        st = hp == 0
                        nc.tensor.matmul(ps_a[:], lhT_h, rw_h, start=st,
                                         stop=(hp == 2 and not has_semb),
                                         perf_mode=PM.DoubleRow)
                        nc.tensor.matmul(ps_b[:], lhT_l, rw_h, start=st,
                                         stop=(hp == 2), perf_mode=PM.DoubleRow)
                        nc.tensor.matmul(ps_c[:], lhT_h, rw_l, start=st,
                                         stop=(hp == 2), perf_mode=PM.DoubleRow)
                        nc.tensor.matmul(ps_d[:], lhT_l, rw_l, start=st,
                                         stop=(hp == 2), perf_mode=PM.DoubleRow)
                    if has_semb:
                        nc.tensor.matmul(ps_a[:], ones_f[:], semb_sb[:],
                                         start=False, stop=True)
                    # combine via SBUF (only one PSUM read per instruction):
                    # sem = 2^-15*ps_a + 2^-20*ps_b + 2^-19*ps_c + 2^-24*ps_d
                    sa = sxp.tile([128, NC30], F32, tag="sa", bufs=3,
                                  name=f"sa_{slot}")
                    nc.scalar.activation(sa[:], ps_a[:], AF.Copy, scale=Z1INV)
                    nc.vector.scalar_tensor_tensor(
                        sa[:], ps_b[:], Z1INV / 32, sa[:],
                        op0=ALU.mult, op1=ALU.add)
                    nc.vector.scalar_tensor_tensor(
                        sa[:], ps_c[:], Z1INV / 16, sa[:],
                        op0=ALU.mult, op1=ALU.add)
                    nc.vector.scalar_tensor_tensor(
                        sem_own[:, slot * NC30:(slot + 1) * NC30],
                        ps_d[:], Z1INV / 512, sa[:],
                        op0=ALU.mult, op1=ALU.add)

            # ---------------- squash over n (fp32, as baseline) ------------
            sem2 = smp.tile([128, NSEM * NC30], F32)
            nc.vector.tensor_tensor(sem2[:], sem_own[:], sem_own[:], op=ALU.mult)
            sqt = smp.tile([128, NSEM * C], F32)
            nc.vector.tensor_reduce(
                sqt[:].rearrange("p (slot cc) -> p slot cc", cc=C),
                sem2[:].rearrange("p (slot n cc) -> p slot cc n", n=N, cc=C),
                axis=mybir.AxisListType.X, op=ALU.add)
            lnq = smp.tile([128, NSEM * C], F32)
            nc.scalar.activation(lnq[:], sqt[:], AF.Ln)
            sqq = smp.tile([128, NSEM * C], F32)
            nc.scalar.activation(sqq[:], lnq[:], AF.Exp, scale=0.5)
            up = smp.tile([128, NSEM * C], F32)
            nc.vector.tensor_scalar_add(up[:], sqt[:], 1.0)
            ru = smp.tile([128, NSEM * C], F32)
            nc.vector.reciprocal(ru[:], up[:])
            fq = smp.tile([128, NSEM * C], F32)
            nc.vector.tensor_tensor(fq[:], sqq[:], ru[:], op=ALU.mult)
            sem_sq = sem2  # reuse
            f_ap = fq[:]
            f_b = bass.AP(f_ap.tensor, f_ap.offset,
                          [f_ap.ap[0], [C, NSEM], [0, N], [1, C]])
            nc.vector.tensor_tensor(
                sem_sq[:].rearrange("p (slot n cc) -> p slot n cc", n=N, cc=C),
                sem_own[:].rearrange("p (slot n cc) -> p slot n cc", n=N, cc=C),
                f_b, op=ALU.mult)
            # pair-ordered bf16 copy: block p=3i+u <- slot i+u
            sem_pair = smp.tile([128, NPAIR * NC30], BF)
            sq_ap = sem_sq[:]
            gather = bass.AP(sq_ap.tensor, sq_ap.offset,
                             [sq_ap.ap[0], [NC30, BL], [NC30, C], [1, NC30]])
            nc.vector.tensor_copy(
                sem_pair[:].rearrange("p (i u nc) -> p i u nc", i=BL, u=C),
                gather)

            # ---------------- phase 2: priors (bf16) -----------------
            sem_v = sem_pair[:].rearrange("p (pair nc) -> p nc pair", nc=NC30)
            priors = rp.tile([NPAIR, act_n * S], F32)
            for g in range(C):
                for n in range(act_n):
                    ps = pp.tile([NPAIR, S], F32, tag="mm", name=f"ps_pr_{g}_{n}")
                    rwt = sxp.tile([128, C * S], BF, tag="rwt", bufs=4,
                                   name=f"rw_{g}_{n}")
                    nc.sync.dma_start(
                        rwt[:], d_rw.ap()[g][:, (n * C) * S:(n * C + C) * S])
                    for cc in range(C):
                        nc.tensor.matmul(
                            ps[:], sem_v[:, n * C + cc, :],
                            rwt[:, cc * S:(cc + 1) * S],
                            start=(cc == 0), stop=(cc == C - 1))
                    dst = priors[:, n * S:(n + 1) * S]
                    if g == 0:
                        nc.vector.tensor_scalar_mul(dst, ps[:], masks_sb[:, 0:1])
                    else:
                        nc.vector.scalar_tensor_tensor(
                            dst, ps[:], masks_sb[:, g:g + 1], dst,
                            op0=ALU.mult, op1=ALU.add)

            # ---------------- phase 3: routing (fp32, as baseline) ---------
            vote = rp.tile([NPAIR, S], F32)
            scr = rp.tile([NPAIR, S], F32)
            La = rp.tile([NPAIR, act_n], F32)
            Lb = rp.tile([NPAIR, act_n], F32)
            sqv = rp.tile([NPAIR, 1], F32)
            lv = rp.tile([NPAIR, 1], F32)
            sv = rp.tile([NPAIR, 1], F32)
            uv = rp.tile([NPAIR, 1], F32)
            rv = rp.tile([NPAIR, 1], F32)
            fv = rp.tile([NPAIR, 1], F32)
            outv = rp.tile([NPAIR, S], F32)
            mx = rp.tile([NPAIR, 1], F32)
            mneg = rp.tile([NPAIR, 1], F32)
            ex = rp.tile([NPAIR, act_n], F32)
            es = rp.tile([NPAIR, 1], F32)
            ers = rp.tile([NPAIR, 1], F32)
            probs = rp.tile([NPAIR, act_n], F32)

            def vote_from(pr_sc, first_const=None):
                for n in range(act_n):
                    blk = priors[:, n * S:(n + 1) * S]
                    sc = first_const if first_const is not None else pr_sc[:, n:n + 1]
                    if n == 0:
                        nc.vector.tensor_scalar_mul(vote[:], blk, sc)
                    else:
                        nc.vector.scalar_tensor_tensor(
                            vote[:], blk, sc, vote[:], op0=ALU.mult, op1=ALU.add)

            def squash_vote():
                nc.vector.tensor_tensor(scr[:], vote[:], vote[:], op=ALU.mult)
                nc.vector.tensor_reduce(sqv[:], scr[:],
                                        axis=mybir.AxisListType.X, op=ALU.add)
                nc.scalar.activation(lv[:], sqv[:], AF.Ln)
                nc.scalar.activation(sv[:], lv[:], AF.Exp, scale=0.5)
                nc.vector.tensor_scalar_add(uv[:], sqv[:], 1.0)
                nc.vector.reciprocal(rv[:], uv[:])
                nc.vector.tensor_tensor(fv[:], sv[:], rv[:], op=ALU.mult)
                nc.vector.tensor_scalar_mul(outv[:], vote[:], fv[:])

            def deltas(Lprev, Lnew, first):
                for n in range(act_n):
                    nc.vector.tensor_tensor(
                        scr[:], priors[:, n * S:(n + 1) * S], outv[:], op=ALU.mult)
                    nc.vector.tensor_reduce(
                        Lnew[:, n:n + 1], scr[:],
                        axis=mybir.AxisListType.X, op=ALU.add)
                if not first:
                    nc.vector.tensor_tensor(Lnew[:], Lnew[:], Lprev[:], op=ALU.add)

            def softmax(L):
                nc.vector.tensor_reduce(mx[:], L[:], axis=mybir.AxisListType.X,
                                        op=ALU.max)
                nc.vector.tensor_scalar_mul(mneg[:], mx[:], -1.0)
                nc.scalar.activation(ex[:], L[:], AF.Exp, bias=mneg[:],
                                     accum_out=es[:])
                nc.vector.reciprocal(ers[:], es[:])
                nc.vector.tensor_scalar_mul(probs[:], ex[:], ers[:])

            vote_from(None, first_const=1.0 / act_n)
            squash_vote()
            deltas(None, La, first=True)
            softmax(La)
            vote_from(probs)
            squash_vote()
            deltas(La, Lb, first=False)
            softmax(Lb)
            vote_from(probs)

            # vote -> fp8 (*8) -> DRAM -> gather VT[c, e*128+s]
            vb = rp.tile([NPAIR, S], F8)
            nc.scalar.activation(vb[:], vote[:], AF.Copy, scale=SVT)
            nc.sync.dma_start(
                d_vcb.ap().rearrange("(p s) -> p s", p=NPAIR), vb[:])
            vflat = d_vcb.ap()
            for e in range(BL):
                src = bass.AP(vflat.tensor, vflat.offset + 3 * e * S,
                              [[1, C], [C, S]])
                vdst = bass.AP(vt_sb[:].tensor, vt_sb[:].offset + e * S,
                               [[vt_sb[:].ap[0][0], C], [1, S]])
                nc.sync.dma_start(vdst, src)

            # ---------------- phase 4: fc1 (fp8 DR, single psum group) -----
            z1p = ap_.tile([128, NPRE * AK * 512], BF)
            h8 = ap_.tile([128, AK * TOK], F8)

            def fc1_x_streams(ps, ak, col, last_stop):
                """9 DR matmuls: xh*w1h + xl*w1h + xh*w1l into one group."""
                for hp in range(HK // 2):
                    woff = (2 * hp) * A + ak * 128
                    xoff = (2 * hp) * TOK + col
                    lh_h = pair_ap(w1h_sb[:], woff, A, 128)
                    lh_l = pair_ap(w1l_sb[:], woff, A, 128)
                    rh_h = pair_ap(xh_sb[:], xoff, TOK, 512)
                    rh_l = pair_ap(xl_sb[:], xoff, TOK, 512)
                    st = hp == 0
                    nc.tensor.matmul(ps[:], lh_h, rh_h, start=st, stop=False,
                                     perf_mode=PM.DoubleRow)
                    nc.tensor.matmul(ps[:], lh_h, rh_l, start=False, stop=False,
                                     perf_mode=PM.DoubleRow)
                    nc.tensor.matmul(ps[:], lh_l, rh_h, start=False,
                                     stop=(hp == 2 and last_stop),
                                     perf_mode=PM.DoubleRow)

            def m1_step(ps, ak, col, start):
                lh = pair_ap(m1_sb[:], ak * 128, A, 128)
                rh = pair_ap(vt_sb[:], col, TOK, 512)
                nc.tensor.matmul(ps[:], lh, rh, start=start, stop=True,
                                 perf_mode=PM.DoubleRow)

            # prepass (x-only) for col chunks 0..NPRE-1
            for cc in range(NPRE):
                for ak in range(AK):
                    ps = pp.tile([128, 512], F32, tag="mm", name=f"ps_f1p_{cc}_{ak}")
                    fc1_x_streams(ps, ak, cc * 512, last_stop=True)
                    nc.scalar.copy(
                        z1p[:, (cc * AK + ak) * 512:(cc * AK + ak + 1) * 512],
                        ps[:])

            def fc2_chunk(tt):
                """fc2 for token chunk tt (128 tokens): h8 x (w2h+w2l)."""
                e = tt
                psa = pp.tile([128, 512], F32, tag="mm", name=f"ps_f2a_{tt}")
                psb = pp.tile([128, 256], F32, tag="mm", name=f"ps_f2b_{tt}")
                for w_sb, first in ((w2h_sb, True), (w2l_sb, False)):
                    for ap8 in range(AK // 2):
                        woff = (2 * ap8) * H
                        hoff = (2 * ap8) * TOK + tt * 128
                        lh = pair_ap(h8[:], hoff, TOK, 128)
                        st = first and ap8 == 0
                        sp_ = (not first) and ap8 == AK // 2 - 1
                        nc.tensor.matmul(psa[:], lh, pair_ap(w_sb[:], woff, H, 512),
                                         start=st, stop=(sp_ and not has_b2),
                                         perf_mode=PM.DoubleRow)
                        nc.tensor.matmul(psb[:], lh,
                                         pair_ap(w_sb[:], woff + 512, H, 256),
                                         start=st, stop=(sp_ and not has_b2),
                                         perf_mode=PM.DoubleRow)
                if has_b2:
                    nc.tensor.matmul(psa[:], ones_bf[:], b2_sb[:, 0:512],
                                     start=False, stop=True)
                    nc.tensor.matmul(psb[:], ones_bf[:], b2_sb[:, 512:H],
                                     start=False, stop=True)
                xt = sp.tile([128, H], F32, tag="x", name=f"x_{tt}", bufs=3)
                nc.sync.dma_start(xt[:], d_xown[e])
                ot = sp.tile([128, H], F32, tag="o", name=f"o_{tt}", bufs=3)
                nc.scalar.activation(ot[:, 0:512], psa[:], AF.Relu)
                nc.scalar.activation(ot[:, 512:H], psb[:], AF.Relu)
                nc.vector.tensor_tensor(ot[:], ot[:], g2rep[:], op=ALU.mult)
                nc.gpsimd.tensor_tensor(ot[:], ot[:], xt[:], op=ALU.add)
                nc.sync.dma_start(d_out[e], ot[:])

            # catch-up (m1 + z1p -> h8) then fc2 per 512-col chunk
            for cc in range(NPRE):
                for ak in range(AK):
                    ps = pp.tile([128, 512], F32, tag="mm", name=f"ps_f1m_{cc}_{ak}")
                    m1_step(ps, ak, cc * 512, start=True)
                    nc.vector.tensor_tensor(
                        ps[:], ps[:],
                        z1p[:, (cc * AK + ak) * 512:(cc * AK + ak + 1) * 512],
                        op=ALU.add)
                    nc.scalar.activation(
                        h8[:, ak * TOK + cc * 512: ak * TOK + (cc + 1) * 512],
                        ps[:], AF.Relu, scale=H8SC, bias=b1_sb[:, ak:ak + 1])
                for tt in range(4 * cc, 4 * cc + 4):
                    fc2_chunk(tt)
            # fused blocks for col chunks NPRE..3
            for cc in range(NPRE, 4):
                for ak in range(AK):
                    ps = pp.tile([128, 512], F32, tag="mm", name=f"ps_f1f_{cc}_{ak}")
                    fc1_x_streams(ps, ak, cc * 512, last_stop=False)
                    m1_step(ps, ak, cc * 512, start=False)
                    nc.scalar.activation(
                        h8[:, ak * TOK + cc * 512: ak * TOK + (cc + 1) * 512],
                        ps[:], AF.Relu, scale=H8SC, bias=b1_sb[:, ak:ak + 1])
                for tt in range(4 * cc, 4 * cc + 4):
                    fc2_chunk(tt)

    nc.compile()
    return nc


# ---------------------------------------------------------------------------
# host marshaling
# ---------------------------------------------------------------------------

def _prep_core_inputs(k, xh_t, xl_t, x, shared):
    # own examples b_i = (48k + 43 i) mod 128; sem examples [16k, 16k+18).
    own = np.array([(48 * k + 43 * i) % B for i in range(BL)])
    sem_ex = np.array([(16 * k + j) % B for j in range(NSEM)])

    # xsem: [6, hk, 128, 3*S] fp8 hi/lo from the pre-transposed full tensors
    def slab(xt):
        xs = xt[:, sem_ex, :].reshape(H, NSEM * S)
        o = np.empty((6, HK, 128, 3 * S), E4)
        for g in range(6):
            for hk in range(HK):
                o[g, hk] = xs[hk * 128:(hk + 1) * 128, g * 3 * S:(g + 1) * 3 * S]
        return o

    xsem_h = slab(xh_t)
    xl_sem = xl_t  # lo at sem scale: recompute below
    xsem_l = slab(shared["_xl_sem_t"])

    xq_h = np.ascontiguousarray(
        xh_t[:, own, :].reshape(H, TOK).reshape(HK, 128, TOK))
    xq_l = np.ascontiguousarray(
        xl_t[:, own, :].reshape(H, TOK).reshape(HK, 128, TOK))
    x_own = np.ascontiguousarray(x[own].astype(np.float32))

    masks = np.zeros((NPAIR, C), np.float32)
    for i in range(BL):
        for u in range(C):
            t = 3 * int(own[i]) + u
            masks[3 * i + u, t // B] = 1.0

    core = {
        "xsem_h": xsem_h, "xsem_l": xsem_l,
        "xq_h": xq_h, "xq_l": xq_l, "x_own": x_own, "masks": masks,
    }
    core.update({n: shared[n] for n in (
        "swh", "swl", "semb", "rw_pack", "w1h", "w1l", "w2h", "w2l",
        "m1q", "b1s", "g2s", "b2row")})
    return core


_CACHE = {}


def _make_shared(inputs):
    fc1_w = np.asarray(inputs["fc1_w"], np.float32)
    fc1_b = np.asarray(inputs["fc1_b"], np.float32)
    fc2_w = np.asarray(inputs["fc2_w"], np.float32)
    fc2_b = np.asarray(inputs["fc2_b"], np.float32)
    efc1 = np.asarray(inputs["efc1"], np.float32)
    efc2 = np.asarray(inputs["efc2"], np.float32)
    sem_w = np.asarray(inputs["sem_w"], np.float32)
    sem_b = np.asarray(inputs["sem_b"], np.float32)
    route_weights = np.asarray(inputs["route_weights"], np.float32)
    larger_w = np.asarray(inputs["larger_w"], np.float32)
    larger_b = np.asarray(inputs["larger_b"], np.float32)
    elarger = np.asarray(inputs["elarger"], np.float32)
    t = int(np.asarray(inputs["t"]))
    sf = np.float32(int(np.asarray(inputs["s"])))
    act_n = t + 1
    x = np.asarray(inputs["x"], np.float32)

    gfc1 = _sigmoid_f32(sf * efc1[t])
    gfc2 = _sigmoid_f32(sf * efc2[t])
    glarger = _sigmoid_f32(sf * elarger[t])

    lwg = (larger_w * glarger[None, :]).astype(np.float32)
    m1mat = (lwg @ fc1_w).astype(np.float32)
    b1 = ((larger_b * glarger) @ fc1_w + fc1_b).astype(np.float32)
    w2g = (fc2_w * gfc1[:, None]).astype(np.float32)
    rw4 = route_weights.reshape(C, N, S, C, S)

    # transposed x quantization (shared across cores; slabs pick columns)
    xt = np.ascontiguousarray(np.transpose(x, (2, 0, 1)))      # [H, B, S]
    xh_t = _q8(xt, SXH).reshape(H, B, S)
    xl_res = xt - xh_t.astype(np.float32) / SXH
    xl_t = _q8(xl_res, SXH).reshape(H, B, S)                   # adapter lo
    xl_sem_t = _q8(xl_res, SXSL).reshape(H, B, S)              # sem lo (finer)

    swh = _q8(np.transpose(sem_w, (1, 0, 2)).reshape(H, NC30), SSW_HI := SW)
    swl = _q8(np.transpose(sem_w, (1, 0, 2)).reshape(H, NC30)
              - swh.astype(np.float32) / SW, SSWL)
    w1h = _q8(fc1_w, SW)
    w1l = _q8(fc1_w - w1h.astype(np.float32) / SW, SW)
    w2h = _q8(w2g, SW)
    w2l = _q8(w2g - w2h.astype(np.float32) / SW, SW)

    shared = {
        "swh": swh.reshape(HK, 128, NC30),
        "swl": swl.reshape(HK, 128, NC30),
        "semb": np.ascontiguousarray(
            (sem_b.reshape(1, NC30) * 32768.0).astype(np.float32)),
        "rw_pack": np.stack([
            np.ascontiguousarray(np.transpose(rw4[c, :act_n], (1, 0, 2, 3))
                                 .reshape(S, act_n * C * S).astype(BF16))
            for c in range(C)]),
        "w1h": w1h.reshape(HK, 128, A),
        "w1l": w1l.reshape(HK, 128, A),
        "w2h": w2h.reshape(AK, 128, H),
        "w2l": w2l.reshape(AK, 128, H),
        "m1q": _q8(m1mat, SM1),
        "b1s": np.ascontiguousarray(
            (b1 * SH8).astype(np.float32).reshape(AK, 128).T),
        "g2s": np.ascontiguousarray((gfc2 * Z1INV).reshape(1, H)),
        "b2row": _bf(fc2_b.reshape(1, H)),
        "_xh_t": xh_t, "_xl_t": xl_t, "_xl_sem_t": xl_sem_t,
        "_has_semb": bool(np.any(sem_b)), "_has_b2": bool(np.any(fc2_b)),
    }
    return shared, act_n


def kernel(**inputs):
    x = np.asarray(inputs["x"], np.float32)
    shared, act_n = _make_shared(inputs)
    key = (act_n, shared["_has_semb"], shared["_has_b2"])
    if key not in _CACHE:
        _CACHE[key] = _build_program(act_n, shared["_has_semb"], shared["_has_b2"])
    nc = _CACHE[key]

    in_maps = [_prep_core_inputs(k, shared["_xh_t"], shared["_xl_t"], x, shared)
               for k in range(NCORES)]
    res = bass_utils.run_bass_kernel_spmd(nc, in_maps, core_ids=list(range(NCORES)))
    out = np.empty((B, S, H), np.float32)
    for k in range(NCORES):
        own = [(48 * k + 43 * i) % B for i in range(BL)]
        out[own] = res.results[k]["out"]
    return out


# revision 9
# speedup vs baseline: 1.4992x; 1.1473x over previous
"""Trainium2 Bass kernel for nn_BertAdapterCapsuleMask (fp8 DoubleRow version).

Strategy (8 NeuronCores, SPMD — identical program, per-core data):

Sharding (same as proven baseline): core k owns routing pairs
t in [48k, 48k+48) and output examples b_i = (48k + 43 i) mod 128, so the
pairs' sem examples are exactly the 18 consecutive [16k, 16k+18) and the
vote rows each core produces are exactly the ones its own examples'
adapter needs — zero cross-core traffic.

Speed: the adapter GEMMs (fc1 768x2048, fc2 2048x768 per 2048 tokens)
run in fp8-e4m3 with MatmulPerfMode.DoubleRow (256-deep contraction,
half-cycle per output column).  Accuracy is restored with residual
("lo") streams quantized at the SAME dequant scale as the hi streams —
fp8's relative precision is scale-invariant, so all streams of one GEMM
accumulate into a single PSUM group with zero combine cost:

  fc1 psum  = xh*w1h + xl*w1h + xh*w1l + vt8*m1q      (all at 2^15)
  fc2 psum  = h8*w2h + h8*w2l                          (at 2^15)

The capsule path keeps near-fp32 accuracy (routing softmax amplifies
sem errors ~10x): sem runs as four fp8 streams with fine residual
scales (separate psums, combines staged through SBUF), squash/routing
stay fp32, priors run bf16.  The capsule term enters fc1 via the
vt8*m1q rank-3 DoubleRow update, so h_out never materializes.

DMA discipline: every DMA instruction costs ~625ns on the single shared
HWDGE descriptor engine, so all bulk tensors move as ONE instruction
each (3D access patterns); the VT gather is a single 3D
gather-descriptor DMA; x/out staging batches 4 examples per DMA.
"""

import numpy as np
import ml_dtypes

import concourse.bass as bass
import concourse.bacc as bacc
import concourse.mybir as mybir
import concourse.tile as tile
from concourse import bass_utils

E4 = ml_dtypes.float8_e4m3
BF16 = ml_dtypes.bfloat16
F32 = mybir.dt.float32
BF = mybir.dt.bfloat16
F8 = mybir.dt.float8e4
AF = mybir.ActivationFunctionType
ALU = mybir.AluOpType
PM = mybir.MatmulPerfMode

B, S, H, A, C, N = 128, 128, 768, 2048, 3, 10
NUM_ITER = 3
NCORES = 8
BL = B // NCORES          # 16 own examples / core
NPAIR = 3 * B // NCORES   # 48 routing pairs / core
HK = H // 128             # 6
AK = A // 128             # 16
TOK = BL * S              # 2048 tokens / core
NC30 = N * C              # 30
NSEM = 18                 # sem examples per core: [16k, 16k+18) mod 128
STOK = NSEM * S           # 2304 sem tokens / core
NPRE = 1                  # fc1 col-chunks (of 512 tokens) via z1p prepass

# fp8 scales.  All adapter streams share dequant 2^-15; sem streams use
# fine residual scales (routing is sensitive) and combine explicitly.
SXH = 16.0
SW = 2048.0
SVT, SM1 = 8.0, 4096.0         # SVT*SM1 == SXH*SW == 2^15
SH8 = 16.0
SXSL = 512.0                   # xsem lo
SSWL = 32768.0                 # sem_w lo
Z1INV = 1.0 / 32768.0          # 2^-15
H8SC = SH8 * Z1INV             # 2^-11


def _sigmoid_f32(z):
    z = np.asarray(z, np.float64)
    return (1.0 / (1.0 + np.exp(-z))).astype(np.float32)


def _q8(a, scale):
    """fp8-e4m3 quantize: stores clip(a*scale, +-240); dequant is 1/scale."""
    z = np.clip(np.asarray(a, np.float32) * np.float32(scale), -240.0, 240.0)
    return np.ascontiguousarray(z.astype(E4))


def _bf(x):
    return np.ascontiguousarray(np.asarray(x, np.float32).astype(BF16))


# ---------------------------------------------------------------------------
# device program
# ---------------------------------------------------------------------------

def _build_program(act_n, has_semb, has_b2):
    nc = bacc.Bacc("TRN2", target_bir_lowering=False, debug=False,
                   num_devices=NCORES)

    d_xsh = nc.dram_tensor("xsem_h", [HK, 128, STOK], F8, kind="ExternalInput")
    d_xsl = nc.dram_tensor("xsem_l", [HK, 128, STOK], F8, kind="ExternalInput")
    d_swh = nc.dram_tensor("swh", [HK, 128, NC30], F8, kind="ExternalInput")
    d_swl = nc.dram_tensor("swl", [HK, 128, NC30], F8, kind="ExternalInput")
    d_semb = nc.dram_tensor("semb", [1, NC30], F32, kind="ExternalInput")
    d_rw = nc.dram_tensor("rw_pack", [C, 128, act_n * C * S], BF, kind="ExternalInput")
    d_masks = nc.dram_tensor("masks", [NPAIR, C], F32, kind="ExternalInput")
    d_xh = nc.dram_tensor("xq_h", [HK, 128, TOK], F8, kind="ExternalInput")
    d_xl = nc.dram_tensor("xq_l", [HK, 128, TOK], F8, kind="ExternalInput")
    d_w1h = nc.dram_tensor("w1h", [HK, 128, A], F8, kind="ExternalInput")
    d_w1l = nc.dram_tensor("w1l", [HK, 128, A], F8, kind="ExternalInput")
    d_w2h = nc.dram_tensor("w2h", [AK, 128, H], F8, kind="ExternalInput")
    d_w2l = nc.dram_tensor("w2l", [AK, 128, H], F8, kind="ExternalInput")
    d_m1q = nc.dram_tensor("m1q", [C, A], F8, kind="ExternalInput")
    d_b1s = nc.dram_tensor("b1s", [128, AK], F32, kind="ExternalInput")
    d_g2s = nc.dram_tensor("g2s", [1, H], F32, kind="ExternalInput")
    d_b2 = nc.dram_tensor("b2row", [1, H], BF, kind="ExternalInput")
    d_xown = nc.dram_tensor("x_own", [BL, S, H], F32, kind="ExternalInput")
    d_vcb = nc.dram_tensor("votecb", [NPAIR * S], F8, kind="Internal")
    d_out = nc.dram_tensor("out", [BL, S, H], F32, kind="ExternalOutput")

    def pair_ap(ap, off, stride, n):
        """[[part], [stride, 2], [1, n]] AP at ap.offset+off (DoubleRow pair)."""
        return bass.AP(ap.tensor, ap.offset + off, [ap.ap[0], [stride, 2], [1, n]])

    def merged_load(queue, dst_tile, dram, inner):
        """One-DMA load of [K, 128, inner] dram into [128, K*inner] sbuf."""
        k = dram.shape[0]
        queue.dma_start(
            dst_tile[:].rearrange("p (k c) -> p k c", k=k),
            dram.ap().rearrange("k p c -> p k c"))

    with tile.TileContext(nc) as tc:
        with (
            tc.tile_pool(name="w", bufs=1) as wp,
            tc.tile_pool(name="semx", bufs=1) as sxp,
            tc.tile_pool(name="sem", bufs=1) as smp,
            tc.tile_pool(name="rt", bufs=1) as rp,
            tc.tile_pool(name="ad", bufs=1) as ap_,
            tc.tile_pool(name="st", bufs=2) as sp,
            tc.tile_pool(name="ps", bufs=8, space="PSUM") as pp,
        ):
            # ---------------- persistent loads (one DMA each) --------------
            # SP queue: capsule-critical data first
            xsh_sb = sxp.tile([128, HK * STOK], F8)
            xsl_sb = sxp.tile([128, HK * STOK], F8)
            merged_load(nc.sync, xsh_sb, d_xsh, STOK)
            merged_load(nc.sync, xsl_sb, d_xsl, STOK)
            swh_sb = wp.tile([128, HK * NC30], F8)
            swl_sb = wp.tile([128, HK * NC30], F8)
            merged_load(nc.sync, swh_sb, d_swh, NC30)
            merged_load(nc.sync, swl_sb, d_swl, NC30)
            semb_sb = wp.tile([1, NC30], F32)
            nc.sync.dma_start(semb_sb[:], d_semb[:])
            masks_sb = wp.tile([NPAIR, C], F32)
            nc.sync.dma_start(masks_sb[:], d_masks[:])
            rw_sb = sxp.tile([128, C * act_n * C * S], BF)
            merged_load(nc.sync, rw_sb, d_rw, act_n * C * S)

            # ACT queue: fc1 prepass data first
            xh_sb = wp.tile([128, HK * TOK], F8, tag="bigx", bufs=1)
            w1h_sb = wp.tile([128, HK * A], F8)
            merged_load(nc.scalar, xh_sb, d_xh, TOK)
            merged_load(nc.scalar, w1h_sb, d_w1h, A)
            xl_sb = wp.tile([128, HK * TOK], F8, tag="bigx2", bufs=1)
            w1l_sb = wp.tile([128, HK * A], F8)
            merged_load(nc.scalar, xl_sb, d_xl, TOK)
            merged_load(nc.scalar, w1l_sb, d_w1l, A)
            w2h_sb = wp.tile([128, AK * H], F8, tag="bigw2", bufs=1)
            w2l_sb = wp.tile([128, AK * H], F8, tag="bigw2l", bufs=1)
            merged_load(nc.scalar, w2h_sb, d_w2h, H)
            merged_load(nc.scalar, w2l_sb, d_w2l, H)

            # m1 lhsT [128, 2*A]: rows 0-2 of k-tile0 hold m1q, rest zero
            m1_sb = wp.tile([128, 2 * A], F8)
            nc.gpsimd.memset(m1_sb[:], 0.0)
            m1_dst = bass.AP(m1_sb[:].tensor, m1_sb[:].offset,
                             [[m1_sb[:].ap[0][0], C], [1, A]])
            nc.scalar.dma_start(m1_dst, d_m1q.ap())
            # vt rhs [128, 2*TOK]: rows 0-2 of k-tile0 get the vote gather;
            # everything else zero (multiplied by zero m1 rows).
            vt_sb = ap_.tile([128, 2 * TOK], F8)
            nc.gpsimd.memset(vt_sb[:], 0.0)

            b1_sb = wp.tile([128, AK], F32)
            nc.scalar.dma_start(b1_sb[:], d_b1s[:])
            g2rep = wp.tile([128, H], F32)
            g2_src = d_g2s.ap()
            nc.scalar.dma_start(
                g2rep[:], bass.AP(g2_src.tensor, g2_src.offset, [[0, 128], [1, H]]))
            ones_f = wp.tile([1, 128], F32)
            nc.gpsimd.memset(ones_f[:], 1.0)
            if has_b2:
                ones_bf = wp.tile([1, 128], BF)
                nc.gpsimd.memset(ones_bf[:], 1.0)
                b2_sb = wp.tile([1, H], BF)
                nc.scalar.dma_start(b2_sb[:], d_b2[:])

            # ---------------- phase 1: sem (4 fp8 streams) -----------------
            sem_own = smp.tile([128, NSEM * NC30], F32)
            for slot in range(NSEM):
                ps_a = pp.tile([128, NC30], F32, tag="mm", name=f"ps_sa_{slot}")
                ps_b = pp.tile([128, NC30], F32, tag="mm", name=f"ps_sb_{slot}")
                ps_c = pp.tile([128, NC30], F32, tag="mm", name=f"ps_sc_{slot}")
                ps_d = pp.tile([128, NC30], F32, tag="mm", name=f"ps_sd_{slot}")
                for hp in range(HK // 2):
                    off = (2 * hp) * STOK + slot * S
                    lhT_h = pair_ap(xsh_sb[:], off, STOK, 128)
                    lhT_l = pair_ap(xsl_sb[:], off, STOK, 128)
                    woff = (2 * hp) * NC30
                    rw_h = pair_ap(swh_sb[:], woff, NC30, NC30)
                    rw_l = pair_ap(swl_sb[:], woff, NC30, NC30)
                    st = hp == 0
                    nc.tensor.matmul(ps_a[:], lhT_h, rw_h, start=st,
                                     stop=(hp == 2 and not has_semb),
                                     perf_mode=PM.DoubleRow)
                    nc.tensor.matmul(ps_b[:], lhT_l, rw_h, start=st,
                                     stop=(hp == 2), perf_mode=PM.DoubleRow)
                    nc.tensor.matmul(ps_c[:], lhT_h, rw_l, start=st,
                                     stop=(hp == 2), perf_mode=PM.DoubleRow)
                    nc.tensor.matmul(ps_d[:], lhT_l, rw_l, start=st,
                                     stop=(hp == 2), perf_mode=PM.DoubleRow)
                if has_semb:
                    nc.tensor.matmul(ps_a[:], ones_f[:], semb_sb[:],
                                     start=False, stop=True)
                # combine via SBUF (only one PSUM read per instruction):
                # sem = 2^-15*ps_a + 2^-20*ps_b + 2^-19*ps_c + 2^-24*ps_d
                sa = smp.tile([128, NC30], F32, tag="sa", bufs=3,
                              name=f"sa_{slot}")
                nc.scalar.activation(sa[:], ps_a[:], AF.Copy, scale=Z1INV)
                nc.vector.scalar_tensor_tensor(
                    sa[:], ps_b[:], Z1INV / 32, sa[:], op0=ALU.mult, op1=ALU.add)
                nc.vector.scalar_tensor_tensor(
                    sa[:], ps_c[:], Z1INV / 16, sa[:], op0=ALU.mult, op1=ALU.add)
                nc.vector.scalar_tensor_tensor(
                    sem_own[:, slot * NC30:(slot + 1) * NC30],
                    ps_d[:], Z1INV / 512, sa[:], op0=ALU.mult, op1=ALU.add)

            # ---------------- squash over n (fp32, as baseline) ------------
            sem2 = smp.tile([128, NSEM * NC30], F32)
            nc.vector.tensor_tensor(sem2[:], sem_own[:], sem_own[:], op=ALU.mult)
            sqt = smp.tile([128, NSEM * C], F32)
            nc.vector.tensor_reduce(
                sqt[:].rearrange("p (slot cc) -> p slot cc", cc=C),
                sem2[:].rearrange("p (slot n cc) -> p slot cc n", n=N, cc=C),
                axis=mybir.AxisListType.X, op=ALU.add)
            lnq = smp.tile([128, NSEM * C], F32)
            nc.scalar.activation(lnq[:], sqt[:], AF.Ln)
            sqq = smp.tile([128, NSEM * C], F32)
            nc.scalar.activation(sqq[:], lnq[:], AF.Exp, scale=0.5)
            up = smp.tile([128, NSEM * C], F32)
            nc.vector.tensor_scalar_add(up[:], sqt[:], 1.0)
            ru = smp.tile([128, NSEM * C], F32)
            nc.vector.reciprocal(ru[:], up[:])
            fq = smp.tile([128, NSEM * C], F32)
            nc.vector.tensor_tensor(fq[:], sqq[:], ru[:], op=ALU.mult)
            sem_sq = sem2  # reuse
            f_ap = fq[:]
            f_b = bass.AP(f_ap.tensor, f_ap.offset,
                          [f_ap.ap[0], [C, NSEM], [0, N], [1, C]])
            nc.vector.tensor_tensor(
                sem_sq[:].rearrange("p (slot n cc) -> p slot n cc", n=N, cc=C),
                sem_own[:].rearrange("p (slot n cc) -> p slot n cc", n=N, cc=C),
                f_b, op=ALU.mult)
            # pair-ordered bf16 copy: block p=3i+u <- slot i+u
            sem_pair = smp.tile([128, NPAIR * NC30], BF)
            sq_ap = sem_sq[:]
            gather = bass.AP(sq_ap.tensor, sq_ap.offset,
                             [sq_ap.ap[0], [NC30, BL], [NC30, C], [1, NC30]])
            nc.vector.tensor_copy(
                sem_pair[:].rearrange("p (i u nc) -> p i u nc", i=BL, u=C),
                gather)

            # ---------------- phase 2: priors (bf16) -----------------
            sem_v = sem_pair[:].rearrange("p (pair nc) -> p nc pair", nc=NC30)
            priors = rp.tile([NPAIR, act_n * S], F32)
            for g in range(C):
                for n in range(act_n):
                    ps = pp.tile([NPAIR, S], F32, tag="mm", name=f"ps_pr_{g}_{n}")
                    rbase = g * (act_n * C * S) + (n * C) * S
                    for cc in range(C):
                        nc.tensor.matmul(
                            ps[:], sem_v[:, n * C + cc, :],
                            rw_sb[:, rbase + cc * S: rbase + (cc + 1) * S],
                            start=(cc == 0), stop=(cc == C - 1))
                    dst = priors[:, n * S:(n + 1) * S]
                    if g == 0:
                        nc.vector.tensor_scalar_mul(dst, ps[:], masks_sb[:, 0:1])
                    else:
                        nc.vector.scalar_tensor_tensor(
                            dst, ps[:], masks_sb[:, g:g + 1], dst,
                            op0=ALU.mult, op1=ALU.add)

            # ---------------- phase 3: routing (fp32, as baseline) ---------
            vote = rp.tile([NPAIR, S], F32)
            scr = rp.tile([NPAIR, S], F32)
            La = rp.tile([NPAIR, act_n], F32)
            Lb = rp.tile([NPAIR, act_n], F32)
            sqv = rp.tile([NPAIR, 1], F32)
            lv = rp.tile([NPAIR, 1], F32)
            sv = rp.tile([NPAIR, 1], F32)
            uv = rp.tile([NPAIR, 1], F32)
            rv = rp.tile([NPAIR, 1], F32)
            fv = rp.tile([NPAIR, 1], F32)
            outv = rp.tile([NPAIR, S], F32)
            mx = rp.tile([NPAIR, 1], F32)
            mneg = rp.tile([NPAIR, 1], F32)
            ex = rp.tile([NPAIR, act_n], F32)
            es = rp.tile([NPAIR, 1], F32)
            ers = rp.tile([NPAIR, 1], F32)
            probs = rp.tile([NPAIR, act_n], F32)

            def vote_from(pr_sc, first_const=None):
                for n in range(act_n):
                    blk = priors[:, n * S:(n + 1) * S]
                    sc = first_const if first_const is not None else pr_sc[:, n:n + 1]
                    if n == 0:
                        nc.vector.tensor_scalar_mul(vote[:], blk, sc)
                    else:
                        nc.vector.scalar_tensor_tensor(
                            vote[:], blk, sc, vote[:], op0=ALU.mult, op1=ALU.add)

            def squash_vote():
                nc.vector.tensor_tensor(scr[:], vote[:], vote[:], op=ALU.mult)
                nc.vector.tensor_reduce(sqv[:], scr[:],
                                        axis=mybir.AxisListType.X, op=ALU.add)
                nc.scalar.activation(lv[:], sqv[:], AF.Ln)
                nc.scalar.activation(sv[:], lv[:], AF.Exp, scale=0.5)
                nc.vector.tensor_scalar_add(uv[:], sqv[:], 1.0)
                nc.vector.reciprocal(rv[:], uv[:])
                nc.vector.tensor_tensor(fv[:], sv[:], rv[:], op=ALU.mult)
                nc.vector.tensor_scalar_mul(outv[:], vote[:], fv[:])

            def deltas(Lprev, Lnew, first):
                for n in range(act_n):
                    nc.vector.tensor_tensor(
                        scr[:], priors[:, n * S:(n + 1) * S], outv[:], op=ALU.mult)
                    nc.vector.tensor_reduce(
                        Lnew[:, n:n + 1], scr[:],
                        axis=mybir.AxisListType.X, op=ALU.add)
                if not first:
                    nc.vector.tensor_tensor(Lnew[:], Lnew[:], Lprev[:], op=ALU.add)

            def softmax(L):
                nc.vector.tensor_reduce(mx[:], L[:], axis=mybir.AxisListType.X,
                                        op=ALU.max)
                nc.vector.tensor_scalar_mul(mneg[:], mx[:], -1.0)
                nc.scalar.activation(ex[:], L[:], AF.Exp, bias=mneg[:],
                                     accum_out=es[:])
                nc.vector.reciprocal(ers[:], es[:])
                nc.vector.tensor_scalar_mul(probs[:], ex[:], ers[:])

            vote_from(None, first_const=1.0 / act_n)
            squash_vote()
            deltas(None, La, first=True)
            softmax(La)
            vote_from(probs)
            squash_vote()
            deltas(La, Lb, first=False)
            softmax(Lb)
            vote_from(probs)

            # vote -> fp8 (*8) -> DRAM -> one gather DMA:
            # vt[u, e*128+s] = votecb[e*384 + 3*s + u]
            vb = rp.tile([NPAIR, S], F8)
            nc.scalar.activation(vb[:], vote[:], AF.Copy, scale=SVT)
            nc.sync.dma_start(
                d_vcb.ap().rearrange("(p s) -> p s", p=NPAIR), vb[:])
            vflat = d_vcb.ap()
            vsrc = bass.AP(vflat.tensor, vflat.offset,
                           [[1, C], [C * S, BL], [C, S]])
            vdst = bass.AP(vt_sb[:].tensor, vt_sb[:].offset,
                           [[vt_sb[:].ap[0][0], C], [S, BL], [1, S]])
            nc.sync.dma_start(vdst, vsrc)

            # ---------------- phase 4: fc1 (fp8 DR, single psum group) -----
            z1p = ap_.tile([128, NPRE * AK * 512], BF)
            h8 = ap_.tile([128, AK * TOK], F8)

            def fc1_x_streams(ps, ak, col, last_stop):
                """9 DR matmuls: xh*w1h + xl*w1h + xh*w1l into one group."""
                for hp in range(HK // 2):
                    woff = (2 * hp) * A + ak * 128
                    xoff = (2 * hp) * TOK + col
                    lh_h = pair_ap(w1h_sb[:], woff, A, 128)
                    lh_l = pair_ap(w1l_sb[:], woff, A, 128)
                    rh_h = pair_ap(xh_sb[:], xoff, TOK, 512)
                    rh_l = pair_ap(xl_sb[:], xoff, TOK, 512)
                    st = hp == 0
                    nc.tensor.matmul(ps[:], lh_h, rh_h, start=st, stop=False,
                                     perf_mode=PM.DoubleRow)
                    nc.tensor.matmul(ps[:], lh_h, rh_l, start=False, stop=False,
                                     perf_mode=PM.DoubleRow)
                    nc.tensor.matmul(ps[:], lh_l, rh_h, start=False,
                                     stop=(hp == 2 and last_stop),
                                     perf_mode=PM.DoubleRow)

            def m1_step(ps, ak, col, start):
                lh = pair_ap(m1_sb[:], ak * 128, A, 128)
                rh = pair_ap(vt_sb[:], col, TOK, 512)
                nc.tensor.matmul(ps[:], lh, rh, start=start, stop=True,
                                 perf_mode=PM.DoubleRow)

            # prepass (x-only) for col chunks 0..NPRE-1
            for cc in range(NPRE):
                for ak in range(AK):
                    ps = pp.tile([128, 512], F32, tag="mm", name=f"ps_f1p_{cc}_{ak}")
                    fc1_x_streams(ps, ak, cc * 512, last_stop=True)
                    nc.scalar.copy(
                        z1p[:, (cc * AK + ak) * 512:(cc * AK + ak + 1) * 512],
                        ps[:])

            def fc2_chunk(tt, xt, ot, j):
                """fc2 for token chunk tt (=example tt): h8 x (w2h+w2l)."""
                psa = pp.tile([128, 512], F32, tag="mm", name=f"ps_f2a_{tt}")
                psb = pp.tile([128, 256], F32, tag="mm", name=f"ps_f2b_{tt}")
                for w_sb, first in ((w2h_sb, True), (w2l_sb, False)):
                    for ap8 in range(AK // 2):
                        woff = (2 * ap8) * H
                        hoff = (2 * ap8) * TOK + tt * 128
                        lh = pair_ap(h8[:], hoff, TOK, 128)
                        st = first and ap8 == 0
                        sp_ = (not first) and ap8 == AK // 2 - 1
                        nc.tensor.matmul(psa[:], lh, pair_ap(w_sb[:], woff, H, 512),
                                         start=st, stop=(sp_ and not has_b2),
                                         perf_mode=PM.DoubleRow)
                        nc.tensor.matmul(psb[:], lh,
                                         pair_ap(w_sb[:], woff + 512, H, 256),
                                         start=st, stop=(sp_ and not has_b2),
                                         perf_mode=PM.DoubleRow)
                if has_b2:
                    nc.tensor.matmul(psa[:], ones_bf[:], b2_sb[:, 0:512],
                                     start=False, stop=True)
                    nc.tensor.matmul(psb[:], ones_bf[:], b2_sb[:, 512:H],
                                     start=False, stop=True)
                osl = ot[:, j * H:(j + 1) * H]
                nc.scalar.activation(osl[:, 0:512], psa[:], AF.Relu)
                nc.scalar.activation(osl[:, 512:H], psb[:], AF.Relu)
                nc.vector.tensor_tensor(osl, osl, g2rep[:], op=ALU.mult)
                nc.gpsimd.tensor_tensor(osl, osl, xt[:, j * H:(j + 1) * H],
                                        op=ALU.add)

            def fc2_group(gg):
                """2 fc2 chunks + batched x-in / out DMAs (1 each)."""
                xt = sp.tile([128, 2 * H], F32, tag="x", name=f"x_{gg}", bufs=1)
                nc.sync.dma_start(
                    xt[:].rearrange("p (e h) -> p e h", e=2),
                    d_xown.ap()[2 * gg:2 * gg + 2].rearrange("e p h -> p e h"))
                ot = sp.tile([128, 2 * H], F32, tag="o", name=f"o_{gg}", bufs=2)
                for j in range(2):
                    fc2_chunk(2 * gg + j, xt, ot, j)
                nc.sync.dma_start(
                    d_out.ap()[2 * gg:2 * gg + 2].rearrange("e p h -> p e h"),
                    ot[:].rearrange("p (e h) -> p e h", e=2))

            # catch-up (m1 + z1p -> h8) then fc2 per 512-col chunk
            for cc in range(NPRE):
                for ak in range(AK):
                    ps = pp.tile([128, 512], F32, tag="mm", name=f"ps_f1m_{cc}_{ak}")
                    m1_step(ps, ak, cc * 512, start=True)
                    nc.vector.tensor_tensor(
                        ps[:], ps[:],
                        z1p[:, (cc * AK + ak) * 512:(cc * AK + ak + 1) * 512],
                        op=ALU.add)
                    nc.scalar.activation(
                        h8[:, ak * TOK + cc * 512: ak * TOK + (cc + 1) * 512],
                        ps[:], AF.Relu, scale=H8SC, bias=b1_sb[:, ak:ak + 1])
                fc2_group(2 * cc)
                fc2_group(2 * cc + 1)
            # fused blocks for col chunks NPRE..3
            for cc in range(NPRE, 4):
                for ak in range(AK):
                    ps = pp.tile([128, 512], F32, tag="mm", name=f"ps_f1f_{cc}_{ak}")
                    fc1_x_streams(ps, ak, cc * 512, last_stop=False)
                    m1_step(ps, ak, cc * 512, start=False)
                    nc.scalar.activation(
                        h8[:, ak * TOK + cc * 512: ak * TOK + (cc + 1) * 512],
                        ps[:], AF.Relu, scale=H8SC, bias=b1_sb[:, ak:ak + 1])
                fc2_group(2 * cc)
                fc2_group(2 * cc + 1)

    nc.compile()
    return nc


# ---------------------------------------------------------------------------
# host marshaling
# ---------------------------------------------------------------------------

def _prep_core_inputs(k, xh_t, xl_t, x, shared):
    # own examples b_i = (48k + 43 i) mod 128; sem examples [16k, 16k+18).
    own = np.array([(48 * k + 43 * i) % B for i in range(BL)])
    sem_ex = np.array([(16 * k + j) % B for j in range(NSEM)])

    def slab(xt):
        return np.ascontiguousarray(
            xt[:, sem_ex, :].reshape(H, STOK).reshape(HK, 128, STOK))

    xsem_h = slab(xh_t)
    xsem_l = slab(shared["_xl_sem_t"])

    xq_h = np.ascontiguousarray(
        xh_t[:, own, :].reshape(H, TOK).reshape(HK, 128, TOK))
    xq_l = np.ascontiguousarray(
        xl_t[:, own, :].reshape(H, TOK).reshape(HK, 128, TOK))
    x_own = np.ascontiguousarray(x[own].astype(np.float32))

    masks = np.zeros((NPAIR, C), np.float32)
    for i in range(BL):
        for u in range(C):
            t = 3 * int(own[i]) + u
            masks[3 * i + u, t // B] = 1.0

    core = {
        "xsem_h": xsem_h, "xsem_l": xsem_l,
        "xq_h": xq_h, "xq_l": xq_l, "x_own": x_own, "masks": masks,
    }
    core.update({n: shared[n] for n in (
        "swh", "swl", "semb", "rw_pack", "w1h", "w1l", "w2h", "w2l",
        "m1q", "b1s", "g2s", "b2row")})
    return core


_CACHE = {}


def _make_shared(inputs):
    fc1_w = np.asarray(inputs["fc1_w"], np.float32)
    fc1_b = np.asarray(inputs["fc1_b"], np.float32)
    fc2_w = np.asarray(inputs["fc2_w"], np.float32)
    fc2_b = np.asarray(inputs["fc2_b"], np.float32)
    efc1 = np.asarray(inputs["efc1"], np.float32)
    efc2 = np.asarray(inputs["efc2"], np.float32)
    sem_w = np.asarray(inputs["sem_w"], np.float32)
    sem_b = np.asarray(inputs["sem_b"], np.float32)
    route_weights = np.asarray(inputs["route_weights"], np.float32)
    larger_w = np.asarray(inputs["larger_w"], np.float32)
    larger_b = np.asarray(inputs["larger_b"], np.float32)
    elarger = np.asarray(inputs["elarger"], np.float32)
    t = int(np.asarray(inputs["t"]))
    sf = np.float32(int(np.asarray(inputs["s"])))
    act_n = t + 1
    x = np.asarray(inputs["x"], np.float32)

    gfc1 = _sigmoid_f32(sf * efc1[t])
    gfc2 = _sigmoid_f32(sf * efc2[t])
    glarger = _sigmoid_f32(sf * elarger[t])

    lwg = (larger_w * glarger[None, :]).astype(np.float32)
    m1mat = (lwg @ fc1_w).astype(np.float32)
    b1 = ((larger_b * glarger) @ fc1_w + fc1_b).astype(np.float32)
    w2g = (fc2_w * gfc1[:, None]).astype(np.float32)
    rw4 = route_weights.reshape(C, N, S, C, S)

    # transposed x quantization (shared across cores; slabs pick columns)
    xt = np.ascontiguousarray(np.transpose(x, (2, 0, 1)))      # [H, B, S]
    xh_t = _q8(xt, SXH).reshape(H, B, S)
    xl_res = xt - xh_t.astype(np.float32) / SXH
    xl_t = _q8(xl_res, SXH).reshape(H, B, S)                   # adapter lo
    xl_sem_t = _q8(xl_res, SXSL).reshape(H, B, S)              # sem lo (finer)

    sw2d = np.transpose(sem_w, (1, 0, 2)).reshape(H, NC30)
    swh = _q8(sw2d, SW)
    swl = _q8(sw2d - swh.astype(np.float32) / SW, SSWL)
    w1h = _q8(fc1_w, SW)
    w1l = _q8(fc1_w - w1h.astype(np.float32) / SW, SW)
    w2h = _q8(w2g, SW)
    w2l = _q8(w2g - w2h.astype(np.float32) / SW, SW)

    shared = {
        "swh": swh.reshape(HK, 128, NC30),
        "swl": swl.reshape(HK, 128, NC30),
        "semb": np.ascontiguousarray(
            (sem_b.reshape(1, NC30) * 32768.0).astype(np.float32)),
        "rw_pack": np.stack([
            np.ascontiguousarray(np.transpose(rw4[c, :act_n], (1, 0, 2, 3))
                                 .reshape(S, act_n * C * S).astype(BF16))
            for c in range(C)]),
        "w1h": w1h.reshape(HK, 128, A),
        "w1l": w1l.reshape(HK, 128, A),
        "w2h": w2h.reshape(AK, 128, H),
        "w2l": w2l.reshape(AK, 128, H),
        "m1q": _q8(m1mat, SM1),
        "b1s": np.ascontiguousarray(
            (b1 * SH8).astype(np.float32).reshape(AK, 128).T),
        "g2s": np.ascontiguousarray((gfc2 * Z1INV).reshape(1, H)),
        "b2row": _bf(fc2_b.reshape(1, H)),
        "_xh_t": xh_t, "_xl_t": xl_t, "_xl_sem_t": xl_sem_t,
        "_has_semb": bool(np.any(sem_b)), "_has_b2": bool(np.any(fc2_b)),
    }
    return shared, act_n


def kernel(**inputs):
    x = np.asarray(inputs["x"], np.float32)
    shared, act_n = _make_shared(inputs)
    key = (act_n, shared["_has_semb"], shared["_has_b2"])
    if key not in _CACHE:
        _CACHE[key] = _build_program(act_n, shared["_has_semb"], shared["_has_b2"])
    nc = _CACHE[key]

    in_maps = [_prep_core_inputs(k, shared["_xh_t"], shared["_xl_t"], x, shared)
               for k in range(NCORES)]
    res = bass_utils.run_bass_kernel_spmd(nc, in_maps, core_ids=list(range(NCORES)))
    out = np.empty((B, S, H), np.float32)
    for k in range(NCORES):
        own = [(48 * k + 43 * i) % B for i in range(BL)]
        out[own] = res.results[k]["out"]
    return out


# revision 14
# speedup vs baseline: 1.5458x; 1.0310x over previous
"""Trainium2 Bass kernel for nn_BertAdapterCapsuleMask (fp8 DoubleRow version).

Strategy (8 NeuronCores, SPMD — identical program, per-core data):

Sharding (same as proven baseline): core k owns routing pairs
t in [48k, 48k+48) and output examples b_i = (48k + 43 i) mod 128, so the
pairs' sem examples are exactly the 18 consecutive [16k, 16k+18) and the
vote rows each core produces are exactly the ones its own examples'
adapter needs — zero cross-core traffic.

Speed: the adapter GEMMs (fc1 768x2048, fc2 2048x768 per 2048 tokens)
run in fp8-e4m3 with MatmulPerfMode.DoubleRow (256-deep contraction,
half-cycle per output column).  Accuracy is restored with residual
("lo") streams quantized at the SAME dequant scale as the hi streams —
fp8's relative precision is scale-invariant, so all streams of one GEMM
accumulate into a single PSUM group with zero combine cost:

  fc1 psum  = xh*w1h + xl*w1h + xh*w1l + vt8*m1q      (all at 2^15)
  fc2 psum  = h8*w2h + h8*w2l                          (at 2^15)

The capsule path keeps near-fp32 accuracy (routing softmax amplifies
sem errors ~10x): sem runs as four fp8 streams with fine residual
scales (separate psums, combines staged through SBUF), squash/routing
stay fp32, priors run bf16.  The capsule term enters fc1 via the
vt8*m1q rank-3 DoubleRow update, so h_out never materializes.

DMA discipline: every DMA instruction costs ~625ns on the single shared
HWDGE descriptor engine, so all bulk tensors move as ONE instruction
each (3D access patterns); the VT gather is a single 3D
gather-descriptor DMA; x/out staging batches 4 examples per DMA.
"""

import numpy as np
import ml_dtypes

import concourse.bass as bass
import concourse.bacc as bacc
import concourse.mybir as mybir
import concourse.tile as tile
from concourse import bass_utils

E4 = ml_dtypes.float8_e4m3
BF16 = ml_dtypes.bfloat16
F32 = mybir.dt.float32
BF = mybir.dt.bfloat16
F8 = mybir.dt.float8e4
AF = mybir.ActivationFunctionType
ALU = mybir.AluOpType
PM = mybir.MatmulPerfMode

B, S, H, A, C, N = 128, 128, 768, 2048, 3, 10
NUM_ITER = 3
NCORES = 8
BL = B // NCORES          # 16 own examples / core
NPAIR = 3 * B // NCORES   # 48 routing pairs / core
HK = H // 128             # 6
AK = A // 128             # 16
TOK = BL * S              # 2048 tokens / core
NC30 = N * C              # 30
NSEM = 18                 # sem examples per core: [16k, 16k+18) mod 128
STOK = NSEM * S           # 2304 sem tokens / core
NPRE = 1                  # fc1 col-chunks (of 512 tokens) via z1p prepass

# fp8 scales.  All adapter streams share dequant 2^-15; sem streams use
# fine residual scales (routing is sensitive) and combine explicitly.
SXH = 16.0
SW = 2048.0
SVT, SM1 = 8.0, 4096.0         # SVT*SM1 == SXH*SW == 2^15
SH8 = 16.0
SXSL = 512.0                   # xsem lo
SSWL = 32768.0                 # sem_w lo
Z1INV = 1.0 / 32768.0          # 2^-15
H8SC = SH8 * Z1INV             # 2^-11


def _sigmoid_f32(z):
    z = np.asarray(z, np.float64)
    return (1.0 / (1.0 + np.exp(-z))).astype(np.float32)


def _q8(a, scale):
    """fp8-e4m3 quantize: stores clip(a*scale, +-240); dequant is 1/scale."""
    z = np.clip(np.asarray(a, np.float32) * np.float32(scale), -240.0, 240.0)
    return np.ascontiguousarray(z.astype(E4))


def _bf(x):
    return np.ascontiguousarray(np.asarray(x, np.float32).astype(BF16))


# ---------------------------------------------------------------------------
# device program
# ---------------------------------------------------------------------------

def _build_program(act_n, has_semb, has_b2):
    nc = bacc.Bacc("TRN2", target_bir_lowering=False, debug=False,
                   num_devices=NCORES)

    d_xsh = nc.dram_tensor("xsem_h", [HK, 128, STOK], F8, kind="ExternalInput")
    d_xsl = nc.dram_tensor("xsem_l", [HK, 128, STOK], F8, kind="ExternalInput")
    d_swh = nc.dram_tensor("swh", [HK, 128, NC30], F8, kind="ExternalInput")
    d_swl = nc.dram_tensor("swl", [HK, 128, NC30], F8, kind="ExternalInput")
    d_semb = nc.dram_tensor("semb", [1, NC30], F32, kind="ExternalInput")
    d_rw = nc.dram_tensor("rw_pack", [C, 128, act_n * C * S], BF, kind="ExternalInput")
    d_masks = nc.dram_tensor("masks", [NPAIR, C], F32, kind="ExternalInput")
    d_xh = nc.dram_tensor("xq_h", [HK, 128, TOK], F8, kind="ExternalInput")
    d_xl = nc.dram_tensor("xq_l", [HK, 128, TOK], F8, kind="ExternalInput")
    d_w1h = nc.dram_tensor("w1h", [HK, 128, A], F8, kind="ExternalInput")
    d_w1l = nc.dram_tensor("w1l", [HK, 128, A], F8, kind="ExternalInput")
    d_w2h = nc.dram_tensor("w2h", [AK, 128, H], F8, kind="ExternalInput")
    d_w2l = nc.dram_tensor("w2l", [AK, 128, H], F8, kind="ExternalInput")
    d_m1q = nc.dram_tensor("m1q", [C, A], F8, kind="ExternalInput")
    d_b1s = nc.dram_tensor("b1s", [128, AK], F32, kind="ExternalInput")
    d_g2s = nc.dram_tensor("g2s", [1, H], F32, kind="ExternalInput")
    d_b2 = nc.dram_tensor("b2row", [1, H], BF, kind="ExternalInput")
    d_xown = nc.dram_tensor("x_own", [BL, S, H], F32, kind="ExternalInput")
    d_vcb = nc.dram_tensor("votecb", [NPAIR * S], F8, kind="Internal")
    d_out = nc.dram_tensor("out", [BL, S, H], F32, kind="ExternalOutput")

    def pair_ap(ap, off, stride, n):
        """[[part], [stride, 2], [1, n]] AP at ap.offset+off (DoubleRow pair)."""
        return bass.AP(ap.tensor, ap.offset + off, [ap.ap[0], [stride, 2], [1, n]])

    def merged_load(queue, dst_tile, dram, inner):
        """One-DMA load of [K, 128, inner] dram into [128, K*inner] sbuf."""
        k = dram.shape[0]
        queue.dma_start(
            dst_tile[:].rearrange("p (k c) -> p k c", k=k),
            dram.ap().rearrange("k p c -> p k c"))

    with tile.TileContext(nc) as tc:
        with (
            tc.tile_pool(name="w", bufs=1) as wp,
            tc.tile_pool(name="semx", bufs=1) as sxp,
            tc.tile_pool(name="sem", bufs=1) as smp,
            tc.tile_pool(name="rt", bufs=1) as rp,
            tc.tile_pool(name="ad", bufs=1) as ap_,
            tc.tile_pool(name="st", bufs=2) as sp,
            tc.tile_pool(name="ps", bufs=8, space="PSUM") as pp,
        ):
            # ---------------- persistent loads (one DMA each) --------------
            # SP queue: capsule-critical data first (smallest first)
            swh_sb = wp.tile([128, HK * NC30], F8)
            swl_sb = wp.tile([128, HK * NC30], F8)
            merged_load(nc.sync, swh_sb, d_swh, NC30)
            merged_load(nc.sync, swl_sb, d_swl, NC30)
            semb_sb = wp.tile([1, NC30], F32)
            nc.sync.dma_start(semb_sb[:], d_semb[:])
            masks_sb = wp.tile([NPAIR, C], F32)
            nc.sync.dma_start(masks_sb[:], d_masks[:])
            xsh_sb = sxp.tile([128, HK * STOK], F8)
            xsl_sb = sxp.tile([128, HK * STOK], F8)
            merged_load(nc.sync, xsh_sb, d_xsh, STOK)
            merged_load(nc.sync, xsl_sb, d_xsl, STOK)
            rw_sb = sxp.tile([128, C * act_n * C * S], BF)
            merged_load(nc.sync, rw_sb, d_rw, act_n * C * S)

            # ACT queue: fc1 prepass data first
            xh_sb = wp.tile([128, HK * TOK], F8, tag="bigx", bufs=1)
            w1h_sb = wp.tile([128, HK * A], F8)
            merged_load(nc.scalar, xh_sb, d_xh, TOK)
            merged_load(nc.scalar, w1h_sb, d_w1h, A)
            xl_sb = wp.tile([128, HK * TOK], F8, tag="bigx2", bufs=1)
            w1l_sb = wp.tile([128, HK * A], F8)
            merged_load(nc.scalar, xl_sb, d_xl, TOK)
            merged_load(nc.scalar, w1l_sb, d_w1l, A)
            w2h_sb = wp.tile([128, AK * H], F8, tag="bigw2", bufs=1)
            w2l_sb = wp.tile([128, AK * H], F8, tag="bigw2l", bufs=1)
            merged_load(nc.scalar, w2h_sb, d_w2h, H)
            merged_load(nc.scalar, w2l_sb, d_w2l, H)

            # m1 lhsT [128, 2*A]: rows 0-2 of k-tile0 hold m1q, rest zero
            m1_sb = wp.tile([128, 2 * A], F8)
            nc.gpsimd.memset(m1_sb[:], 0.0)
            m1_dst = bass.AP(m1_sb[:].tensor, m1_sb[:].offset,
                             [[m1_sb[:].ap[0][0], C], [1, A]])
            nc.scalar.dma_start(m1_dst, d_m1q.ap())
            # vt rhs [128, 2*TOK]: rows 0-2 of k-tile0 get the vote gather;
            # everything else zero (multiplied by zero m1 rows).
            vt_sb = ap_.tile([128, 2 * TOK], F8)
            nc.gpsimd.memset(vt_sb[:], 0.0)

            b1_sb = wp.tile([128, AK], F32)
            nc.scalar.dma_start(b1_sb[:], d_b1s[:])
            g2rep = wp.tile([128, H], F32)
            g2_src = d_g2s.ap()
            nc.scalar.dma_start(
                g2rep[:], bass.AP(g2_src.tensor, g2_src.offset, [[0, 128], [1, H]]))
            ones_f = wp.tile([1, 128], F32)
            nc.gpsimd.memset(ones_f[:], 1.0)
            if has_b2:
                ones_bf = wp.tile([1, 128], BF)
                nc.gpsimd.memset(ones_bf[:], 1.0)
                b2_sb = wp.tile([1, H], BF)
                nc.scalar.dma_start(b2_sb[:], d_b2[:])

            # ---------------- phase 1: sem (4 fp8 streams, one psum) -------
            # all streams share dequant 2^-15, so they accumulate into a
            # single PSUM group; one Act scale-copy finishes the slot.
            sem_own = smp.tile([128, NSEM * NC30], F32)
            for slot in range(NSEM):
                ps_a = pp.tile([128, NC30], F32, tag="mm", name=f"ps_sa_{slot}")
                for hp in range(HK // 2):
                    off = (2 * hp) * STOK + slot * S
                    lhT_h = pair_ap(xsh_sb[:], off, STOK, 128)
                    lhT_l = pair_ap(xsl_sb[:], off, STOK, 128)
                    woff = (2 * hp) * NC30
                    rw_h = pair_ap(swh_sb[:], woff, NC30, NC30)
                    rw_l = pair_ap(swl_sb[:], woff, NC30, NC30)
                    st = hp == 0
                    nc.tensor.matmul(ps_a[:], lhT_h, rw_h, start=st, stop=False,
                                     perf_mode=PM.DoubleRow)
                    nc.tensor.matmul(ps_a[:], lhT_l, rw_h, start=False, stop=False,
                                     perf_mode=PM.DoubleRow)
                    nc.tensor.matmul(ps_a[:], lhT_h, rw_l, start=False, stop=False,
                                     perf_mode=PM.DoubleRow)
                    nc.tensor.matmul(ps_a[:], lhT_l, rw_l, start=False,
                                     stop=(hp == 2 and not has_semb),
                                     perf_mode=PM.DoubleRow)
                if has_semb:
                    nc.tensor.matmul(ps_a[:], ones_f[:], semb_sb[:],
                                     start=False, stop=True)
                nc.scalar.activation(
                    sem_own[:, slot * NC30:(slot + 1) * NC30], ps_a[:],
                    AF.Copy, scale=Z1INV)

            # ---------------- squash over n (fp32, as baseline) ------------
            sem2 = smp.tile([128, NSEM * NC30], F32)
            nc.vector.tensor_tensor(sem2[:], sem_own[:], sem_own[:], op=ALU.mult)
            sqt = smp.tile([128, NSEM * C], F32)
            nc.vector.tensor_reduce(
                sqt[:].rearrange("p (slot cc) -> p slot cc", cc=C),
                sem2[:].rearrange("p (slot n cc) -> p slot cc n", n=N, cc=C),
                axis=mybir.AxisListType.X, op=ALU.add)
            lnq = smp.tile([128, NSEM * C], F32)
            nc.scalar.activation(lnq[:], sqt[:], AF.Ln)
            sqq = smp.tile([128, NSEM * C], F32)
            nc.scalar.activation(sqq[:], lnq[:], AF.Exp, scale=0.5)
            up = smp.tile([128, NSEM * C], F32)
            nc.vector.tensor_scalar_add(up[:], sqt[:], 1.0)
            ru = smp.tile([128, NSEM * C], F32)
            nc.vector.reciprocal(ru[:], up[:])
            fq = smp.tile([128, NSEM * C], F32)
            nc.vector.tensor_tensor(fq[:], sqq[:], ru[:], op=ALU.mult)
            sem_sq = sem2  # reuse
            f_ap = fq[:]
            f_b = bass.AP(f_ap.tensor, f_ap.offset,
                          [f_ap.ap[0], [C, NSEM], [0, N], [1, C]])
            nc.vector.tensor_tensor(
                sem_sq[:].rearrange("p (slot n cc) -> p slot n cc", n=N, cc=C),
                sem_own[:].rearrange("p (slot n cc) -> p slot n cc", n=N, cc=C),
                f_b, op=ALU.mult)
            # pair-ordered bf16 copy: block p=3i+u <- slot i+u
            sem_pair = smp.tile([128, NPAIR * NC30], BF)
            sq_ap = sem_sq[:]
            gather = bass.AP(sq_ap.tensor, sq_ap.offset,
                             [sq_ap.ap[0], [NC30, BL], [NC30, C], [1, NC30]])
            nc.vector.tensor_copy(
                sem_pair[:].rearrange("p (i u nc) -> p i u nc", i=BL, u=C),
                gather)

            # ---------------- phase 2: priors (bf16) -----------------
            sem_v = sem_pair[:].rearrange("p (pair nc) -> p nc pair", nc=NC30)
            priors = rp.tile([NPAIR, act_n * S], F32)
            for g in range(C):
                for n in range(act_n):
                    ps = pp.tile([NPAIR, S], F32, tag="mm", name=f"ps_pr_{g}_{n}")
                    rbase = g * (act_n * C * S) + (n * C) * S
                    for cc in range(C):
                        nc.tensor.matmul(
                            ps[:], sem_v[:, n * C + cc, :],
                            rw_sb[:, rbase + cc * S: rbase + (cc + 1) * S],
                            start=(cc == 0), stop=(cc == C - 1))
                    dst = priors[:, n * S:(n + 1) * S]
                    if g == 0:
                        nc.vector.tensor_scalar_mul(dst, ps[:], masks_sb[:, 0:1])
                    else:
                        nc.vector.scalar_tensor_tensor(
                            dst, ps[:], masks_sb[:, g:g + 1], dst,
                            op0=ALU.mult, op1=ALU.add)

            # ---------------- phase 3: routing (fp32, as baseline) ---------
            vote = rp.tile([NPAIR, S], F32)
            scr = rp.tile([NPAIR, S], F32)
            La = rp.tile([NPAIR, act_n], F32)
            Lb = rp.tile([NPAIR, act_n], F32)
            sqv = rp.tile([NPAIR, 1], F32)
            lv = rp.tile([NPAIR, 1], F32)
            sv = rp.tile([NPAIR, 1], F32)
            uv = rp.tile([NPAIR, 1], F32)
            rv = rp.tile([NPAIR, 1], F32)
            fv = rp.tile([NPAIR, 1], F32)
            outv = rp.tile([NPAIR, S], F32)
            mx = rp.tile([NPAIR, 1], F32)
            mneg = rp.tile([NPAIR, 1], F32)
            ex = rp.tile([NPAIR, act_n], F32)
            es = rp.tile([NPAIR, 1], F32)
            ers = rp.tile([NPAIR, 1], F32)
            probs = rp.tile([NPAIR, act_n], F32)

            def vote_from(pr_sc, first_const=None):
                for n in range(act_n):
                    blk = priors[:, n * S:(n + 1) * S]
                    sc = first_const if first_const is not None else pr_sc[:, n:n + 1]
                    if n == 0:
                        nc.vector.tensor_scalar_mul(vote[:], blk, sc)
                    else:
                        nc.vector.scalar_tensor_tensor(
                            vote[:], blk, sc, vote[:], op0=ALU.mult, op1=ALU.add)

            def squash_vote():
                nc.vector.tensor_tensor(scr[:], vote[:], vote[:], op=ALU.mult)
                nc.vector.tensor_reduce(sqv[:], scr[:],
                                        axis=mybir.AxisListType.X, op=ALU.add)
                nc.scalar.activation(lv[:], sqv[:], AF.Ln)
                nc.scalar.activation(sv[:], lv[:], AF.Exp, scale=0.5)
                nc.vector.tensor_scalar_add(uv[:], sqv[:], 1.0)
                nc.vector.reciprocal(rv[:], uv[:])
                nc.vector.tensor_tensor(fv[:], sv[:], rv[:], op=ALU.mult)
                nc.vector.tensor_scalar_mul(outv[:], vote[:], fv[:])

            def deltas(Lprev, Lnew, first):
                for n in range(act_n):
                    nc.vector.tensor_tensor(
                        scr[:], priors[:, n * S:(n + 1) * S], outv[:], op=ALU.mult)
                    nc.vector.tensor_reduce(
                        Lnew[:, n:n + 1], scr[:],
                        axis=mybir.AxisListType.X, op=ALU.add)
                if not first:
                    nc.vector.tensor_tensor(Lnew[:], Lnew[:], Lprev[:], op=ALU.add)

            def softmax(L):
                nc.vector.tensor_reduce(mx[:], L[:], axis=mybir.AxisListType.X,
                                        op=ALU.max)
                nc.vector.tensor_scalar_mul(mneg[:], mx[:], -1.0)
                nc.scalar.activation(ex[:], L[:], AF.Exp, bias=mneg[:],
                                     accum_out=es[:])
                nc.vector.reciprocal(ers[:], es[:])
                nc.vector.tensor_scalar_mul(probs[:], ex[:], ers[:])

            vote_from(None, first_const=1.0 / act_n)
            squash_vote()
            deltas(None, La, first=True)
            softmax(La)
            vote_from(probs)
            squash_vote()
            deltas(La, Lb, first=False)
            softmax(Lb)
            vote_from(probs)

            # vote -> fp8 (*8) -> DRAM -> one gather DMA:
            # vt[u, e*128+s] = votecb[e*384 + 3*s + u]
            vb = rp.tile([NPAIR, S], F8)
            nc.scalar.activation(vb[:], vote[:], AF.Copy, scale=SVT)
            nc.sync.dma_start(
                d_vcb.ap().rearrange("(p s) -> p s", p=NPAIR), vb[:])
            vflat = d_vcb.ap()
            vsrc = bass.AP(vflat.tensor, vflat.offset,
                           [[1, C], [C * S, BL], [C, S]])
            vdst = bass.AP(vt_sb[:].tensor, vt_sb[:].offset,
                           [[vt_sb[:].ap[0][0], C], [S, BL], [1, S]])
            nc.sync.dma_start(vdst, vsrc)

            # ---------------- phase 4: fc1 (fp8 DR, single psum group) -----
            z1p = ap_.tile([128, NPRE * AK * 512], BF)
            h8 = ap_.tile([128, AK * TOK], F8)

            def fc1_x_streams(ps, ak, col, last_stop):
                """9 DR matmuls: xh*w1h + xl*w1h + xh*w1l into one group."""
                for hp in range(HK // 2):
                    woff = (2 * hp) * A + ak * 128
                    xoff = (2 * hp) * TOK + col
                    lh_h = pair_ap(w1h_sb[:], woff, A, 128)
                    lh_l = pair_ap(w1l_sb[:], woff, A, 128)
                    rh_h = pair_ap(xh_sb[:], xoff, TOK, 512)
                    rh_l = pair_ap(xl_sb[:], xoff, TOK, 512)
                    st = hp == 0
                    nc.tensor.matmul(ps[:], lh_h, rh_h, start=st, stop=False,
                                     perf_mode=PM.DoubleRow)
                    nc.tensor.matmul(ps[:], lh_h, rh_l, start=False, stop=False,
                                     perf_mode=PM.DoubleRow)
                    nc.tensor.matmul(ps[:], lh_l, rh_h, start=False,
                                     stop=(hp == 2 and last_stop),
                                     perf_mode=PM.DoubleRow)

            def m1_step(ps, ak, col, start):
                lh = pair_ap(m1_sb[:], ak * 128, A, 128)
                rh = pair_ap(vt_sb[:], col, TOK, 512)
                nc.tensor.matmul(ps[:], lh, rh, start=start, stop=True,
                                 perf_mode=PM.DoubleRow)

            # prepass (x-only) for col chunks 0..NPRE-1
            for cc in range(NPRE):
                for ak in range(AK):
                    ps = pp.tile([128, 512], F32, tag="mm", name=f"ps_f1p_{cc}_{ak}")
                    fc1_x_streams(ps, ak, cc * 512, last_stop=True)
                    nc.scalar.copy(
                        z1p[:, (cc * AK + ak) * 512:(cc * AK + ak + 1) * 512],
                        ps[:])

            def fc2_chunk(tt, xt, ot, j):
                """fc2 for token chunk tt (=example tt): h8 x (w2h+w2l)."""
                psa = pp.tile([128, 512], F32, tag="mm", name=f"ps_f2a_{tt}")
                psb = pp.tile([128, 256], F32, tag="mm", name=f"ps_f2b_{tt}")
                for w_sb, first in ((w2h_sb, True), (w2l_sb, False)):
                    for ap8 in range(AK // 2):
                        woff = (2 * ap8) * H
                        hoff = (2 * ap8) * TOK + tt * 128
                        lh = pair_ap(h8[:], hoff, TOK, 128)
                        st = first and ap8 == 0
                        sp_ = (not first) and ap8 == AK // 2 - 1
                        nc.tensor.matmul(psa[:], lh, pair_ap(w_sb[:], woff, H, 512),
                                         start=st, stop=(sp_ and not has_b2),
                                         perf_mode=PM.DoubleRow)
                        nc.tensor.matmul(psb[:], lh,
                                         pair_ap(w_sb[:], woff + 512, H, 256),
                                         start=st, stop=(sp_ and not has_b2),
                                         perf_mode=PM.DoubleRow)
                if has_b2:
                    nc.tensor.matmul(psa[:], ones_bf[:], b2_sb[:, 0:512],
                                     start=False, stop=True)
                    nc.tensor.matmul(psb[:], ones_bf[:], b2_sb[:, 512:H],
                                     start=False, stop=True)
                osl = ot[:, j * H:(j + 1) * H]
                nc.scalar.activation(osl[:, 0:512], psa[:], AF.Relu)
                nc.scalar.activation(osl[:, 512:H], psb[:], AF.Relu)
                nc.vector.tensor_tensor(osl, osl, g2rep[:], op=ALU.mult)
                nc.gpsimd.tensor_tensor(osl, osl, xt[:, j * H:(j + 1) * H],
                                        op=ALU.add)

            def fc2_group(gg):
                """2 fc2 chunks + batched x-in / out DMAs (1 each)."""
                xt = sp.tile([128, 2 * H], F32, tag="x", name=f"x_{gg}", bufs=1)
                nc.sync.dma_start(
                    xt[:].rearrange("p (e h) -> p e h", e=2),
                    d_xown.ap()[2 * gg:2 * gg + 2].rearrange("e p h -> p e h"))
                ot = sp.tile([128, 2 * H], F32, tag="o", name=f"o_{gg}", bufs=2)
                for j in range(2):
                    fc2_chunk(2 * gg + j, xt, ot, j)
                nc.sync.dma_start(
                    d_out.ap()[2 * gg:2 * gg + 2].rearrange("e p h -> p e h"),
                    ot[:].rearrange("p (e h) -> p e h", e=2))

            # catch-up (m1 + z1p -> h8) then fc2 per 512-col chunk
            for cc in range(NPRE):
                for ak in range(AK):
                    ps = pp.tile([128, 512], F32, tag="mm", name=f"ps_f1m_{cc}_{ak}")
                    m1_step(ps, ak, cc * 512, start=True)
                    nc.vector.tensor_tensor(
                        ps[:], ps[:],
                        z1p[:, (cc * AK + ak) * 512:(cc * AK + ak + 1) * 512],
                        op=ALU.add)
                    nc.scalar.activation(
                        h8[:, ak * TOK + cc * 512: ak * TOK + (cc + 1) * 512],
                        ps[:], AF.Relu, scale=H8SC, bias=b1_sb[:, ak:ak + 1])
                fc2_group(2 * cc)
                fc2_group(2 * cc + 1)
            # fused blocks for col chunks NPRE..3
            for cc in range(NPRE, 4):
                for ak in range(AK):
                    ps = pp.tile([128, 512], F32, tag="mm", name=f"ps_f1f_{cc}_{ak}")
                    fc1_x_streams(ps, ak, cc * 512, last_stop=False)
                    m1_step(ps, ak, cc * 512, start=False)
                    nc.scalar.activation(
                        h8[:, ak * TOK + cc * 512: ak * TOK + (cc + 1) * 512],
                        ps[:], AF.Relu, scale=H8SC, bias=b1_sb[:, ak:ak + 1])
                fc2_group(2 * cc)
                fc2_group(2 * cc + 1)

    nc.compile()
    return nc


# ---------------------------------------------------------------------------
# host marshaling
# ---------------------------------------------------------------------------

def _prep_core_inputs(k, xh_t, xl_t, x, shared):
    # own examples b_i = (48k + 43 i) mod 128; sem examples [16k, 16k+18).
    own = np.array([(48 * k + 43 * i) % B for i in range(BL)])
    sem_ex = np.array([(16 * k + j) % B for j in range(NSEM)])

    def slab(xt):
        return np.ascontiguousarray(
            xt[:, sem_ex, :].reshape(H, STOK).reshape(HK, 128, STOK))

    xsem_h = slab(xh_t)
    xsem_l = slab(xl_t)

    xq_h = np.ascontiguousarray(
        xh_t[:, own, :].reshape(H, TOK).reshape(HK, 128, TOK))
    xq_l = np.ascontiguousarray(
        xl_t[:, own, :].reshape(H, TOK).reshape(HK, 128, TOK))
    x_own = np.ascontiguousarray(x[own].astype(np.float32))

    masks = np.zeros((NPAIR, C), np.float32)
    for i in range(BL):
        for u in range(C):
            t = 3 * int(own[i]) + u
            masks[3 * i + u, t // B] = 1.0

    core = {
        "xsem_h": xsem_h, "xsem_l": xsem_l,
        "xq_h": xq_h, "xq_l": xq_l, "x_own": x_own, "masks": masks,
    }
    core.update({n: shared[n] for n in (
        "swh", "swl", "semb", "rw_pack", "w1h", "w1l", "w2h", "w2l",
        "m1q", "b1s", "g2s", "b2row")})
    return core


_CACHE = {}


def _make_shared(inputs):
    fc1_w = np.asarray(inputs["fc1_w"], np.float32)
    fc1_b = np.asarray(inputs["fc1_b"], np.float32)
    fc2_w = np.asarray(inputs["fc2_w"], np.float32)
    fc2_b = np.asarray(inputs["fc2_b"], np.float32)
    efc1 = np.asarray(inputs["efc1"], np.float32)
    efc2 = np.asarray(inputs["efc2"], np.float32)
    sem_w = np.asarray(inputs["sem_w"], np.float32)
    sem_b = np.asarray(inputs["sem_b"], np.float32)
    route_weights = np.asarray(inputs["route_weights"], np.float32)
    larger_w = np.asarray(inputs["larger_w"], np.float32)
    larger_b = np.asarray(inputs["larger_b"], np.float32)
    elarger = np.asarray(inputs["elarger"], np.float32)
    t = int(np.asarray(inputs["t"]))
    sf = np.float32(int(np.asarray(inputs["s"])))
    act_n = t + 1
    x = np.asarray(inputs["x"], np.float32)

    gfc1 = _sigmoid_f32(sf * efc1[t])
    gfc2 = _sigmoid_f32(sf * efc2[t])
    glarger = _sigmoid_f32(sf * elarger[t])

    lwg = (larger_w * glarger[None, :]).astype(np.float32)
    m1mat = (lwg @ fc1_w).astype(np.float32)
    b1 = ((larger_b * glarger) @ fc1_w + fc1_b).astype(np.float32)
    w2g = (fc2_w * gfc1[:, None]).astype(np.float32)
    rw4 = route_weights.reshape(C, N, S, C, S)

    # transposed x quantization (shared across cores; slabs pick columns)
    xt = np.ascontiguousarray(np.transpose(x, (2, 0, 1)))      # [H, B, S]
    xh_t = _q8(xt, SXH).reshape(H, B, S)
    xl_res = xt - xh_t.astype(np.float32) / SXH
    xl_t = _q8(xl_res, SXH).reshape(H, B, S)                   # lo at same scale

    sw2d = np.transpose(sem_w, (1, 0, 2)).reshape(H, NC30)
    swh = _q8(sw2d, SW)
    swl = _q8(sw2d - swh.astype(np.float32) / SW, SW)
    w1h = _q8(fc1_w, SW)
    w1l = _q8(fc1_w - w1h.astype(np.float32) / SW, SW)
    w2h = _q8(w2g, SW)
    w2l = _q8(w2g - w2h.astype(np.float32) / SW, SW)

    shared = {
        "swh": swh.reshape(HK, 128, NC30),
        "swl": swl.reshape(HK, 128, NC30),
        "semb": np.ascontiguousarray(
            (sem_b.reshape(1, NC30) * 32768.0).astype(np.float32)),
        "rw_pack": np.stack([
            np.ascontiguousarray(np.transpose(rw4[c, :act_n], (1, 0, 2, 3))
                                 .reshape(S, act_n * C * S).astype(BF16))
            for c in range(C)]),
        "w1h": w1h.reshape(HK, 128, A),
        "w1l": w1l.reshape(HK, 128, A),
        "w2h": w2h.reshape(AK, 128, H),
        "w2l": w2l.reshape(AK, 128, H),
        "m1q": _q8(m1mat, SM1),
        "b1s": np.ascontiguousarray(
            (b1 * SH8).astype(np.float32).reshape(AK, 128).T),
        "g2s": np.ascontiguousarray((gfc2 * Z1INV).reshape(1, H)),
        "b2row": _bf(fc2_b.reshape(1, H)),
        "_xh_t": xh_t, "_xl_t": xl_t,
        "_has_semb": bool(np.any(sem_b)), "_has_b2": bool(np.any(fc2_b)),
    }
    return shared, act_n


def kernel(**inputs):
    x = np.asarray(inputs["x"], np.float32)
    shared, act_n = _make_shared(inputs)
    key = (act_n, shared["_has_semb"], shared["_has_b2"])
    if key not in _CACHE:
        _CACHE[key] = _build_program(act_n, shared["_has_semb"], shared["_has_b2"])
    nc = _CACHE[key]

    in_maps = [_prep_core_inputs(k, shared["_xh_t"], shared["_xl_t"], x, shared)
               for k in range(NCORES)]
    res = bass_utils.run_bass_kernel_spmd(nc, in_maps, core_ids=list(range(NCORES)))
    out = np.empty((B, S, H), np.float32)
    for k in range(NCORES):
        own = [(48 * k + 43 * i) % B for i in range(BL)]
        out[own] = res.results[k]["out"]
    return out


# revision 26
# speedup vs baseline: 1.5930x; 1.0305x over previous
"""Trainium2 Bass kernel for nn_BertAdapterCapsuleMask (fp8 DoubleRow version).

Strategy (8 NeuronCores, SPMD — identical program, per-core data):

Sharding (same as proven baseline): core k owns routing pairs
t in [48k, 48k+48) and output examples b_i = (48k + 43 i) mod 128, so the
pairs' sem examples are exactly the 18 consecutive [16k, 16k+18) and the
vote rows each core produces are exactly the ones its own examples'
adapter needs — zero cross-core traffic.

Speed: the adapter GEMMs (fc1 768x2048, fc2 2048x768 per 2048 tokens)
run in fp8-e4m3 with MatmulPerfMode.DoubleRow (256-deep contraction,
half-cycle per output column).  Accuracy is restored with residual
("lo") streams quantized at the SAME dequant scale as the hi streams —
fp8's relative precision is scale-invariant, so all streams of one GEMM
accumulate into a single PSUM group with zero combine cost:

  fc1 psum  = xh*w1h + xl*w1h + xh*w1l + vt8*m1q      (all at 2^15)
  fc2 psum  = h8*w2h + h8*w2l                          (at 2^15)

The capsule path keeps near-fp32 accuracy (routing softmax amplifies
sem errors ~10x): sem runs as four fp8 streams with fine residual
scales (separate psums, combines staged through SBUF), squash/routing
stay fp32, priors run bf16.  The capsule term enters fc1 via the
vt8*m1q rank-3 DoubleRow update, so h_out never materializes.

DMA discipline: every DMA instruction costs ~625ns on the single shared
HWDGE descriptor engine, so all bulk tensors move as ONE instruction
each (3D access patterns); the VT gather is a single 3D
gather-descriptor DMA; x/out staging batches 4 examples per DMA.
"""

import numpy as np
import ml_dtypes

import concourse.bass as bass
import concourse.bacc as bacc
import concourse.mybir as mybir
import concourse.tile as tile
from concourse import bass_utils

E4 = ml_dtypes.float8_e4m3
BF16 = ml_dtypes.bfloat16
F32 = mybir.dt.float32
BF = mybir.dt.bfloat16
F8 = mybir.dt.float8e4
AF = mybir.ActivationFunctionType
ALU = mybir.AluOpType
PM = mybir.MatmulPerfMode

B, S, H, A, C, N = 128, 128, 768, 2048, 3, 10
NUM_ITER = 3
NCORES = 8
BL = B // NCORES          # 16 own examples / core
NPAIR = 3 * B // NCORES   # 48 routing pairs / core
HK = H // 128             # 6
AK = A // 128             # 16
TOK = BL * S              # 2048 tokens / core
NC30 = N * C              # 30
NSEM = 18                 # sem examples per core: [16k, 16k+18) mod 128
STOK = NSEM * S           # 2304 sem tokens / core
NPRE = 1                  # fc1 col-chunks (of 512 tokens) via z1p prepass

# fp8 scales.  All adapter streams share dequant 2^-15; sem streams use
# fine residual scales (routing is sensitive) and combine explicitly.
SXH = 16.0
SW = 2048.0
SVT, SM1 = 8.0, 4096.0         # SVT*SM1 == SXH*SW == 2^15
SH8 = 16.0
SXSL = 512.0                   # xsem lo
SSWL = 32768.0                 # sem_w lo
Z1INV = 1.0 / 32768.0          # 2^-15
H8SC = SH8 * Z1INV             # 2^-11


def _sigmoid_f32(z):
    z = np.asarray(z, np.float64)
    return (1.0 / (1.0 + np.exp(-z))).astype(np.float32)


def _q8(a, scale):
    """fp8-e4m3 quantize: stores clip(a*scale, +-240); dequant is 1/scale."""
    z = np.clip(np.asarray(a, np.float32) * np.float32(scale), -240.0, 240.0)
    return np.ascontiguousarray(z.astype(E4))


def _bf(x):
    return np.ascontiguousarray(np.asarray(x, np.float32).astype(BF16))


# ---------------------------------------------------------------------------
# device program
# ---------------------------------------------------------------------------

def _build_program(act_n, has_semb, has_b2):
    nc = bacc.Bacc("TRN2", target_bir_lowering=False, debug=False,
                   num_devices=NCORES)

    d_xsh = nc.dram_tensor("xsem_h", [HK, 128, STOK], F8, kind="ExternalInput")
    d_xsl = nc.dram_tensor("xsem_l", [HK, 128, STOK], F8, kind="ExternalInput")
    d_swc = nc.dram_tensor("sw_cat", [HK, 128, 2 * NC30], F8, kind="ExternalInput")
    d_semb = nc.dram_tensor("semb", [1, NC30], F32, kind="ExternalInput")
    d_rw = nc.dram_tensor("rw_pack", [C, 128, act_n * C * S], BF, kind="ExternalInput")
    d_masks = nc.dram_tensor("masks", [NPAIR, C], F32, kind="ExternalInput")
    d_xh = nc.dram_tensor("xq_h", [HK, 128, TOK], F8, kind="ExternalInput")
    d_xl = nc.dram_tensor("xq_l", [HK, 128, TOK], F8, kind="ExternalInput")
    d_w1h = nc.dram_tensor("w1h", [HK, 128, A], F8, kind="ExternalInput")
    d_w1l = nc.dram_tensor("w1l", [HK, 128, A], F8, kind="ExternalInput")
    d_w2h = nc.dram_tensor("w2h", [AK, 128, H], F8, kind="ExternalInput")
    d_w2l = nc.dram_tensor("w2l", [AK, 128, H], F8, kind="ExternalInput")
    d_m1q = nc.dram_tensor("m1q", [C, A], F8, kind="ExternalInput")
    d_b1s = nc.dram_tensor("b1s", [128, AK], F32, kind="ExternalInput")
    d_g2s = nc.dram_tensor("g2s", [1, H], F32, kind="ExternalInput")
    d_b2 = nc.dram_tensor("b2row", [1, H], BF, kind="ExternalInput")
    d_xown = nc.dram_tensor("x_own", [BL, S, H], BF, kind="ExternalInput")
    d_vcb = nc.dram_tensor("votecb", [NPAIR * S], F8, kind="Internal")
    d_out = nc.dram_tensor("out", [BL, S, H], F32, kind="ExternalOutput")

    def pair_ap(ap, off, stride, n):
        """[[part], [stride, 2], [1, n]] AP at ap.offset+off (DoubleRow pair)."""
        return bass.AP(ap.tensor, ap.offset + off, [ap.ap[0], [stride, 2], [1, n]])

    def merged_load(queue, dst_tile, dram, inner):
        """One-DMA load of [K, 128, inner] dram into [128, K*inner] sbuf."""
        k = dram.shape[0]
        queue.dma_start(
            dst_tile[:].rearrange("p (k c) -> p k c", k=k),
            dram.ap().rearrange("k p c -> p k c"))

    with tile.TileContext(nc) as tc:
        with (
            tc.tile_pool(name="w", bufs=1) as wp,
            tc.tile_pool(name="semx", bufs=1) as sxp,
            tc.tile_pool(name="sem", bufs=1) as smp,
            tc.tile_pool(name="rt", bufs=1) as rp,
            tc.tile_pool(name="ad", bufs=1) as ap_,
            tc.tile_pool(name="st", bufs=2) as sp,
            tc.tile_pool(name="ps", bufs=8, space="PSUM") as pp,
        ):
            # ---------------- persistent loads (one DMA each) --------------
            # capsule-critical data first, split across both queues
            xsh_sb = sxp.tile([128, HK * STOK], F8)
            xsl_sb = sxp.tile([128, HK * STOK], F8)
            merged_load(nc.sync, xsh_sb, d_xsh, STOK)
            merged_load(nc.sync, xsl_sb, d_xsl, STOK)
            semb_sb = wp.tile([1, NC30], F32)
            nc.sync.dma_start(semb_sb[:], d_semb[:])
            masks_sb = wp.tile([NPAIR, C], F32)
            nc.sync.dma_start(masks_sb[:], d_masks[:])
            swc_sb = wp.tile([128, HK * 2 * NC30], F8)
            merged_load(nc.scalar, swc_sb, d_swc, 2 * NC30)
            rw_sb = sxp.tile([128, C * act_n * C * S], BF)
            merged_load(nc.scalar, rw_sb, d_rw, act_n * C * S)

            # adapter data
            xh_sb = wp.tile([128, HK * TOK], F8, tag="bigx", bufs=1)
            w1h_sb = wp.tile([128, HK * A], F8)
            merged_load(nc.scalar, xh_sb, d_xh, TOK)
            merged_load(nc.scalar, w1h_sb, d_w1h, A)
            xl_sb = wp.tile([128, HK * TOK], F8, tag="bigx2", bufs=1)
            w1l_sb = wp.tile([128, HK * A], F8)
            merged_load(nc.scalar, xl_sb, d_xl, TOK)
            merged_load(nc.scalar, w1l_sb, d_w1l, A)

            # m1 lhsT [128, 2*A]: rows 0-2 of k-tile0 hold m1q, rest zero
            m1_sb = wp.tile([128, 2 * A], F8)
            nc.gpsimd.memset(m1_sb[:], 0.0)
            m1_dst = bass.AP(m1_sb[:].tensor, m1_sb[:].offset,
                             [[m1_sb[:].ap[0][0], C], [1, A]])
            nc.scalar.dma_start(m1_dst, d_m1q.ap())
            # vt rhs [128, 2*TOK]: rows 0-2 of k-tile0 get the vote gather;
            # everything else zero (multiplied by zero m1 rows).
            vt_sb = ap_.tile([128, 2 * TOK], F8)
            nc.gpsimd.memset(vt_sb[:], 0.0)

            b1_sb = wp.tile([128, AK], F32)
            nc.scalar.dma_start(b1_sb[:], d_b1s[:])
            g2rep = wp.tile([128, H], F32)
            g2_src = d_g2s.ap()
            nc.scalar.dma_start(
                g2rep[:], bass.AP(g2_src.tensor, g2_src.offset, [[0, 128], [1, H]]))
            w2h_sb = wp.tile([128, AK * H], F8, tag="bigw2", bufs=1)
            w2l_sb = wp.tile([128, AK * H], F8, tag="bigw2l", bufs=1)
            merged_load(nc.scalar, w2h_sb, d_w2h, H)
            merged_load(nc.scalar, w2l_sb, d_w2l, H)
            ones_f = wp.tile([1, 128], F32)
            nc.gpsimd.memset(ones_f[:], 1.0)
            if has_b2:
                ones_bf = wp.tile([1, 128], BF)
                nc.gpsimd.memset(ones_bf[:], 1.0)
                b2_sb = wp.tile([1, H], BF)
                nc.scalar.dma_start(b2_sb[:], d_b2[:])

            # ---------------- phase 1: sem (4 fp8 streams) -----------------
            # swh||swl are concatenated in the rhs, so each DR matmul feeds
            # two streams at once (6 matmuls per slot); 4 slots share one
            # PSUM bank under a single accumulation bracket (start zeroes
            # the whole 2KB zero-region).  hi+lo halves then fold via one
            # Act copy + one strided DVE add per group.
            sem_own = smp.tile([128, NSEM * NC30], F32)
            SEMG = 4
            ngrp = (NSEM + SEMG - 1) // SEMG
            for grp in range(ngrp):
                slots = range(grp * SEMG, min((grp + 1) * SEMG, NSEM))
                nsl = len(slots)
                ps = pp.tile([128, SEMG * 2 * NC30], F32, tag="mm",
                             name=f"ps_sem_{grp}")
                for j, slot in enumerate(slots):
                    for hp in range(HK // 2):
                        off = (2 * hp) * STOK + slot * S
                        lhT_h = pair_ap(xsh_sb[:], off, STOK, 128)
                        lhT_l = pair_ap(xsl_sb[:], off, STOK, 128)
                        rw_c = pair_ap(swc_sb[:], (2 * hp) * 2 * NC30,
                                       2 * NC30, 2 * NC30)
                        dst = ps[:, j * 2 * NC30:(j + 1) * 2 * NC30]
                        st = j == 0 and hp == 0
                        sp_last = (j == nsl - 1 and hp == 2 and not has_semb)
                        nc.tensor.matmul(dst, lhT_h, rw_c, start=st, stop=False,
                                         perf_mode=PM.DoubleRow)
                        nc.tensor.matmul(dst, lhT_l, rw_c, start=False,
                                         stop=sp_last, perf_mode=PM.DoubleRow)
                    if has_semb:
                        nc.tensor.matmul(
                            ps[:, j * 2 * NC30: j * 2 * NC30 + NC30],
                            ones_f[:], semb_sb[:], start=False,
                            stop=(j == nsl - 1))
                # fold hi/lo halves: sem = 2^-15 * (ps[,0:30] + ps[,30:60])
                sc = smp.tile([128, SEMG * 2 * NC30], F32, tag="sc", bufs=2,
                              name=f"sc_{grp}")
                nc.scalar.activation(sc[:, 0:nsl * 2 * NC30],
                                     ps[:, 0:nsl * 2 * NC30],
                                     AF.Copy, scale=Z1INV)
                sc_ap = sc[:]
                hi = bass.AP(sc_ap.tensor, sc_ap.offset,
                             [sc_ap.ap[0], [2 * NC30, nsl], [1, NC30]])
                lo = bass.AP(sc_ap.tensor, sc_ap.offset + NC30,
                             [sc_ap.ap[0], [2 * NC30, nsl], [1, NC30]])
                nc.vector.tensor_tensor(
                    sem_own[:, grp * SEMG * NC30:
                            (grp * SEMG + nsl) * NC30].rearrange(
                        "p (s c) -> p s c", c=NC30),
                    hi, lo, op=ALU.add)

            # ---------------- squash over n (fp32, as baseline) ------------
            sem2 = smp.tile([128, NSEM * NC30], F32)
            nc.vector.tensor_tensor(sem2[:], sem_own[:], sem_own[:], op=ALU.mult)
            sqt = smp.tile([128, NSEM * C], F32)
            nc.vector.tensor_reduce(
                sqt[:].rearrange("p (slot cc) -> p slot cc", cc=C),
                sem2[:].rearrange("p (slot n cc) -> p slot cc n", n=N, cc=C),
                axis=mybir.AxisListType.X, op=ALU.add)
            lnq = smp.tile([128, NSEM * C], F32)
            nc.scalar.activation(lnq[:], sqt[:], AF.Ln)
            sqq = smp.tile([128, NSEM * C], F32)
            nc.scalar.activation(sqq[:], lnq[:], AF.Exp, scale=0.5)
            up = smp.tile([128, NSEM * C], F32)
            nc.vector.tensor_scalar_add(up[:], sqt[:], 1.0)
            ru = smp.tile([128, NSEM * C], F32)
            nc.vector.reciprocal(ru[:], up[:])
            fq = smp.tile([128, NSEM * C], F32)
            nc.vector.tensor_tensor(fq[:], sqq[:], ru[:], op=ALU.mult)
            sem_sq = sem2  # reuse
            f_ap = fq[:]
            f_b = bass.AP(f_ap.tensor, f_ap.offset,
                          [f_ap.ap[0], [C, NSEM], [0, N], [1, C]])
            nc.vector.tensor_tensor(
                sem_sq[:].rearrange("p (slot n cc) -> p slot n cc", n=N, cc=C),
                sem_own[:].rearrange("p (slot n cc) -> p slot n cc", n=N, cc=C),
                f_b, op=ALU.mult)
            # pair-ordered bf16 copy: block p=3i+u <- slot i+u
            sem_pair = smp.tile([128, NPAIR * NC30], BF)
            sq_ap = sem_sq[:]
            gather = bass.AP(sq_ap.tensor, sq_ap.offset,
                             [sq_ap.ap[0], [NC30, BL], [NC30, C], [1, NC30]])
            nc.vector.tensor_copy(
                sem_pair[:].rearrange("p (i u nc) -> p i u nc", i=BL, u=C),
                gather)

            # ---------------- phase 2: priors (bf16) -----------------
            sem_v = sem_pair[:].rearrange("p (pair nc) -> p nc pair", nc=NC30)
            priors = rp.tile([NPAIR, act_n * S], F32)
            for g in range(C):
                for n in range(act_n):
                    ps = pp.tile([NPAIR, S], F32, tag="mm", name=f"ps_pr_{g}_{n}")
                    rbase = g * (act_n * C * S) + (n * C) * S
                    for cc in range(C):
                        nc.tensor.matmul(
                            ps[:], sem_v[:, n * C + cc, :],
                            rw_sb[:, rbase + cc * S: rbase + (cc + 1) * S],
                            start=(cc == 0), stop=(cc == C - 1))
                    dst = priors[:, n * S:(n + 1) * S]
                    if g == 0:
                        nc.vector.tensor_scalar_mul(dst, ps[:], masks_sb[:, 0:1])
                    else:
                        nc.vector.scalar_tensor_tensor(
                            dst, ps[:], masks_sb[:, g:g + 1], dst,
                            op0=ALU.mult, op1=ALU.add)

            # ---------------- phase 3: routing (fp32, as baseline) ---------
            vote = rp.tile([NPAIR, S], F32)
            scr = rp.tile([NPAIR, S], F32)
            La = rp.tile([NPAIR, act_n], F32)
            Lb = rp.tile([NPAIR, act_n], F32)
            sqv = rp.tile([NPAIR, 1], F32)
            lv = rp.tile([NPAIR, 1], F32)
            sv = rp.tile([NPAIR, 1], F32)
            uv = rp.tile([NPAIR, 1], F32)
            rv = rp.tile([NPAIR, 1], F32)
            fv = rp.tile([NPAIR, 1], F32)
            outv = rp.tile([NPAIR, S], F32)
            mx = rp.tile([NPAIR, 1], F32)
            mneg = rp.tile([NPAIR, 1], F32)
            ex = rp.tile([NPAIR, act_n], F32)
            es = rp.tile([NPAIR, 1], F32)
            ers = rp.tile([NPAIR, 1], F32)
            probs = rp.tile([NPAIR, act_n], F32)

            def vote_from(pr_sc, first_const=None):
                for n in range(act_n):
                    blk = priors[:, n * S:(n + 1) * S]
                    sc = first_const if first_const is not None else pr_sc[:, n:n + 1]
                    if n == 0:
                        nc.vector.tensor_scalar_mul(vote[:], blk, sc)
                    else:
                        nc.vector.scalar_tensor_tensor(
                            vote[:], blk, sc, vote[:], op0=ALU.mult, op1=ALU.add)

            def squash_vote():
                nc.vector.tensor_tensor(scr[:], vote[:], vote[:], op=ALU.mult)
                nc.vector.tensor_reduce(sqv[:], scr[:],
                                        axis=mybir.AxisListType.X, op=ALU.add)
                nc.scalar.activation(lv[:], sqv[:], AF.Ln)
                nc.scalar.activation(sv[:], lv[:], AF.Exp, scale=0.5)
                nc.vector.tensor_scalar_add(uv[:], sqv[:], 1.0)
                nc.vector.reciprocal(rv[:], uv[:])
                nc.vector.tensor_tensor(fv[:], sv[:], rv[:], op=ALU.mult)
                nc.vector.tensor_scalar_mul(outv[:], vote[:], fv[:])

            def deltas(Lprev, Lnew, first):
                for n in range(act_n):
                    nc.vector.tensor_tensor(
                        scr[:], priors[:, n * S:(n + 1) * S], outv[:], op=ALU.mult)
                    nc.vector.tensor_reduce(
                        Lnew[:, n:n + 1], scr[:],
                        axis=mybir.AxisListType.X, op=ALU.add)
                if not first:
                    nc.vector.tensor_tensor(Lnew[:], Lnew[:], Lprev[:], op=ALU.add)

            def softmax(L):
                nc.vector.tensor_reduce(mx[:], L[:], axis=mybir.AxisListType.X,
                                        op=ALU.max)
                nc.vector.tensor_scalar_mul(mneg[:], mx[:], -1.0)
                nc.scalar.activation(ex[:], L[:], AF.Exp, bias=mneg[:],
                                     accum_out=es[:])
                nc.vector.reciprocal(ers[:], es[:])
                nc.vector.tensor_scalar_mul(probs[:], ex[:], ers[:])

            vote_from(None, first_const=1.0 / act_n)
            squash_vote()
            deltas(None, La, first=True)
            softmax(La)
            vote_from(probs)
            squash_vote()
            deltas(La, Lb, first=False)
            softmax(Lb)
            vote_from(probs)

            # vote -> fp8 (*8) -> DRAM -> one gather DMA:
            # vt[u, e*128+s] = votecb[e*384 + 3*s + u]
            vb = rp.tile([NPAIR, S], F8)
            nc.scalar.activation(vb[:], vote[:], AF.Copy, scale=SVT)
            nc.sync.dma_start(
                d_vcb.ap().rearrange("(p s) -> p s", p=NPAIR), vb[:])
            vflat = d_vcb.ap()
            vsrc = bass.AP(vflat.tensor, vflat.offset,
                           [[1, C], [C * S, BL], [C, S]])
            vdst = bass.AP(vt_sb[:].tensor, vt_sb[:].offset,
                           [[vt_sb[:].ap[0][0], C], [S, BL], [1, S]])
            nc.sync.dma_start(vdst, vsrc)

            # ---------------- phase 4: fc1 (fp8 DR, single psum group) -----
            h8 = ap_.tile([128, AK * TOK], F8)

            def fc1_x_streams(ps, ak, col, last_stop):
                """9 DR matmuls: xh*w1h + xl*w1h + xh*w1l into one group."""
                for hp in range(HK // 2):
                    woff = (2 * hp) * A + ak * 128
                    xoff = (2 * hp) * TOK + col
                    lh_h = pair_ap(w1h_sb[:], woff, A, 128)
                    lh_l = pair_ap(w1l_sb[:], woff, A, 128)
                    rh_h = pair_ap(xh_sb[:], xoff, TOK, 512)
                    rh_l = pair_ap(xl_sb[:], xoff, TOK, 512)
                    st = hp == 0
                    nc.tensor.matmul(ps[:], lh_h, rh_h, start=st, stop=False,
                                     perf_mode=PM.DoubleRow)
                    nc.tensor.matmul(ps[:], lh_h, rh_l, start=False, stop=False,
                                     perf_mode=PM.DoubleRow)
                    nc.tensor.matmul(ps[:], lh_l, rh_h, start=False,
                                     stop=(hp == 2 and last_stop),
                                     perf_mode=PM.DoubleRow)

            def m1_step(ps, ak, col, start):
                lh = pair_ap(m1_sb[:], ak * 128, A, 128)
                rh = pair_ap(vt_sb[:], col, TOK, 512)
                nc.tensor.matmul(ps[:], lh, rh, start=start, stop=True,
                                 perf_mode=PM.DoubleRow)

            def fc2_chunk(tt, xt, ot, j):
                """fc2 for token chunk tt (=example tt): h8 x (w2h+w2l)."""
                psa = pp.tile([128, 512], F32, tag="mm", name=f"ps_f2a_{tt}")
                psb = pp.tile([128, 256], F32, tag="mm", name=f"ps_f2b_{tt}")
                for w_sb, first in ((w2h_sb, True), (w2l_sb, False)):
                    for ap8 in range(AK // 2):
                        woff = (2 * ap8) * H
                        hoff = (2 * ap8) * TOK + tt * 128
                        lh = pair_ap(h8[:], hoff, TOK, 128)
                        st = first and ap8 == 0
                        sp_ = (not first) and ap8 == AK // 2 - 1
                        nc.tensor.matmul(psa[:], lh, pair_ap(w_sb[:], woff, H, 512),
                                         start=st, stop=(sp_ and not has_b2),
                                         perf_mode=PM.DoubleRow)
                        nc.tensor.matmul(psb[:], lh,
                                         pair_ap(w_sb[:], woff + 512, H, 256),
                                         start=st, stop=(sp_ and not has_b2),
                                         perf_mode=PM.DoubleRow)
                if has_b2:
                    nc.tensor.matmul(psa[:], ones_bf[:], b2_sb[:, 0:512],
                                     start=False, stop=True)
                    nc.tensor.matmul(psb[:], ones_bf[:], b2_sb[:, 512:H],
                                     start=False, stop=True)
                osl = ot[:, j * H:(j + 1) * H]
                nc.scalar.activation(osl[:, 0:512], psa[:], AF.Relu)
                nc.scalar.activation(osl[:, 512:H], psb[:], AF.Relu)
                nc.vector.tensor_tensor(osl, osl, g2rep[:], op=ALU.mult)
                nc.vector.tensor_tensor(osl, osl, xt[:, j * H:(j + 1) * H],
                                        op=ALU.add)

            def fc2_group(gg):
                """2 fc2 chunks + batched x-in / out DMAs (1 each)."""
                xt = sp.tile([128, 2 * H], BF, tag="x", name=f"x_{gg}", bufs=2)
                nc.sync.dma_start(
                    xt[:].rearrange("p (e h) -> p e h", e=2),
                    d_xown.ap()[2 * gg:2 * gg + 2].rearrange("e p h -> p e h"))
                ot = sp.tile([128, 2 * H], F32, tag="o", name=f"o_{gg}", bufs=2)
                for j in range(2):
                    fc2_chunk(2 * gg + j, xt, ot, j)
                nc.sync.dma_start(
                    d_out.ap()[2 * gg:2 * gg + 2].rearrange("e p h -> p e h"),
                    ot[:].rearrange("p (e h) -> p e h", e=2))

            # fused fc1 blocks (m1 last in each psum group), fc2 per chunk
            for cc in range(4):
                for ak in range(AK):
                    ps = pp.tile([128, 512], F32, tag="mm", name=f"ps_f1f_{cc}_{ak}")
                    fc1_x_streams(ps, ak, cc * 512, last_stop=False)
                    m1_step(ps, ak, cc * 512, start=False)
                    nc.scalar.activation(
                        h8[:, ak * TOK + cc * 512: ak * TOK + (cc + 1) * 512],
                        ps[:], AF.Relu, scale=H8SC, bias=b1_sb[:, ak:ak + 1])
                fc2_group(2 * cc)
                fc2_group(2 * cc + 1)

    nc.compile()
    return nc


# ---------------------------------------------------------------------------
# host marshaling
# ---------------------------------------------------------------------------

def _prep_core_inputs(k, xh_t, xl_t, x, shared):
    # own examples b_i = (48k + 43 i) mod 128; sem examples [16k, 16k+18).
    own = np.array([(48 * k + 43 * i) % B for i in range(BL)])
    sem_ex = np.array([(16 * k + j) % B for j in range(NSEM)])

    def slab(xt):
        return np.ascontiguousarray(
            xt[:, sem_ex, :].reshape(H, STOK).reshape(HK, 128, STOK))

    xsem_h = slab(xh_t)
    xsem_l = slab(xl_t)

    xq_h = np.ascontiguousarray(
        xh_t[:, own, :].reshape(H, TOK).reshape(HK, 128, TOK))
    xq_l = np.ascontiguousarray(
        xl_t[:, own, :].reshape(H, TOK).reshape(HK, 128, TOK))
    x_own = _bf(x[own])

    masks = np.zeros((NPAIR, C), np.float32)
    for i in range(BL):
        for u in range(C):
            t = 3 * int(own[i]) + u
            masks[3 * i + u, t // B] = 1.0

    core = {
        "xsem_h": xsem_h, "xsem_l": xsem_l,
        "xq_h": xq_h, "xq_l": xq_l, "x_own": x_own, "masks": masks,
    }
    core.update({n: shared[n] for n in (
        "sw_cat", "semb", "rw_pack", "w1h", "w1l", "w2h", "w2l",
        "m1q", "b1s", "g2s", "b2row")})
    return core


_CACHE = {}


def _make_shared(inputs):
    fc1_w = np.asarray(inputs["fc1_w"], np.float32)
    fc1_b = np.asarray(inputs["fc1_b"], np.float32)
    fc2_w = np.asarray(inputs["fc2_w"], np.float32)
    fc2_b = np.asarray(inputs["fc2_b"], np.float32)
    efc1 = np.asarray(inputs["efc1"], np.float32)
    efc2 = np.asarray(inputs["efc2"], np.float32)
    sem_w = np.asarray(inputs["sem_w"], np.float32)
    sem_b = np.asarray(inputs["sem_b"], np.float32)
    route_weights = np.asarray(inputs["route_weights"], np.float32)
    larger_w = np.asarray(inputs["larger_w"], np.float32)
    larger_b = np.asarray(inputs["larger_b"], np.float32)
    elarger = np.asarray(inputs["elarger"], np.float32)
    t = int(np.asarray(inputs["t"]))
    sf = np.float32(int(np.asarray(inputs["s"])))
    act_n = t + 1
    x = np.asarray(inputs["x"], np.float32)

    gfc1 = _sigmoid_f32(sf * efc1[t])
    gfc2 = _sigmoid_f32(sf * efc2[t])
    glarger = _sigmoid_f32(sf * elarger[t])

    lwg = (larger_w * glarger[None, :]).astype(np.float32)
    m1mat = (lwg @ fc1_w).astype(np.float32)
    b1 = ((larger_b * glarger) @ fc1_w + fc1_b).astype(np.float32)
    w2g = (fc2_w * gfc1[:, None]).astype(np.float32)
    rw4 = route_weights.reshape(C, N, S, C, S)

    # transposed x quantization (shared across cores; slabs pick columns)
    xt = np.ascontiguousarray(np.transpose(x, (2, 0, 1)))      # [H, B, S]
    xh_t = _q8(xt, SXH).reshape(H, B, S)
    xl_res = xt - xh_t.astype(np.float32) / SXH
    xl_t = _q8(xl_res, SXH).reshape(H, B, S)                   # lo at same scale

    sw2d = np.transpose(sem_w, (1, 0, 2)).reshape(H, NC30)
    swh = _q8(sw2d, SW)
    swl = _q8(sw2d - swh.astype(np.float32) / SW, SW)
    sw_cat = np.concatenate([swh, swl], axis=1)        # [H, 60]
    w1h = _q8(fc1_w, SW)
    w1l = _q8(fc1_w - w1h.astype(np.float32) / SW, SW)
    w2h = _q8(w2g, SW)
    w2l = _q8(w2g - w2h.astype(np.float32) / SW, SW)

    shared = {
        "sw_cat": np.ascontiguousarray(sw_cat.reshape(HK, 128, 2 * NC30)),
        "semb": np.ascontiguousarray(
            (sem_b.reshape(1, NC30) * 32768.0).astype(np.float32)),
        "rw_pack": np.stack([
            np.ascontiguousarray(np.transpose(rw4[c, :act_n], (1, 0, 2, 3))
                                 .reshape(S, act_n * C * S).astype(BF16))
            for c in range(C)]),
        "w1h": w1h.reshape(HK, 128, A),
        "w1l": w1l.reshape(HK, 128, A),
        "w2h": w2h.reshape(AK, 128, H),
        "w2l": w2l.reshape(AK, 128, H),
        "m1q": _q8(m1mat, SM1),
        "b1s": np.ascontiguousarray(
            (b1 * SH8).astype(np.float32).reshape(AK, 128).T),
        "g2s": np.ascontiguousarray((gfc2 * Z1INV).reshape(1, H)),
        "b2row": _bf(fc2_b.reshape(1, H)),
        "_xh_t": xh_t, "_xl_t": xl_t,
        "_has_semb": bool(np.any(sem_b)), "_has_b2": bool(np.any(fc2_b)),
    }
    return shared, act_n


def kernel(**inputs):
    x = np.asarray(inputs["x"], np.float32)
    shared, act_n = _make_shared(inputs)
    key = (act_n, shared["_has_semb"], shared["_has_b2"])
    if key not in _CACHE:
        _CACHE[key] = _build_program(act_n, shared["_has_semb"], shared["_has_b2"])
    nc = _CACHE[key]

    in_maps = [_prep_core_inputs(k, shared["_xh_t"], shared["_xl_t"], x, shared)
               for k in range(NCORES)]
    res = bass_utils.run_bass_kernel_spmd(nc, in_maps, core_ids=list(range(NCORES)))
    out = np.empty((B, S, H), np.float32)
    for k in range(NCORES):
        own = [(48 * k + 43 * i) % B for i in range(BL)]
        out[own] = res.results[k]["out"]
    return out


# revision 34
# speedup vs baseline: 1.7358x; 1.0897x over previous
"""Trainium2 Bass kernel for nn_BertAdapterCapsuleMask (fp8 DoubleRow version).

Strategy (8 NeuronCores, SPMD — identical program, per-core data):

Sharding (same as proven baseline): core k owns routing pairs
t in [48k, 48k+48) and output examples b_i = (48k + 43 i) mod 128, so the
pairs' sem examples are exactly the 18 consecutive [16k, 16k+18) and the
vote rows each core produces are exactly the ones its own examples'
adapter needs — zero cross-core traffic.

Speed: the adapter GEMMs (fc1 768x2048, fc2 2048x768 per 2048 tokens)
run in fp8-e4m3 with MatmulPerfMode.DoubleRow (256-deep contraction,
half-cycle per output column).  Accuracy is restored with residual
("lo") streams quantized at the SAME dequant scale as the hi streams —
fp8's relative precision is scale-invariant, so all streams of one GEMM
accumulate into a single PSUM group with zero combine cost:

  fc1 psum  = xh*w1h + xl*w1h + xh*w1l + vt8*m1q      (all at 2^15)
  fc2 psum  = h8*w2h + h8*w2l                          (at 2^15)

The capsule path keeps near-fp32 accuracy (routing softmax amplifies
sem errors ~10x): sem runs as four fp8 streams with fine residual
scales (separate psums, combines staged through SBUF), squash/routing
stay fp32, priors run bf16.  The capsule term enters fc1 via the
vt8*m1q rank-3 DoubleRow update, so h_out never materializes.

DMA discipline: every DMA instruction costs ~625ns on the single shared
HWDGE descriptor engine, so all bulk tensors move as ONE instruction
each (3D access patterns); the VT gather is a single 3D
gather-descriptor DMA; x/out staging batches 4 examples per DMA.
"""

import numpy as np
import ml_dtypes

import concourse.bass as bass
import concourse.bacc as bacc
import concourse.mybir as mybir
import concourse.tile as tile
from concourse import bass_utils

E4 = ml_dtypes.float8_e4m3
BF16 = ml_dtypes.bfloat16
F32 = mybir.dt.float32
BF = mybir.dt.bfloat16
F8 = mybir.dt.float8e4
AF = mybir.ActivationFunctionType
ALU = mybir.AluOpType
PM = mybir.MatmulPerfMode

B, S, H, A, C, N = 128, 128, 768, 2048, 3, 10
NUM_ITER = 3
NCORES = 8
BL = B // NCORES          # 16 own examples / core
NPAIR = 3 * B // NCORES   # 48 routing pairs / core
HK = H // 128             # 6
AK = A // 128             # 16
TOK = BL * S              # 2048 tokens / core
NC30 = N * C              # 30
NSEM = 18                 # sem examples per core: [16k, 16k+18) mod 128
STOK = NSEM * S           # 2304 sem tokens / core
NPRE = 1                  # fc1 col-chunks (of 512 tokens) via z1p prepass

# fp8 scales.  All adapter streams share dequant 2^-15; sem streams use
# fine residual scales (routing is sensitive) and combine explicitly.
SXH = 16.0
SW = 2048.0
SVT, SM1 = 8.0, 4096.0         # SVT*SM1 == SXH*SW == 2^15
SH8 = 16.0
SXSL = 512.0                   # xsem lo
SSWL = 32768.0                 # sem_w lo
Z1INV = 1.0 / 32768.0          # 2^-15
H8SC = SH8 * Z1INV             # 2^-11


def _sigmoid_f32(z):
    z = np.asarray(z, np.float64)
    return (1.0 / (1.0 + np.exp(-z))).astype(np.float32)


def _q8(a, scale):
    """fp8-e4m3 quantize: stores clip(a*scale, +-240); dequant is 1/scale."""
    z = np.clip(np.asarray(a, np.float32) * np.float32(scale), -240.0, 240.0)
    return np.ascontiguousarray(z.astype(E4))


def _bf(x):
    return np.ascontiguousarray(np.asarray(x, np.float32).astype(BF16))


# ---------------------------------------------------------------------------
# device program
# ---------------------------------------------------------------------------

def _build_program(act_n, has_semb, has_b2):
    nc = bacc.Bacc("TRN2", target_bir_lowering=False, debug=False,
                   num_devices=NCORES)

    d_xsh = nc.dram_tensor("xsem_h", [HK, 128, STOK], F8, kind="ExternalInput")
    d_xsl = nc.dram_tensor("xsem_l", [HK, 128, STOK], F8, kind="ExternalInput")
    d_swc = nc.dram_tensor("sw_cat", [HK, 128, 2 * NC30], F8, kind="ExternalInput")
    d_semb = nc.dram_tensor("semb", [1, NC30], F32, kind="ExternalInput")
    d_rw = nc.dram_tensor("rw_pack", [C, 128, act_n * C * S], BF, kind="ExternalInput")
    d_masks = nc.dram_tensor("masks3", [128, C * NPAIR], F32, kind="ExternalInput")
    d_xh = nc.dram_tensor("xq_h", [HK, 128, TOK], F8, kind="ExternalInput")
    d_xl = nc.dram_tensor("xq_l", [HK, 128, TOK], F8, kind="ExternalInput")
    d_w1h = nc.dram_tensor("w1h", [HK, 128, A], F8, kind="ExternalInput")
    d_w1l = nc.dram_tensor("w1l", [HK, 128, A], F8, kind="ExternalInput")
    d_w2h = nc.dram_tensor("w2h", [AK, 128, H], F8, kind="ExternalInput")
    d_w2l = nc.dram_tensor("w2l", [AK, 128, H], F8, kind="ExternalInput")
    d_m1q = nc.dram_tensor("m1q", [C, A], F8, kind="ExternalInput")
    d_b1s = nc.dram_tensor("b1s", [128, AK], F32, kind="ExternalInput")
    d_g2s = nc.dram_tensor("g2s", [1, H], F32, kind="ExternalInput")
    d_b2 = nc.dram_tensor("b2row", [1, H], BF, kind="ExternalInput")
    d_xown = nc.dram_tensor("x_own", [BL, S, H], BF, kind="ExternalInput")
    d_vcb = nc.dram_tensor("votecb", [NPAIR * S], F8, kind="Internal")
    d_out = nc.dram_tensor("out", [BL, S, H], F32, kind="ExternalOutput")

    def pair_ap(ap, off, stride, n):
        """[[part], [stride, 2], [1, n]] AP at ap.offset+off (DoubleRow pair)."""
        return bass.AP(ap.tensor, ap.offset + off, [ap.ap[0], [stride, 2], [1, n]])

    def merged_load(queue, dst_tile, dram, inner):
        """One-DMA load of [K, 128, inner] dram into [128, K*inner] sbuf."""
        k = dram.shape[0]
        queue.dma_start(
            dst_tile[:].rearrange("p (k c) -> p k c", k=k),
            dram.ap().rearrange("k p c -> p k c"))

    with tile.TileContext(nc) as tc:
        with (
            tc.tile_pool(name="w", bufs=1) as wp,
            tc.tile_pool(name="semx", bufs=1) as sxp,
            tc.tile_pool(name="sem", bufs=1) as smp,
            tc.tile_pool(name="rt", bufs=1) as rp,
            tc.tile_pool(name="ad", bufs=1) as ap_,
            tc.tile_pool(name="st", bufs=2) as sp,
            tc.tile_pool(name="ps", bufs=8, space="PSUM") as pp,
        ):
            # ---------------- persistent loads (one DMA each) --------------
            # ALL bulk DMA on the SP queue in priority order: a DMA holds
            # its SEQ while waiting for a ring credit, so the Act queue
            # must stay DMA-free for compute.
            xsh_sb = sxp.tile([128, HK * STOK], F8)
            xsl_sb = sxp.tile([128, HK * STOK], F8)
            merged_load(nc.sync, xsh_sb, d_xsh, STOK)
            swc_sb = wp.tile([128, HK * 2 * NC30], F8)
            merged_load(nc.sync, swc_sb, d_swc, 2 * NC30)
            merged_load(nc.sync, xsl_sb, d_xsl, STOK)
            semb_sb = wp.tile([1, NC30], F32)
            nc.sync.dma_start(semb_sb[:], d_semb[:])
            masks_sb = wp.tile([128, C * NPAIR], F32)
            nc.sync.dma_start(masks_sb[:], d_masks[:])
            rw_sb = sxp.tile([128, C * act_n * C * S], BF)
            merged_load(nc.sync, rw_sb, d_rw, act_n * C * S)

            # adapter data
            xh_sb = wp.tile([128, HK * TOK], F8, tag="bigx", bufs=1)
            w1h_sb = wp.tile([128, HK * A], F8)
            merged_load(nc.sync, xh_sb, d_xh, TOK)
            merged_load(nc.sync, w1h_sb, d_w1h, A)
            xl_sb = wp.tile([128, HK * TOK], F8, tag="bigx2", bufs=1)
            w1l_sb = wp.tile([128, HK * A], F8)
            merged_load(nc.sync, xl_sb, d_xl, TOK)
            merged_load(nc.sync, w1l_sb, d_w1l, A)

            # m1 lhsT [128, 2*A]: rows 0-2 of k-tile0 hold m1q, rest zero
            m1_sb = wp.tile([128, 2 * A], F8)
            nc.gpsimd.memset(m1_sb[:], 0.0)
            m1_dst = bass.AP(m1_sb[:].tensor, m1_sb[:].offset,
                             [[m1_sb[:].ap[0][0], C], [1, A]])
            nc.sync.dma_start(m1_dst, d_m1q.ap())
            # vt rhs [128, 2*TOK]: rows 0-2 of k-tile0 get the vote gather;
            # everything else zero (multiplied by zero m1 rows).
            vt_sb = ap_.tile([128, 2 * TOK], F8)
            nc.gpsimd.memset(vt_sb[:], 0.0)

            b1_sb = wp.tile([128, AK], F32)
            nc.sync.dma_start(b1_sb[:], d_b1s[:])
            g2rep = wp.tile([128, H], F32)
            g2_src = d_g2s.ap()
            nc.sync.dma_start(
                g2rep[:], bass.AP(g2_src.tensor, g2_src.offset, [[0, 128], [1, H]]))
            w2h_sb = wp.tile([128, AK * H], F8, tag="bigw2", bufs=1)
            w2l_sb = wp.tile([128, AK * H], F8, tag="bigw2l", bufs=1)
            merged_load(nc.sync, w2h_sb, d_w2h, H)
            merged_load(nc.sync, w2l_sb, d_w2l, H)
            ones_f = wp.tile([1, 128], F32)
            nc.gpsimd.memset(ones_f[:], 1.0)
            if has_b2:
                ones_bf = wp.tile([1, 128], BF)
                nc.gpsimd.memset(ones_bf[:], 1.0)
                b2_sb = wp.tile([1, H], BF)
                nc.sync.dma_start(b2_sb[:], d_b2[:])

            # ---------------- phase 1: sem (4 fp8 streams) -----------------
            # swh||swl are concatenated in the rhs, so each DR matmul feeds
            # two streams at once (6 matmuls per slot); 4 slots share one
            # PSUM bank under a single accumulation bracket (start zeroes
            # the whole 2KB zero-region).  hi+lo halves then fold via one
            # Act copy + one strided DVE add per group.
            sem_own = smp.tile([128, NSEM * NC30], F32)
            SEMG = 4
            ngrp = (NSEM + SEMG - 1) // SEMG
            for grp in range(ngrp):
                slots = range(grp * SEMG, min((grp + 1) * SEMG, NSEM))
                nsl = len(slots)
                ps = pp.tile([128, SEMG * 2 * NC30], F32, tag="mm",
                             name=f"ps_sem_{grp}")
                for j, slot in enumerate(slots):
                    for hp in range(HK // 2):
                        off = (2 * hp) * STOK + slot * S
                        lhT_h = pair_ap(xsh_sb[:], off, STOK, 128)
                        lhT_l = pair_ap(xsl_sb[:], off, STOK, 128)
                        rw_c = pair_ap(swc_sb[:], (2 * hp) * 2 * NC30,
                                       2 * NC30, 2 * NC30)
                        dst = ps[:, j * 2 * NC30:(j + 1) * 2 * NC30]
                        st = j == 0 and hp == 0
                        sp_last = (j == nsl - 1 and hp == 2 and not has_semb)
                        nc.tensor.matmul(dst, lhT_h, rw_c, start=st, stop=False,
                                         perf_mode=PM.DoubleRow)
                        nc.tensor.matmul(dst, lhT_l, rw_c, start=False,
                                         stop=sp_last, perf_mode=PM.DoubleRow)
                    if has_semb:
                        nc.tensor.matmul(
                            ps[:, j * 2 * NC30: j * 2 * NC30 + NC30],
                            ones_f[:], semb_sb[:], start=False,
                            stop=(j == nsl - 1))
                # fold hi/lo halves: sem = 2^-15 * (ps[,0:30] + ps[,30:60])
                sc = smp.tile([128, SEMG * 2 * NC30], F32, tag="sc", bufs=2,
                              name=f"sc_{grp}")
                nc.scalar.activation(sc[:, 0:nsl * 2 * NC30],
                                     ps[:, 0:nsl * 2 * NC30],
                                     AF.Copy, scale=Z1INV)
                sc_ap = sc[:]
                hi = bass.AP(sc_ap.tensor, sc_ap.offset,
                             [sc_ap.ap[0], [2 * NC30, nsl], [1, NC30]])
                lo = bass.AP(sc_ap.tensor, sc_ap.offset + NC30,
                             [sc_ap.ap[0], [2 * NC30, nsl], [1, NC30]])
                nc.vector.tensor_tensor(
                    sem_own[:, grp * SEMG * NC30:
                            (grp * SEMG + nsl) * NC30].rearrange(
                        "p (s c) -> p s c", c=NC30),
                    hi, lo, op=ALU.add)

            # ---------------- squash over n (fp32, as baseline) ------------
            sem2 = smp.tile([128, NSEM * NC30], F32)
            nc.vector.tensor_tensor(sem2[:], sem_own[:], sem_own[:], op=ALU.mult)
            sqt = smp.tile([128, NSEM * C], F32)
            nc.vector.tensor_reduce(
                sqt[:].rearrange("p (slot cc) -> p slot cc", cc=C),
                sem2[:].rearrange("p (slot n cc) -> p slot cc n", n=N, cc=C),
                axis=mybir.AxisListType.X, op=ALU.add)
            lnq = smp.tile([128, NSEM * C], F32)
            nc.scalar.activation(lnq[:], sqt[:], AF.Ln)
            sqq = smp.tile([128, NSEM * C], F32)
            nc.scalar.activation(sqq[:], lnq[:], AF.Exp, scale=0.5)
            up = smp.tile([128, NSEM * C], F32)
            nc.vector.tensor_scalar_add(up[:], sqt[:], 1.0)
            ru = smp.tile([128, NSEM * C], F32)
            nc.vector.reciprocal(ru[:], up[:])
            fq = smp.tile([128, NSEM * C], F32)
            nc.vector.tensor_tensor(fq[:], sqq[:], ru[:], op=ALU.mult)
            sem_sq = sem2  # reuse
            f_ap = fq[:]
            f_b = bass.AP(f_ap.tensor, f_ap.offset,
                          [f_ap.ap[0], [C, NSEM], [0, N], [1, C]])
            nc.vector.tensor_tensor(
                sem_sq[:].rearrange("p (slot n cc) -> p slot n cc", n=N, cc=C),
                sem_own[:].rearrange("p (slot n cc) -> p slot n cc", n=N, cc=C),
                f_b, op=ALU.mult)
            # pair-ordered bf16 copies, pre-masked per c'-group g:
            # sem_pair_g[p, (i,u), nc] = sem_sq[p, slot i+u, nc] * mask[3i+u, g]
            sem_pairs = []
            sq_ap = sem_sq[:]
            gather = bass.AP(sq_ap.tensor, sq_ap.offset,
                             [sq_ap.ap[0], [NC30, BL], [NC30, C], [1, NC30]])
            mk_ap = masks_sb[:]
            for g in range(C):
                spg = smp.tile([128, NPAIR * NC30], BF, name=f"sem_pair_{g}")
                sem_pairs.append(spg)
                mk_b = bass.AP(mk_ap.tensor, mk_ap.offset + g * NPAIR,
                               [mk_ap.ap[0], [3, BL], [1, C], [0, NC30]])
                nc.vector.tensor_tensor(
                    spg[:].rearrange("p (i u nc) -> p i u nc", i=BL, u=C),
                    gather, mk_b, op=ALU.mult)

            # ---------------- phase 2: priors (bf16, one psum per n) -------
            priors = rp.tile([NPAIR, act_n * S], F32)
            for n in range(act_n):
                ps = pp.tile([NPAIR, S], F32, tag="mm", name=f"ps_pr_{n}")
                k = 0
                for g in range(C):
                    sem_v = sem_pairs[g][:].rearrange(
                        "p (pair nc) -> p nc pair", nc=NC30)
                    rbase = g * (act_n * C * S) + (n * C) * S
                    for cc in range(C):
                        nc.tensor.matmul(
                            ps[:], sem_v[:, n * C + cc, :],
                            rw_sb[:, rbase + cc * S: rbase + (cc + 1) * S],
                            start=(k == 0), stop=(k == 3 * C - 1))
                        k += 1
                nc.scalar.copy(priors[:, n * S:(n + 1) * S], ps[:])

            # ---------------- phase 3: routing (fp32, as baseline) ---------
            vote = rp.tile([NPAIR, S], F32)
            scr = rp.tile([NPAIR, S], F32)
            La = rp.tile([NPAIR, act_n], F32)
            Lb = rp.tile([NPAIR, act_n], F32)
            sqv = rp.tile([NPAIR, 1], F32)
            lv = rp.tile([NPAIR, 1], F32)
            sv = rp.tile([NPAIR, 1], F32)
            uv = rp.tile([NPAIR, 1], F32)
            rv = rp.tile([NPAIR, 1], F32)
            fv = rp.tile([NPAIR, 1], F32)
            outv = rp.tile([NPAIR, S], F32)
            mx = rp.tile([NPAIR, 1], F32)
            mneg = rp.tile([NPAIR, 1], F32)
            ex = rp.tile([NPAIR, act_n], F32)
            es = rp.tile([NPAIR, 1], F32)
            ers = rp.tile([NPAIR, 1], F32)
            probs = rp.tile([NPAIR, act_n], F32)

            def vote_from(pr_sc, first_const=None):
                for n in range(act_n):
                    blk = priors[:, n * S:(n + 1) * S]
                    sc = first_const if first_const is not None else pr_sc[:, n:n + 1]
                    if n == 0:
                        nc.vector.tensor_scalar_mul(vote[:], blk, sc)
                    else:
                        nc.vector.scalar_tensor_tensor(
                            vote[:], blk, sc, vote[:], op0=ALU.mult, op1=ALU.add)

            def squash_vote():
                nc.vector.tensor_tensor(scr[:], vote[:], vote[:], op=ALU.mult)
                nc.vector.tensor_reduce(sqv[:], scr[:],
                                        axis=mybir.AxisListType.X, op=ALU.add)
                nc.scalar.activation(lv[:], sqv[:], AF.Ln)
                nc.scalar.activation(sv[:], lv[:], AF.Exp, scale=0.5)
                nc.vector.tensor_scalar_add(uv[:], sqv[:], 1.0)
                nc.vector.reciprocal(rv[:], uv[:])
                nc.vector.tensor_tensor(fv[:], sv[:], rv[:], op=ALU.mult)
                nc.vector.tensor_scalar_mul(outv[:], vote[:], fv[:])

            def deltas(Lprev, Lnew, first):
                for n in range(act_n):
                    nc.vector.tensor_tensor(
                        scr[:], priors[:, n * S:(n + 1) * S], outv[:], op=ALU.mult)
                    nc.vector.tensor_reduce(
                        Lnew[:, n:n + 1], scr[:],
                        axis=mybir.AxisListType.X, op=ALU.add)
                if not first:
                    nc.vector.tensor_tensor(Lnew[:], Lnew[:], Lprev[:], op=ALU.add)

            def softmax(L):
                nc.vector.tensor_reduce(mx[:], L[:], axis=mybir.AxisListType.X,
                                        op=ALU.max)
                nc.vector.tensor_scalar_mul(mneg[:], mx[:], -1.0)
                nc.scalar.activation(ex[:], L[:], AF.Exp, bias=mneg[:],
                                     accum_out=es[:])
                nc.vector.reciprocal(ers[:], es[:])
                nc.vector.tensor_scalar_mul(probs[:], ex[:], ers[:])

            vote_from(None, first_const=1.0 / act_n)
            squash_vote()
            deltas(None, La, first=True)
            softmax(La)
            vote_from(probs)
            squash_vote()
            deltas(La, Lb, first=False)
            softmax(Lb)
            vote_from(probs)

            # vote -> fp8 (*8) -> DRAM -> one gather DMA:
            # vt[u, e*128+s] = votecb[e*384 + 3*s + u]
            vb = rp.tile([NPAIR, S], F8)
            nc.scalar.activation(vb[:], vote[:], AF.Copy, scale=SVT)
            nc.scalar.dma_start(
                d_vcb.ap().rearrange("(p s) -> p s", p=NPAIR), vb[:])
            vflat = d_vcb.ap()
            vsrc = bass.AP(vflat.tensor, vflat.offset,
                           [[1, C], [C * S, BL], [C, S]])
            vdst = bass.AP(vt_sb[:].tensor, vt_sb[:].offset,
                           [[vt_sb[:].ap[0][0], C], [S, BL], [1, S]])
            nc.scalar.dma_start(vdst, vsrc)

            # ---------------- phase 4: fc1 (fp8 DR, single psum group) -----
            h8 = ap_.tile([128, AK * TOK], F8)

            def fc1_x_streams(ps, ak, col, last_stop):
                """9 DR matmuls: xh*w1h + xl*w1h + xh*w1l into one group."""
                for hp in range(HK // 2):
                    woff = (2 * hp) * A + ak * 128
                    xoff = (2 * hp) * TOK + col
                    lh_h = pair_ap(w1h_sb[:], woff, A, 128)
                    lh_l = pair_ap(w1l_sb[:], woff, A, 128)
                    rh_h = pair_ap(xh_sb[:], xoff, TOK, 512)
                    rh_l = pair_ap(xl_sb[:], xoff, TOK, 512)
                    st = hp == 0
                    nc.tensor.matmul(ps[:], lh_h, rh_h, start=st, stop=False,
                                     perf_mode=PM.DoubleRow)
                    nc.tensor.matmul(ps[:], lh_h, rh_l, start=False, stop=False,
                                     perf_mode=PM.DoubleRow)
                    nc.tensor.matmul(ps[:], lh_l, rh_h, start=False,
                                     stop=(hp == 2 and last_stop),
                                     perf_mode=PM.DoubleRow)

            def m1_step(ps, ak, col, start):
                lh = pair_ap(m1_sb[:], ak * 128, A, 128)
                rh = pair_ap(vt_sb[:], col, TOK, 512)
                nc.tensor.matmul(ps[:], lh, rh, start=start, stop=True,
                                 perf_mode=PM.DoubleRow)

            def fc2_chunk(tt, xt, ot, j):
                """fc2 for token chunk tt (=example tt): h8 x (w2h+w2l)."""
                psa = pp.tile([128, 512], F32, tag="mm", name=f"ps_f2a_{tt}")
                psb = pp.tile([128, 256], F32, tag="mm", name=f"ps_f2b_{tt}")
                for w_sb, first in ((w2h_sb, True), (w2l_sb, False)):
                    for ap8 in range(AK // 2):
                        woff = (2 * ap8) * H
                        hoff = (2 * ap8) * TOK + tt * 128
                        lh = pair_ap(h8[:], hoff, TOK, 128)
                        st = first and ap8 == 0
                        sp_ = (not first) and ap8 == AK // 2 - 1
                        nc.tensor.matmul(psa[:], lh, pair_ap(w_sb[:], woff, H, 512),
                                         start=st, stop=(sp_ and not has_b2),
                                         perf_mode=PM.DoubleRow)
                        nc.tensor.matmul(psb[:], lh,
                                         pair_ap(w_sb[:], woff + 512, H, 256),
                                         start=st, stop=(sp_ and not has_b2),
                                         perf_mode=PM.DoubleRow)
                if has_b2:
                    nc.tensor.matmul(psa[:], ones_bf[:], b2_sb[:, 0:512],
                                     start=False, stop=True)
                    nc.tensor.matmul(psb[:], ones_bf[:], b2_sb[:, 512:H],
                                     start=False, stop=True)
                osl = ot[:, j * H:(j + 1) * H]
                nc.scalar.activation(osl[:, 0:512], psa[:], AF.Relu)
                nc.scalar.activation(osl[:, 512:H], psb[:], AF.Relu)
                nc.vector.tensor_tensor(osl, osl, g2rep[:], op=ALU.mult)
                nc.vector.tensor_tensor(osl, osl, xt[:, j * H:(j + 1) * H],
                                        op=ALU.add)

            def fc2_group(gg):
                """2 fc2 chunks + batched x-in / out DMAs (1 each)."""
                xt = sp.tile([128, 2 * H], BF, tag="x", name=f"x_{gg}", bufs=2)
                nc.sync.dma_start(
                    xt[:].rearrange("p (e h) -> p e h", e=2),
                    d_xown.ap()[2 * gg:2 * gg + 2].rearrange("e p h -> p e h"))
                ot = sp.tile([128, 2 * H], F32, tag="o", name=f"o_{gg}", bufs=2)
                for j in range(2):
                    fc2_chunk(2 * gg + j, xt, ot, j)
                nc.sync.dma_start(
                    d_out.ap()[2 * gg:2 * gg + 2].rearrange("e p h -> p e h"),
                    ot[:].rearrange("p (e h) -> p e h", e=2))

            # fused fc1 blocks (m1 last in each psum group), fc2 per chunk
            for cc in range(4):
                for ak in range(AK):
                    ps = pp.tile([128, 512], F32, tag="mm", name=f"ps_f1f_{cc}_{ak}")
                    fc1_x_streams(ps, ak, cc * 512, last_stop=False)
                    m1_step(ps, ak, cc * 512, start=False)
                    nc.scalar.activation(
                        h8[:, ak * TOK + cc * 512: ak * TOK + (cc + 1) * 512],
                        ps[:], AF.Relu, scale=H8SC, bias=b1_sb[:, ak:ak + 1])
                fc2_group(2 * cc)
                fc2_group(2 * cc + 1)

    nc.compile()
    return nc


# ---------------------------------------------------------------------------
# host marshaling
# ---------------------------------------------------------------------------

def _prep_core_inputs(k, xh_t, xl_t, x, shared):
    # own examples b_i = (48k + 43 i) mod 128; sem examples [16k, 16k+18).
    own = np.array([(48 * k + 43 * i) % B for i in range(BL)])
    sem_ex = np.array([(16 * k + j) % B for j in range(NSEM)])

    def slab(xt):
        return np.ascontiguousarray(
            xt[:, sem_ex, :].reshape(H, STOK).reshape(HK, 128, STOK))

    xsem_h = slab(xh_t)
    xsem_l = slab(xl_t)

    xq_h = np.ascontiguousarray(
        xh_t[:, own, :].reshape(H, TOK).reshape(HK, 128, TOK))
    xq_l = np.ascontiguousarray(
        xl_t[:, own, :].reshape(H, TOK).reshape(HK, 128, TOK))
    x_own = _bf(x[own])

    masks = np.zeros((C, NPAIR), np.float32)
    for i in range(BL):
        for u in range(C):
            t = 3 * int(own[i]) + u
            masks[t // B, 3 * i + u] = 1.0
    masks3 = np.ascontiguousarray(
        np.broadcast_to(masks.reshape(1, C * NPAIR), (128, C * NPAIR)))

    core = {
        "xsem_h": xsem_h, "xsem_l": xsem_l,
        "xq_h": xq_h, "xq_l": xq_l, "x_own": x_own, "masks3": masks3,
    }
    core.update({n: shared[n] for n in (
        "sw_cat", "semb", "rw_pack", "w1h", "w1l", "w2h", "w2l",
        "m1q", "b1s", "g2s", "b2row")})
    return core


_CACHE = {}


def _make_shared(inputs):
    fc1_w = np.asarray(inputs["fc1_w"], np.float32)
    fc1_b = np.asarray(inputs["fc1_b"], np.float32)
    fc2_w = np.asarray(inputs["fc2_w"], np.float32)
    fc2_b = np.asarray(inputs["fc2_b"], np.float32)
    efc1 = np.asarray(inputs["efc1"], np.float32)
    efc2 = np.asarray(inputs["efc2"], np.float32)
    sem_w = np.asarray(inputs["sem_w"], np.float32)
    sem_b = np.asarray(inputs["sem_b"], np.float32)
    route_weights = np.asarray(inputs["route_weights"], np.float32)
    larger_w = np.asarray(inputs["larger_w"], np.float32)
    larger_b = np.asarray(inputs["larger_b"], np.float32)
    elarger = np.asarray(inputs["elarger"], np.float32)
    t = int(np.asarray(inputs["t"]))
    sf = np.float32(int(np.asarray(inputs["s"])))
    act_n = t + 1
    x = np.asarray(inputs["x"], np.float32)

    gfc1 = _sigmoid_f32(sf * efc1[t])
    gfc2 = _sigmoid_f32(sf * efc2[t])
    glarger = _sigmoid_f32(sf * elarger[t])

    lwg = (larger_w * glarger[None, :]).astype(np.float32)
    m1mat = (lwg @ fc1_w).astype(np.float32)
    b1 = ((larger_b * glarger) @ fc1_w + fc1_b).astype(np.float32)
    w2g = (fc2_w * gfc1[:, None]).astype(np.float32)
    rw4 = route_weights.reshape(C, N, S, C, S)

    # transposed x quantization (shared across cores; slabs pick columns)
    xt = np.ascontiguousarray(np.transpose(x, (2, 0, 1)))      # [H, B, S]
    xh_t = _q8(xt, SXH).reshape(H, B, S)
    xl_res = xt - xh_t.astype(np.float32) / SXH
    xl_t = _q8(xl_res, SXH).reshape(H, B, S)                   # lo at same scale

    sw2d = np.transpose(sem_w, (1, 0, 2)).reshape(H, NC30)
    swh = _q8(sw2d, SW)
    swl = _q8(sw2d - swh.astype(np.float32) / SW, SW)
    sw_cat = np.concatenate([swh, swl], axis=1)        # [H, 60]
    w1h = _q8(fc1_w, SW)
    w1l = _q8(fc1_w - w1h.astype(np.float32) / SW, SW)
    w2h = _q8(w2g, SW)
    w2l = _q8(w2g - w2h.astype(np.float32) / SW, SW)

    shared = {
        "sw_cat": np.ascontiguousarray(sw_cat.reshape(HK, 128, 2 * NC30)),
        "semb": np.ascontiguousarray(
            (sem_b.reshape(1, NC30) * 32768.0).astype(np.float32)),
        "rw_pack": np.stack([
            np.ascontiguousarray(np.transpose(rw4[c, :act_n], (1, 0, 2, 3))
                                 .reshape(S, act_n * C * S).astype(BF16))
            for c in range(C)]),
        "w1h": w1h.reshape(HK, 128, A),
        "w1l": w1l.reshape(HK, 128, A),
        "w2h": w2h.reshape(AK, 128, H),
        "w2l": w2l.reshape(AK, 128, H),
        "m1q": _q8(m1mat, SM1),
        "b1s": np.ascontiguousarray(
            (b1 * SH8).astype(np.float32).reshape(AK, 128).T),
        "g2s": np.ascontiguousarray((gfc2 * Z1INV).reshape(1, H)),
        "b2row": _bf(fc2_b.reshape(1, H)),
        "_xh_t": xh_t, "_xl_t": xl_t,
        "_has_semb": bool(np.any(sem_b)), "_has_b2": bool(np.any(fc2_b)),
    }
    return shared, act_n


def kernel(**inputs):
    x = np.asarray(inputs["x"], np.float32)
    shared, act_n = _make_shared(inputs)
    key = (act_n, shared["_has_semb"], shared["_has_b2"])
    if key not in _CACHE:
        _CACHE[key] = _build_program(act_n, shared["_has_semb"], shared["_has_b2"])
    nc = _CACHE[key]

    in_maps = [_prep_core_inputs(k, shared["_xh_t"], shared["_xl_t"], x, shared)
               for k in range(NCORES)]
    res = bass_utils.run_bass_kernel_spmd(nc, in_maps, core_ids=list(range(NCORES)))
    out = np.empty((B, S, H), np.float32)
    for k in range(NCORES):
        own = [(48 * k + 43 * i) % B for i in range(BL)]
        out[own] = res.results[k]["out"]
    return out


# revision 41
# speedup vs baseline: 1.7851x; 1.0284x over previous
"""Trainium2 Bass kernel for nn_BertAdapterCapsuleMask (fp8 DoubleRow version).

Strategy (8 NeuronCores, SPMD — identical program, per-core data):

Sharding (same as proven baseline): core k owns routing pairs
t in [48k, 48k+48) and output examples b_i = (48k + 43 i) mod 128, so the
pairs' sem examples are exactly the 18 consecutive [16k, 16k+18) and the
vote rows each core produces are exactly the ones its own examples'
adapter needs — zero cross-core traffic.

Speed: the adapter GEMMs (fc1 768x2048, fc2 2048x768 per 2048 tokens)
run in fp8-e4m3 with MatmulPerfMode.DoubleRow (256-deep contraction,
half-cycle per output column).  Accuracy is restored with residual
("lo") streams quantized at the SAME dequant scale as the hi streams —
fp8's relative precision is scale-invariant, so all streams of one GEMM
accumulate into a single PSUM group with zero combine cost:

  fc1 psum  = xh*w1h + xl*w1h + xh*w1l + vt8*m1q      (all at 2^15)
  fc2 psum  = h8*w2h + h8*w2l                          (at 2^15)

The capsule path keeps near-fp32 accuracy (routing softmax amplifies
sem errors ~10x): sem runs as four fp8 streams with fine residual
scales (separate psums, combines staged through SBUF), squash/routing
stay fp32, priors run bf16.  The capsule term enters fc1 via the
vt8*m1q rank-3 DoubleRow update, so h_out never materializes.

DMA discipline: every DMA instruction costs ~625ns on the single shared
HWDGE descriptor engine, so all bulk tensors move as ONE instruction
each (3D access patterns); the VT gather is a single 3D
gather-descriptor DMA; x/out staging batches 4 examples per DMA.
"""

import numpy as np
import ml_dtypes

import concourse.bass as bass
import concourse.bacc as bacc
import concourse.mybir as mybir
import concourse.tile as tile
from concourse import bass_utils

E4 = ml_dtypes.float8_e4m3
BF16 = ml_dtypes.bfloat16
F32 = mybir.dt.float32
BF = mybir.dt.bfloat16
F8 = mybir.dt.float8e4
AF = mybir.ActivationFunctionType
ALU = mybir.AluOpType
PM = mybir.MatmulPerfMode

B, S, H, A, C, N = 128, 128, 768, 2048, 3, 10
NUM_ITER = 3
NCORES = 8
BL = B // NCORES          # 16 own examples / core
NPAIR = 3 * B // NCORES   # 48 routing pairs / core
HK = H // 128             # 6
AK = A // 128             # 16
TOK = BL * S              # 2048 tokens / core
NC30 = N * C              # 30
NSEM = 18                 # sem examples per core: [16k, 16k+18) mod 128
STOK = NSEM * S           # 2304 sem tokens / core
NPRE = 1                  # fc1 col-chunks (of 512 tokens) via z1p prepass

# fp8 scales.  All adapter streams share dequant 2^-15; sem streams use
# fine residual scales (routing is sensitive) and combine explicitly.
SXH = 16.0
SW = 2048.0
SVT, SM1 = 8.0, 4096.0         # SVT*SM1 == SXH*SW == 2^15
SH8 = 16.0
SXSL = 512.0                   # xsem lo
SSWL = 32768.0                 # sem_w lo
Z1INV = 1.0 / 32768.0          # 2^-15
H8SC = SH8 * Z1INV             # 2^-11


def _sigmoid_f32(z):
    z = np.asarray(z, np.float64)
    return (1.0 / (1.0 + np.exp(-z))).astype(np.float32)


def _q8(a, scale):
    """fp8-e4m3 quantize: stores clip(a*scale, +-240); dequant is 1/scale."""
    z = np.clip(np.asarray(a, np.float32) * np.float32(scale), -240.0, 240.0)
    return np.ascontiguousarray(z.astype(E4))


def _bf(x):
    return np.ascontiguousarray(np.asarray(x, np.float32).astype(BF16))


# ---------------------------------------------------------------------------
# device program
# ---------------------------------------------------------------------------

def _build_program(act_n, has_semb, has_b2):
    nc = bacc.Bacc("TRN2", target_bir_lowering=False, debug=False,
                   num_devices=NCORES)

    d_xsh = nc.dram_tensor("xsem_h", [HK, 128, STOK], F8, kind="ExternalInput")
    d_xsl = nc.dram_tensor("xsem_l", [HK, 128, STOK], F8, kind="ExternalInput")
    d_swc = nc.dram_tensor("sw_cat", [HK, 128, 2 * NC30], F8, kind="ExternalInput")
    d_semb = nc.dram_tensor("semb", [1, NC30], F32, kind="ExternalInput")
    d_rw = nc.dram_tensor("rw_pack", [C, 128, act_n * C * S], BF, kind="ExternalInput")
    d_masks = nc.dram_tensor("masks3", [128, C * NPAIR], F32, kind="ExternalInput")
    d_xh = nc.dram_tensor("xq_h", [HK, 128, TOK], F8, kind="ExternalInput")
    d_xl = nc.dram_tensor("xq_l", [HK, 128, TOK], F8, kind="ExternalInput")
    d_w1h = nc.dram_tensor("w1h", [HK, 128, A], F8, kind="ExternalInput")
    d_w1l = nc.dram_tensor("w1l", [HK, 128, A], F8, kind="ExternalInput")
    d_w2h = nc.dram_tensor("w2h", [AK, 128, H], F8, kind="ExternalInput")
    d_w2l = nc.dram_tensor("w2l", [AK, 128, H], F8, kind="ExternalInput")
    d_m1q = nc.dram_tensor("m1q", [C, A], F8, kind="ExternalInput")
    d_b1s = nc.dram_tensor("b1s", [128, AK], F32, kind="ExternalInput")
    d_g2s = nc.dram_tensor("g2s", [1, H], F32, kind="ExternalInput")
    d_b2 = nc.dram_tensor("b2row", [1, H], BF, kind="ExternalInput")
    d_xown = nc.dram_tensor("x_own", [BL, S, H], BF, kind="ExternalInput")
    d_vcb = nc.dram_tensor("votecb", [NPAIR * S + 8], F8, kind="Internal")
    d_out = nc.dram_tensor("out", [BL, S, H], F32, kind="ExternalOutput")

    def pair_ap(ap, off, stride, n):
        """[[part], [stride, 2], [1, n]] AP at ap.offset+off (DoubleRow pair)."""
        return bass.AP(ap.tensor, ap.offset + off, [ap.ap[0], [stride, 2], [1, n]])

    def merged_load(queue, dst_tile, dram, inner):
        """One-DMA load of [K, 128, inner] dram into [128, K*inner] sbuf."""
        k = dram.shape[0]
        queue.dma_start(
            dst_tile[:].rearrange("p (k c) -> p k c", k=k),
            dram.ap().rearrange("k p c -> p k c"))

    with tile.TileContext(nc) as tc:
        with (
            tc.tile_pool(name="w", bufs=1) as wp,
            tc.tile_pool(name="semx", bufs=1) as sxp,
            tc.tile_pool(name="sem", bufs=1) as smp,
            tc.tile_pool(name="rt", bufs=1) as rp,
            tc.tile_pool(name="ad", bufs=1) as ap_,
            tc.tile_pool(name="st", bufs=2) as sp,
            tc.tile_pool(name="ps", bufs=8, space="PSUM") as pp,
        ):
            # ---------------- persistent loads (one DMA each) --------------
            # ALL bulk DMA on the SP queue in priority order: a DMA holds
            # its SEQ while waiting for a ring credit, so the Act queue
            # must stay DMA-free for compute.
            xsh_sb = sxp.tile([128, HK * STOK], F8)
            xsl_sb = sxp.tile([128, HK * STOK], F8)
            merged_load(nc.sync, xsh_sb, d_xsh, STOK)
            swc_sb = wp.tile([128, HK * 2 * NC30], F8)
            merged_load(nc.sync, swc_sb, d_swc, 2 * NC30)
            merged_load(nc.sync, xsl_sb, d_xsl, STOK)
            semb_sb = wp.tile([1, NC30], F32)
            nc.sync.dma_start(semb_sb[:], d_semb[:])
            masks_sb = wp.tile([128, C * NPAIR], F32)
            nc.sync.dma_start(masks_sb[:], d_masks[:])
            rw_sb = sxp.tile([128, C * act_n * C * S], BF)
            merged_load(nc.sync, rw_sb, d_rw, act_n * C * S)

            # adapter data
            xh_sb = wp.tile([128, HK * TOK], F8, tag="bigx", bufs=1)
            w1h_sb = wp.tile([128, HK * A], F8)
            merged_load(nc.sync, xh_sb, d_xh, TOK)
            merged_load(nc.sync, w1h_sb, d_w1h, A)
            xl_sb = wp.tile([128, HK * TOK], F8, tag="bigx2", bufs=1)
            w1l_sb = wp.tile([128, HK * A], F8)
            merged_load(nc.sync, xl_sb, d_xl, TOK)
            merged_load(nc.sync, w1l_sb, d_w1l, A)

            # m1 lhsT [128, 2*A]: rows 0-2 of k-tile0 hold m1q, rest zero
            m1_sb = wp.tile([128, 2 * A], F8)
            nc.gpsimd.memset(m1_sb[:], 0.0)
            m1_dst = bass.AP(m1_sb[:].tensor, m1_sb[:].offset,
                             [[m1_sb[:].ap[0][0], C], [1, A]])
            nc.sync.dma_start(m1_dst, d_m1q.ap())
            # vt rhs: partition c (c<3) holds the flat vote buffer shifted
            # by c bytes; other partitions hit zero m1 rows (memset for the
            # simulator's uninitialized-read check).
            vt_sb = ap_.tile([128, NPAIR * S], F8)
            nc.gpsimd.memset(vt_sb[:], 0.0)

            b1_sb = wp.tile([128, AK], F32)
            nc.sync.dma_start(b1_sb[:], d_b1s[:])
            g2rep = wp.tile([128, H], F32)
            g2_src = d_g2s.ap()
            nc.sync.dma_start(
                g2rep[:], bass.AP(g2_src.tensor, g2_src.offset, [[0, 128], [1, H]]))
            w2h_sb = wp.tile([128, AK * H], F8, tag="bigw2", bufs=1)
            w2l_sb = wp.tile([128, AK * H], F8, tag="bigw2l", bufs=1)
            merged_load(nc.sync, w2h_sb, d_w2h, H)
            merged_load(nc.sync, w2l_sb, d_w2l, H)
            ones_f = wp.tile([1, 128], F32)
            nc.gpsimd.memset(ones_f[:], 1.0)
            if has_b2:
                ones_bf = wp.tile([1, 128], BF)
                nc.gpsimd.memset(ones_bf[:], 1.0)
                b2_sb = wp.tile([1, H], BF)
                nc.sync.dma_start(b2_sb[:], d_b2[:])

            # ---------------- phase 1: sem (4 fp8 streams) -----------------
            # swh||swl are concatenated in the rhs, so each DR matmul feeds
            # two streams at once (6 matmuls per slot); 4 slots share one
            # PSUM bank under a single accumulation bracket (start zeroes
            # the whole 2KB zero-region).  hi+lo halves then fold via one
            # Act copy + one strided DVE add per group.
            sem_own = smp.tile([128, NSEM * NC30], F32)
            SEMG = 4
            ngrp = (NSEM + SEMG - 1) // SEMG
            for grp in range(ngrp):
                slots = range(grp * SEMG, min((grp + 1) * SEMG, NSEM))
                nsl = len(slots)
                ps = pp.tile([128, SEMG * 2 * NC30], F32, tag="mm",
                             name=f"ps_sem_{grp}")
                for j, slot in enumerate(slots):
                    for hp in range(HK // 2):
                        off = (2 * hp) * STOK + slot * S
                        lhT_h = pair_ap(xsh_sb[:], off, STOK, 128)
                        lhT_l = pair_ap(xsl_sb[:], off, STOK, 128)
                        rw_c = pair_ap(swc_sb[:], (2 * hp) * 2 * NC30,
                                       2 * NC30, 2 * NC30)
                        dst = ps[:, j * 2 * NC30:(j + 1) * 2 * NC30]
                        st = j == 0 and hp == 0
                        sp_last = (j == nsl - 1 and hp == 2 and not has_semb)
                        nc.tensor.matmul(dst, lhT_h, rw_c, start=st, stop=False,
                                         perf_mode=PM.DoubleRow)
                        nc.tensor.matmul(dst, lhT_l, rw_c, start=False,
                                         stop=sp_last, perf_mode=PM.DoubleRow)
                    if has_semb:
                        nc.tensor.matmul(
                            ps[:, j * 2 * NC30: j * 2 * NC30 + NC30],
                            ones_f[:], semb_sb[:], start=False,
                            stop=(j == nsl - 1))
                # fold hi/lo halves: sem = 2^-15 * (ps[,0:30] + ps[,30:60])
                sc = smp.tile([128, SEMG * 2 * NC30], F32, tag="sc", bufs=2,
                              name=f"sc_{grp}")
                nc.scalar.activation(sc[:, 0:nsl * 2 * NC30],
                                     ps[:, 0:nsl * 2 * NC30],
                                     AF.Copy, scale=Z1INV)
                sc_ap = sc[:]
                hi = bass.AP(sc_ap.tensor, sc_ap.offset,
                             [sc_ap.ap[0], [2 * NC30, nsl], [1, NC30]])
                lo = bass.AP(sc_ap.tensor, sc_ap.offset + NC30,
                             [sc_ap.ap[0], [2 * NC30, nsl], [1, NC30]])
                nc.vector.tensor_tensor(
                    sem_own[:, grp * SEMG * NC30:
                            (grp * SEMG + nsl) * NC30].rearrange(
                        "p (s c) -> p s c", c=NC30),
                    hi, lo, op=ALU.add)

            # ---------------- squash over n (fp32, as baseline) ------------
            sem2 = smp.tile([128, NSEM * NC30], F32)
            nc.vector.tensor_tensor(sem2[:], sem_own[:], sem_own[:], op=ALU.mult)
            sqt = smp.tile([128, NSEM * C], F32)
            nc.vector.tensor_reduce(
                sqt[:].rearrange("p (slot cc) -> p slot cc", cc=C),
                sem2[:].rearrange("p (slot n cc) -> p slot cc n", n=N, cc=C),
                axis=mybir.AxisListType.X, op=ALU.add)
            lnq = smp.tile([128, NSEM * C], F32)
            nc.scalar.activation(lnq[:], sqt[:], AF.Ln)
            sqq = smp.tile([128, NSEM * C], F32)
            nc.scalar.activation(sqq[:], lnq[:], AF.Exp, scale=0.5)
            up = smp.tile([128, NSEM * C], F32)
            nc.vector.tensor_scalar_add(up[:], sqt[:], 1.0)
            ru = smp.tile([128, NSEM * C], F32)
            nc.vector.reciprocal(ru[:], up[:])
            fq = smp.tile([128, NSEM * C], F32)
            nc.vector.tensor_tensor(fq[:], sqq[:], ru[:], op=ALU.mult)
            sem_sq = sem2  # reuse
            f_ap = fq[:]
            f_b = bass.AP(f_ap.tensor, f_ap.offset,
                          [f_ap.ap[0], [C, NSEM], [0, N], [1, C]])
            nc.vector.tensor_tensor(
                sem_sq[:].rearrange("p (slot n cc) -> p slot n cc", n=N, cc=C),
                sem_own[:].rearrange("p (slot n cc) -> p slot n cc", n=N, cc=C),
                f_b, op=ALU.mult)
            # pair-ordered bf16 copies, pre-masked per c'-group g:
            # sem_pair_g[p, (i,u), nc] = sem_sq[p, slot i+u, nc] * mask[3i+u, g]
            sem_pairs = []
            sq_ap = sem_sq[:]
            gather = bass.AP(sq_ap.tensor, sq_ap.offset,
                             [sq_ap.ap[0], [NC30, BL], [NC30, C], [1, NC30]])
            mk_ap = masks_sb[:]
            for g in range(C):
                spg = smp.tile([128, NPAIR * NC30], BF, name=f"sem_pair_{g}")
                sem_pairs.append(spg)
                mk_b = bass.AP(mk_ap.tensor, mk_ap.offset + g * NPAIR,
                               [mk_ap.ap[0], [3, BL], [1, C], [0, NC30]])
                nc.vector.tensor_tensor(
                    spg[:].rearrange("p (i u nc) -> p i u nc", i=BL, u=C),
                    gather, mk_b, op=ALU.mult)

            # ---------------- phase 2: priors (bf16, one psum per n) -------
            priors = rp.tile([NPAIR, act_n * S], F32)
            for n in range(act_n):
                ps = pp.tile([NPAIR, S], F32, tag="mm", name=f"ps_pr_{n}")
                k = 0
                for g in range(C):
                    sem_v = sem_pairs[g][:].rearrange(
                        "p (pair nc) -> p nc pair", nc=NC30)
                    rbase = g * (act_n * C * S) + (n * C) * S
                    for cc in range(C):
                        nc.tensor.matmul(
                            ps[:], sem_v[:, n * C + cc, :],
                            rw_sb[:, rbase + cc * S: rbase + (cc + 1) * S],
                            start=(k == 0), stop=(k == 3 * C - 1))
                        k += 1
                nc.scalar.copy(priors[:, n * S:(n + 1) * S], ps[:])

            # ---------------- phase 3: routing (fp32, as baseline) ---------
            vote = rp.tile([NPAIR, S], F32)
            scr = rp.tile([NPAIR, S], F32)
            La = rp.tile([NPAIR, act_n], F32)
            Lb = rp.tile([NPAIR, act_n], F32)
            sqv = rp.tile([NPAIR, 1], F32)
            lv = rp.tile([NPAIR, 1], F32)
            sv = rp.tile([NPAIR, 1], F32)
            uv = rp.tile([NPAIR, 1], F32)
            rv = rp.tile([NPAIR, 1], F32)
            fv = rp.tile([NPAIR, 1], F32)
            outv = rp.tile([NPAIR, S], F32)
            mx = rp.tile([NPAIR, 1], F32)
            mneg = rp.tile([NPAIR, 1], F32)
            ex = rp.tile([NPAIR, act_n], F32)
            es = rp.tile([NPAIR, 1], F32)
            ers = rp.tile([NPAIR, 1], F32)
            probs = rp.tile([NPAIR, act_n], F32)

            scrT = rp.tile([NPAIR, act_n * S], F32)   # [pair, s*act_n + n]

            def vote_from(pr_sc, first_const=None):
                """vote[p,s] = sum_n probs[p,n] * priors[p,n,s] in 2 wide ops:
                scrT[p, s*an+n] = priors[p,n,s]*probs[p,n]; reduce over n."""
                pr_ap = priors[:]
                pr_v = bass.AP(pr_ap.tensor, pr_ap.offset,
                               [pr_ap.ap[0], [S, act_n], [1, S]])
                if first_const is not None:
                    sc_v = None
                else:
                    sc_ap = pr_sc[:]
                    sc_v = bass.AP(sc_ap.tensor, sc_ap.offset,
                                   [sc_ap.ap[0], [1, act_n], [0, S]])
                dstT = bass.AP(scrT[:].tensor, scrT[:].offset,
                               [scrT[:].ap[0], [1, act_n], [act_n, S]])
                if first_const is not None:
                    nc.vector.tensor_scalar_mul(dstT, pr_v, first_const)
                else:
                    nc.vector.tensor_tensor(dstT, pr_v, sc_v, op=ALU.mult)
                nc.vector.tensor_reduce(
                    vote[:].rearrange("p (s one) -> p s one", one=1),
                    scrT[:].rearrange("p (s n) -> p s n", n=act_n),
                    axis=mybir.AxisListType.X, op=ALU.add)

            def squash_vote():
                nc.vector.tensor_tensor(scr[:], vote[:], vote[:], op=ALU.mult)
                nc.vector.tensor_reduce(sqv[:], scr[:],
                                        axis=mybir.AxisListType.X, op=ALU.add)
                nc.scalar.activation(lv[:], sqv[:], AF.Ln)
                nc.scalar.activation(sv[:], lv[:], AF.Exp, scale=0.5)
                nc.vector.tensor_scalar_add(uv[:], sqv[:], 1.0)
                nc.vector.reciprocal(rv[:], uv[:])
                nc.vector.tensor_tensor(fv[:], sv[:], rv[:], op=ALU.mult)
                nc.vector.tensor_scalar_mul(outv[:], vote[:], fv[:])

            scrA = rp.tile([NPAIR, act_n * S], F32)

            def deltas(Lprev, Lnew, first):
                """L[p,n] = sum_s priors[p,n,s]*outv[p,s] in 2 wide ops."""
                ov_ap = outv[:]
                ov_b = bass.AP(ov_ap.tensor, ov_ap.offset,
                               [ov_ap.ap[0], [0, act_n], [1, S]])
                nc.vector.tensor_tensor(
                    scrA[:].rearrange("p (n s) -> p n s", n=act_n),
                    priors[:].rearrange("p (n s) -> p n s", n=act_n),
                    ov_b, op=ALU.mult)
                nc.vector.tensor_reduce(
                    Lnew[:].rearrange("p (n one) -> p n one", one=1),
                    scrA[:].rearrange("p (n s) -> p n s", n=act_n),
                    axis=mybir.AxisListType.X, op=ALU.add)
                if not first:
                    nc.vector.tensor_tensor(Lnew[:], Lnew[:], Lprev[:], op=ALU.add)

            def softmax(L):
                nc.vector.tensor_reduce(mx[:], L[:], axis=mybir.AxisListType.X,
                                        op=ALU.max)
                nc.vector.tensor_scalar_mul(mneg[:], mx[:], -1.0)
                nc.scalar.activation(ex[:], L[:], AF.Exp, bias=mneg[:],
                                     accum_out=es[:])
                nc.vector.reciprocal(ers[:], es[:])
                nc.vector.tensor_scalar_mul(probs[:], ex[:], ers[:])

            vote_from(None, first_const=1.0 / act_n)
            squash_vote()
            deltas(None, La, first=True)
            softmax(La)
            vote_from(probs)
            squash_vote()
            deltas(La, Lb, first=False)
            softmax(Lb)
            vote_from(probs)

            # vote -> fp8 (*8) -> DRAM -> one gather DMA:
            # vt[u, e*128+s] = votecb[e*384 + 3*s + u]
            # vote -> fp8 -> flat DRAM (one contiguous write).  The capsule
            # coefficient of token t, channel c is flat[3t+c], so vt_sb
            # partition c holds flat[c:] (three contiguous byte-shifted
            # reads) and the m1 matmul reads it with column stride 3.
            vb = rp.tile([NPAIR, S], F8)
            nc.scalar.activation(vb[:], vote[:], AF.Copy, scale=SVT)
            vflat = d_vcb.ap()
            nc.scalar.dma_start(
                bass.AP(vflat.tensor, vflat.offset, [[S, NPAIR], [1, S]]),
                vb[:])
            vpitch = vt_sb[:].ap[0][0]
            for u in range(C):
                vdst = bass.AP(vt_sb[:].tensor, vt_sb[:].offset + u * vpitch,
                               [[vpitch, 1], [1, NPAIR * S]])
                vsrc2 = bass.AP(vflat.tensor, vflat.offset + u,
                                [[NPAIR * S, 1], [1, NPAIR * S]])
                nc.scalar.dma_start(vdst, vsrc2)

            # ---------------- phase 4: fc1 (fp8 DR, single psum group) -----
            h8 = ap_.tile([128, AK * TOK], F8)

            def fc1_x_streams(ps, ak, col, last_stop):
                """9 DR matmuls: xh*w1h + xl*w1h + xh*w1l into one group."""
                for hp in range(HK // 2):
                    woff = (2 * hp) * A + ak * 128
                    xoff = (2 * hp) * TOK + col
                    lh_h = pair_ap(w1h_sb[:], woff, A, 128)
                    lh_l = pair_ap(w1l_sb[:], woff, A, 128)
                    rh_h = pair_ap(xh_sb[:], xoff, TOK, 512)
                    rh_l = pair_ap(xl_sb[:], xoff, TOK, 512)
                    st = hp == 0
                    nc.tensor.matmul(ps[:], lh_h, rh_h, start=st, stop=False,
                                     perf_mode=PM.DoubleRow)
                    nc.tensor.matmul(ps[:], lh_h, rh_l, start=False, stop=False,
                                     perf_mode=PM.DoubleRow)
                    nc.tensor.matmul(ps[:], lh_l, rh_h, start=False,
                                     stop=(hp == 2 and last_stop),
                                     perf_mode=PM.DoubleRow)

            def m1_step(ps, ak, col, start):
                lh = pair_ap(m1_sb[:], ak * 128, A, 128)
                vt_ap = vt_sb[:]
                rh = bass.AP(vt_ap.tensor, vt_ap.offset + 3 * col,
                             [vt_ap.ap[0], [0, 2], [3, 512]])
                nc.tensor.matmul(ps[:], lh, rh, start=start, stop=True,
                                 perf_mode=PM.DoubleRow)

            def fc2_chunk(tt, xt, ot, j):
                """fc2 for token chunk tt (=example tt): h8 x (w2h+w2l)."""
                psa = pp.tile([128, 512], F32, tag="mm", name=f"ps_f2a_{tt}")
                psb = pp.tile([128, 256], F32, tag="mm", name=f"ps_f2b_{tt}")
                for w_sb, first in ((w2h_sb, True), (w2l_sb, False)):
                    for ap8 in range(AK // 2):
                        woff = (2 * ap8) * H
                        hoff = (2 * ap8) * TOK + tt * 128
                        lh = pair_ap(h8[:], hoff, TOK, 128)
                        st = first and ap8 == 0
                        sp_ = (not first) and ap8 == AK // 2 - 1
                        nc.tensor.matmul(psa[:], lh, pair_ap(w_sb[:], woff, H, 512),
                                         start=st, stop=(sp_ and not has_b2),
                                         perf_mode=PM.DoubleRow)
                        nc.tensor.matmul(psb[:], lh,
                                         pair_ap(w_sb[:], woff + 512, H, 256),
                                         start=st, stop=(sp_ and not has_b2),
                                         perf_mode=PM.DoubleRow)
                if has_b2:
                    nc.tensor.matmul(psa[:], ones_bf[:], b2_sb[:, 0:512],
                                     start=False, stop=True)
                    nc.tensor.matmul(psb[:], ones_bf[:], b2_sb[:, 512:H],
                                     start=False, stop=True)
                osl = ot[:, j * H:(j + 1) * H]
                nc.scalar.activation(osl[:, 0:512], psa[:], AF.Relu)
                nc.scalar.activation(osl[:, 512:H], psb[:], AF.Relu)
                nc.vector.tensor_tensor(osl, osl, g2rep[:], op=ALU.mult)
                nc.vector.tensor_tensor(osl, osl, xt[:, j * H:(j + 1) * H],
                                        op=ALU.add)

            def fc2_group(gg):
                """2 fc2 chunks + batched x-in / out DMAs (1 each)."""
                xt = sp.tile([128, 2 * H], BF, tag="x", name=f"x_{gg}", bufs=2)
                nc.sync.dma_start(
                    xt[:].rearrange("p (e h) -> p e h", e=2),
                    d_xown.ap()[2 * gg:2 * gg + 2].rearrange("e p h -> p e h"))
                ot = sp.tile([128, 2 * H], F32, tag="o", name=f"o_{gg}", bufs=2)
                for j in range(2):
                    fc2_chunk(2 * gg + j, xt, ot, j)
                nc.sync.dma_start(
                    d_out.ap()[2 * gg:2 * gg + 2].rearrange("e p h -> p e h"),
                    ot[:].rearrange("p (e h) -> p e h", e=2))

            # fused fc1 blocks (m1 last in each psum group), fc2 per chunk
            for cc in range(4):
                for ak in range(AK):
                    ps = pp.tile([128, 512], F32, tag="mm", name=f"ps_f1f_{cc}_{ak}")
                    fc1_x_streams(ps, ak, cc * 512, last_stop=False)
                    m1_step(ps, ak, cc * 512, start=False)
                    nc.scalar.activation(
                        h8[:, ak * TOK + cc * 512: ak * TOK + (cc + 1) * 512],
                        ps[:], AF.Relu, scale=H8SC, bias=b1_sb[:, ak:ak + 1])
                fc2_group(2 * cc)
                fc2_group(2 * cc + 1)

    nc.compile()
    return nc


# ---------------------------------------------------------------------------
# host marshaling
# ---------------------------------------------------------------------------

def _prep_core_inputs(k, xh_t, xl_t, x, shared):
    # own examples b_i = (48k + 43 i) mod 128; sem examples [16k, 16k+18).
    own = np.array([(48 * k + 43 * i) % B for i in range(BL)])
    sem_ex = np.array([(16 * k + j) % B for j in range(NSEM)])

    def slab(xt):
        return np.ascontiguousarray(
            xt[:, sem_ex, :].reshape(H, STOK).reshape(HK, 128, STOK))

    xsem_h = slab(xh_t)
    xsem_l = slab(xl_t)

    xq_h = np.ascontiguousarray(
        xh_t[:, own, :].reshape(H, TOK).reshape(HK, 128, TOK))
    xq_l = np.ascontiguousarray(
        xl_t[:, own, :].reshape(H, TOK).reshape(HK, 128, TOK))
    x_own = _bf(x[own])

    masks = np.zeros((C, NPAIR), np.float32)
    for i in range(BL):
        for u in range(C):
            t = 3 * int(own[i]) + u
            masks[t // B, 3 * i + u] = 1.0
    masks3 = np.ascontiguousarray(
        np.broadcast_to(masks.reshape(1, C * NPAIR), (128, C * NPAIR)))

    core = {
        "xsem_h": xsem_h, "xsem_l": xsem_l,
        "xq_h": xq_h, "xq_l": xq_l, "x_own": x_own, "masks3": masks3,
    }
    core.update({n: shared[n] for n in (
        "sw_cat", "semb", "rw_pack", "w1h", "w1l", "w2h", "w2l",
        "m1q", "b1s", "g2s", "b2row")})
    return core


_CACHE = {}


def _make_shared(inputs):
    fc1_w = np.asarray(inputs["fc1_w"], np.float32)
    fc1_b = np.asarray(inputs["fc1_b"], np.float32)
    fc2_w = np.asarray(inputs["fc2_w"], np.float32)
    fc2_b = np.asarray(inputs["fc2_b"], np.float32)
    efc1 = np.asarray(inputs["efc1"], np.float32)
    efc2 = np.asarray(inputs["efc2"], np.float32)
    sem_w = np.asarray(inputs["sem_w"], np.float32)
    sem_b = np.asarray(inputs["sem_b"], np.float32)
    route_weights = np.asarray(inputs["route_weights"], np.float32)
    larger_w = np.asarray(inputs["larger_w"], np.float32)
    larger_b = np.asarray(inputs["larger_b"], np.float32)
    elarger = np.asarray(inputs["elarger"], np.float32)
    t = int(np.asarray(inputs["t"]))
    sf = np.float32(int(np.asarray(inputs["s"])))
    act_n = t + 1
    x = np.asarray(inputs["x"], np.float32)

    gfc1 = _sigmoid_f32(sf * efc1[t])
    gfc2 = _sigmoid_f32(sf * efc2[t])
    glarger = _sigmoid_f32(sf * elarger[t])

    lwg = (larger_w * glarger[None, :]).astype(np.float32)
    m1mat = (lwg @ fc1_w).astype(np.float32)
    b1 = ((larger_b * glarger) @ fc1_w + fc1_b).astype(np.float32)
    w2g = (fc2_w * gfc1[:, None]).astype(np.float32)
    rw4 = route_weights.reshape(C, N, S, C, S)

    # transposed x quantization (shared across cores; slabs pick columns)
    xt = np.ascontiguousarray(np.transpose(x, (2, 0, 1)))      # [H, B, S]
    xh_t = _q8(xt, SXH).reshape(H, B, S)
    xl_res = xt - xh_t.astype(np.float32) / SXH
    xl_t = _q8(xl_res, SXH).reshape(H, B, S)                   # lo at same scale

    sw2d = np.transpose(sem_w, (1, 0, 2)).reshape(H, NC30)
    swh = _q8(sw2d, SW)
    swl = _q8(sw2d - swh.astype(np.float32) / SW, SW)
    sw_cat = np.concatenate([swh, swl], axis=1)        # [H, 60]
    w1h = _q8(fc1_w, SW)
    w1l = _q8(fc1_w - w1h.astype(np.float32) / SW, SW)
    w2h = _q8(w2g, SW)
    w2l = _q8(w2g - w2h.astype(np.float32) / SW, SW)

    shared = {
        "sw_cat": np.ascontiguousarray(sw_cat.reshape(HK, 128, 2 * NC30)),
        "semb": np.ascontiguousarray(
            (sem_b.reshape(1, NC30) * 32768.0).astype(np.float32)),
        "rw_pack": np.stack([
            np.ascontiguousarray(np.transpose(rw4[c, :act_n], (1, 0, 2, 3))
                                 .reshape(S, act_n * C * S).astype(BF16))
            for c in range(C)]),
        "w1h": w1h.reshape(HK, 128, A),
        "w1l": w1l.reshape(HK, 128, A),
        "w2h": w2h.reshape(AK, 128, H),
        "w2l": w2l.reshape(AK, 128, H),
        "m1q": _q8(m1mat, SM1),
        "b1s": np.ascontiguousarray(
            (b1 * SH8).astype(np.float32).reshape(AK, 128).T),
        "g2s": np.ascontiguousarray((gfc2 * Z1INV).reshape(1, H)),
        "b2row": _bf(fc2_b.reshape(1, H)),
        "_xh_t": xh_t, "_xl_t": xl_t,
        "_has_semb": bool(np.any(sem_b)), "_has_b2": bool(np.any(fc2_b)),
    }
    return shared, act_n


def kernel(**inputs):
    x = np.asarray(inputs["x"], np.float32)
    shared, act_n = _make_shared(inputs)
    key = (act_n, shared["_has_semb"], shared["_has_b2"])
    if key not in _CACHE:
        _CACHE[key] = _build_program(act_n, shared["_has_semb"], shared["_has_b2"])
    nc = _CACHE[key]

    in_maps = [_prep_core_inputs(k, shared["_xh_t"], shared["_xl_t"], x, shared)
               for k in range(NCORES)]
    res = bass_utils.run_bass_kernel_spmd(nc, in_maps, core_ids=list(range(NCORES)))
    out = np.empty((B, S, H), np.float32)
    for k in range(NCORES):
        own = [(48 * k + 43 * i) % B for i in range(BL)]
        out[own] = res.results[k]["out"]
    return out


# revision 54
# speedup vs baseline: 1.8671x; 1.0459x over previous
"""Trainium2 Bass kernel for nn_BertAdapterCapsuleMask (fp8 DoubleRow version).

Strategy (8 NeuronCores, SPMD — identical program, per-core data):

Sharding (same as proven baseline): core k owns routing pairs
t in [48k, 48k+48) and output examples b_i = (48k + 43 i) mod 128, so the
pairs' sem examples are exactly the 18 consecutive [16k, 16k+18) and the
vote rows each core produces are exactly the ones its own examples'
adapter needs — zero cross-core traffic.

Speed: the adapter GEMMs (fc1 768x2048, fc2 2048x768 per 2048 tokens)
run in fp8-e4m3 with MatmulPerfMode.DoubleRow (256-deep contraction,
half-cycle per output column).  Accuracy is restored with residual
("lo") streams quantized at the SAME dequant scale as the hi streams —
fp8's relative precision is scale-invariant, so all streams of one GEMM
accumulate into a single PSUM group with zero combine cost:

  fc1 psum  = xh*w1h + xl*w1h + xh*w1l + vt8*m1q      (all at 2^15)
  fc2 psum  = h8*w2h + h8*w2l                          (at 2^15)

The capsule path keeps near-fp32 accuracy (routing softmax amplifies
sem errors ~10x): sem runs as four fp8 streams with fine residual
scales (separate psums, combines staged through SBUF), squash/routing
stay fp32, priors run bf16.  The capsule term enters fc1 via the
vt8*m1q rank-3 DoubleRow update, so h_out never materializes.

DMA discipline: every DMA instruction costs ~625ns on the single shared
HWDGE descriptor engine, so all bulk tensors move as ONE instruction
each (3D access patterns); the VT gather is a single 3D
gather-descriptor DMA; x/out staging batches 4 examples per DMA.
"""

import numpy as np
import ml_dtypes

import concourse.bass as bass
import concourse.bacc as bacc
import concourse.mybir as mybir
import concourse.tile as tile
from concourse import bass_utils

E4 = ml_dtypes.float8_e4m3
BF16 = ml_dtypes.bfloat16
F32 = mybir.dt.float32
BF = mybir.dt.bfloat16
F8 = mybir.dt.float8e4
AF = mybir.ActivationFunctionType
ALU = mybir.AluOpType
PM = mybir.MatmulPerfMode

B, S, H, A, C, N = 128, 128, 768, 2048, 3, 10
NUM_ITER = 3
NCORES = 8
BL = B // NCORES          # 16 own examples / core
NPAIR = 3 * B // NCORES   # 48 routing pairs / core
HK = H // 128             # 6
AK = A // 128             # 16
TOK = BL * S              # 2048 tokens / core
NC30 = N * C              # 30
NSEM = 18                 # sem examples per core: [16k, 16k+18) mod 128
STOK = NSEM * S           # 2304 sem tokens / core
NPRE = 1                  # fc1 col-chunks (of 512 tokens) via z1p prepass

# fp8 scales.  All adapter streams share dequant 2^-15; sem streams use
# fine residual scales (routing is sensitive) and combine explicitly.
SXH = 16.0
SW = 2048.0
SVT, SM1 = 8.0, 4096.0         # SVT*SM1 == SXH*SW == 2^15
SH8 = 16.0
SXSL = 512.0                   # xsem lo
SSWL = 32768.0                 # sem_w lo
Z1INV = 1.0 / 32768.0          # 2^-15
H8SC = SH8 * Z1INV             # 2^-11


def _sigmoid_f32(z):
    z = np.asarray(z, np.float64)
    return (1.0 / (1.0 + np.exp(-z))).astype(np.float32)


def _q8(a, scale):
    """fp8-e4m3 quantize: stores clip(a*scale, +-240); dequant is 1/scale."""
    z = np.clip(np.asarray(a, np.float32) * np.float32(scale), -240.0, 240.0)
    return np.ascontiguousarray(z.astype(E4))


def _bf(x):
    return np.ascontiguousarray(np.asarray(x, np.float32).astype(BF16))


# ---------------------------------------------------------------------------
# device program
# ---------------------------------------------------------------------------

def _build_program(act_n, has_semb, has_b2):
    nc = bacc.Bacc("TRN2", target_bir_lowering=False, debug=False,
                   num_devices=NCORES)

    d_xsh = nc.dram_tensor("xsem_h", [HK, 128, STOK], F8, kind="ExternalInput")
    d_xsl = nc.dram_tensor("xsem_l", [HK, 128, STOK], F8, kind="ExternalInput")
    d_swc = nc.dram_tensor("sw_cat", [HK, 128, 2 * NC30], F8, kind="ExternalInput")
    d_semb = nc.dram_tensor("semb", [1, NC30], F32, kind="ExternalInput")
    d_rw = nc.dram_tensor("rw_pack", [C, 128, act_n * C * S], BF, kind="ExternalInput")
    d_masks = nc.dram_tensor("masks3", [128, C * NPAIR], F32, kind="ExternalInput")
    d_xh = nc.dram_tensor("xq_h", [4, HK, 128, 512], F8, kind="ExternalInput")
    d_xl = nc.dram_tensor("xq_l", [4, HK, 128, 512], F8, kind="ExternalInput")
    d_w1h = nc.dram_tensor("w1h", [HK, 128, A], F8, kind="ExternalInput")
    d_w1l = nc.dram_tensor("w1l", [HK, 128, A], F8, kind="ExternalInput")
    d_w2h = nc.dram_tensor("w2h", [AK, 128, H], F8, kind="ExternalInput")
    d_w2l = nc.dram_tensor("w2l", [AK, 128, H], F8, kind="ExternalInput")
    d_m1q = nc.dram_tensor("m1q", [C, A], F8, kind="ExternalInput")
    d_b1s = nc.dram_tensor("b1s", [128, AK], F32, kind="ExternalInput")
    d_g2s = nc.dram_tensor("g2s", [1, H], F32, kind="ExternalInput")
    d_b2 = nc.dram_tensor("b2row", [1, H], BF, kind="ExternalInput")
    d_xown = nc.dram_tensor("x_own", [BL, S, H], BF, kind="ExternalInput")
    d_vcb = nc.dram_tensor("votecb", [NPAIR * S + 8], F8, kind="Internal")
    d_out = nc.dram_tensor("out", [BL, S, H], F32, kind="ExternalOutput")

    def pair_ap(ap, off, stride, n):
        """[[part], [stride, 2], [1, n]] AP at ap.offset+off (DoubleRow pair)."""
        return bass.AP(ap.tensor, ap.offset + off, [ap.ap[0], [stride, 2], [1, n]])

    def merged_load(queue, dst_tile, dram, inner):
        """One-DMA load of [K, 128, inner] dram into [128, K*inner] sbuf."""
        k = dram.shape[0]
        queue.dma_start(
            dst_tile[:].rearrange("p (k c) -> p k c", k=k),
            dram.ap().rearrange("k p c -> p k c"))

    with tile.TileContext(nc) as tc:
        with (
            tc.tile_pool(name="w", bufs=1) as wp,
            tc.tile_pool(name="semx", bufs=1) as sxp,
            tc.tile_pool(name="sem", bufs=1) as smp,
            tc.tile_pool(name="rt", bufs=1) as rp,
            tc.tile_pool(name="ad", bufs=1) as ap_,
            tc.tile_pool(name="st", bufs=2) as sp,
            tc.tile_pool(name="ps", bufs=8, space="PSUM") as pp,
        ):
            # ---------------- persistent loads (one DMA each) --------------
            # ALL bulk DMA on the SP queue in priority order: a DMA holds
            # its SEQ while waiting for a ring credit, so the Act queue
            # must stay DMA-free for compute.
            xsh_sb = sxp.tile([128, HK * STOK], F8)
            xsl_sb = sxp.tile([128, HK * STOK], F8)
            merged_load(nc.sync, xsh_sb, d_xsh, STOK)
            swc_sb = wp.tile([128, HK * 2 * NC30], F8)
            merged_load(nc.sync, swc_sb, d_swc, 2 * NC30)
            merged_load(nc.sync, xsl_sb, d_xsl, STOK)
            semb_sb = wp.tile([1, NC30], F32)
            nc.sync.dma_start(semb_sb[:], d_semb[:])
            masks_sb = wp.tile([128, C * NPAIR], F32)
            nc.sync.dma_start(masks_sb[:], d_masks[:])
            rw_sb = sxp.tile([128, C * act_n * C * S], BF)
            merged_load(nc.sync, rw_sb, d_rw, act_n * C * S)

            # adapter data; x arrives cc-major so fc1's first column chunk
            # (the z1p prepass) can start before the rest streams in.
            xh_sb = wp.tile([128, HK * TOK], F8, tag="bigx", bufs=1)
            xl_sb = wp.tile([128, HK * TOK], F8, tag="bigx2", bufs=1)
            w1h_sb = wp.tile([128, HK * A], F8)
            w1l_sb = wp.tile([128, HK * A], F8)

            def x_cc_load(dst_tile, dram, cc):
                dst = bass.AP(dst_tile[:].tensor,
                              dst_tile[:].offset + cc * 512,
                              [dst_tile[:].ap[0], [TOK, HK], [1, 512]])
                nc.sync.dma_start(dst, dram.ap()[cc].rearrange("hk p c -> p hk c"))

            x_cc_load(xh_sb, d_xh, 0)
            x_cc_load(xl_sb, d_xl, 0)
            merged_load(nc.sync, w1h_sb, d_w1h, A)
            merged_load(nc.sync, w1l_sb, d_w1l, A)
            for cc in range(1, 4):
                x_cc_load(xh_sb, d_xh, cc)
                x_cc_load(xl_sb, d_xl, cc)

            # m1 lhsT [128, 2*A]: rows 0-2 of k-tile0 hold m1q, rest zero
            m1_sb = wp.tile([128, 2 * A], F8)
            nc.gpsimd.memset(m1_sb[:], 0.0)
            m1_dst = bass.AP(m1_sb[:].tensor, m1_sb[:].offset,
                             [[m1_sb[:].ap[0][0], C], [1, A]])
            nc.sync.dma_start(m1_dst, d_m1q.ap())
            # vt rhs: partition c (c<3) holds the flat vote buffer shifted
            # by c bytes; other partitions hit zero m1 rows (memset for the
            # simulator's uninitialized-read check).
            vt_sb = ap_.tile([128, NPAIR * S], F8)
            nc.gpsimd.memset(vt_sb[:], 0.0)

            b1_sb = wp.tile([128, AK], F32)
            nc.sync.dma_start(b1_sb[:], d_b1s[:])
            g2rep = wp.tile([128, H], F32)
            g2_src = d_g2s.ap()
            nc.sync.dma_start(
                g2rep[:], bass.AP(g2_src.tensor, g2_src.offset, [[0, 128], [1, H]]))
            w2h_sb = wp.tile([128, AK * H], F8, tag="bigw2", bufs=1)
            w2l_sb = wp.tile([128, AK * H], F8, tag="bigw2l", bufs=1)
            merged_load(nc.sync, w2h_sb, d_w2h, H)
            merged_load(nc.sync, w2l_sb, d_w2l, H)
            ones_f = wp.tile([1, 128], F32)
            nc.gpsimd.memset(ones_f[:], 1.0)
            if has_b2:
                ones_bf = wp.tile([1, 128], BF)
                nc.gpsimd.memset(ones_bf[:], 1.0)
                b2_sb = wp.tile([1, H], BF)
                nc.sync.dma_start(b2_sb[:], d_b2[:])

            # ---------------- phase 1: sem (4 fp8 streams) -----------------
            # swh||swl are concatenated in the rhs, so each DR matmul feeds
            # two streams at once (6 matmuls per slot); 4 slots share one
            # PSUM bank under a single accumulation bracket (start zeroes
            # the whole 2KB zero-region).  hi+lo halves then fold via one
            # Act copy + one strided DVE add per group.
            sem_own = smp.tile([128, NSEM * NC30], F32)
            SEMG = 4
            ngrp = (NSEM + SEMG - 1) // SEMG
            for grp in range(ngrp):
                slots = range(grp * SEMG, min((grp + 1) * SEMG, NSEM))
                nsl = len(slots)
                ps = pp.tile([128, SEMG * 2 * NC30], F32, tag="mm",
                             name=f"ps_sem_{grp}")
                for j, slot in enumerate(slots):
                    for hp in range(HK // 2):
                        off = (2 * hp) * STOK + slot * S
                        lhT_h = pair_ap(xsh_sb[:], off, STOK, 128)
                        lhT_l = pair_ap(xsl_sb[:], off, STOK, 128)
                        rw_c = pair_ap(swc_sb[:], (2 * hp) * 2 * NC30,
                                       2 * NC30, 2 * NC30)
                        dst = ps[:, j * 2 * NC30:(j + 1) * 2 * NC30]
                        st = j == 0 and hp == 0
                        sp_last = (j == nsl - 1 and hp == 2 and not has_semb)
                        nc.tensor.matmul(dst, lhT_h, rw_c, start=st, stop=False,
                                         perf_mode=PM.DoubleRow)
                        nc.tensor.matmul(dst, lhT_l, rw_c, start=False,
                                         stop=sp_last, perf_mode=PM.DoubleRow)
                    if has_semb:
                        nc.tensor.matmul(
                            ps[:, j * 2 * NC30: j * 2 * NC30 + NC30],
                            ones_f[:], semb_sb[:], start=False,
                            stop=(j == nsl - 1))
                # fold hi/lo halves: sem = 2^-15 * (ps[,0:30] + ps[,30:60])
                sc = smp.tile([128, SEMG * 2 * NC30], F32, tag="sc", bufs=2,
                              name=f"sc_{grp}")
                nc.scalar.activation(sc[:, 0:nsl * 2 * NC30],
                                     ps[:, 0:nsl * 2 * NC30],
                                     AF.Copy, scale=Z1INV)
                sc_ap = sc[:]
                hi = bass.AP(sc_ap.tensor, sc_ap.offset,
                             [sc_ap.ap[0], [2 * NC30, nsl], [1, NC30]])
                lo = bass.AP(sc_ap.tensor, sc_ap.offset + NC30,
                             [sc_ap.ap[0], [2 * NC30, nsl], [1, NC30]])
                nc.vector.tensor_tensor(
                    sem_own[:, grp * SEMG * NC30:
                            (grp * SEMG + nsl) * NC30].rearrange(
                        "p (s c) -> p s c", c=NC30),
                    hi, lo, op=ALU.add)

            # ---------------- squash over n (fp32, as baseline) ------------
            sem2 = smp.tile([128, NSEM * NC30], F32)
            nc.vector.tensor_tensor(sem2[:], sem_own[:], sem_own[:], op=ALU.mult)
            sqt = smp.tile([128, NSEM * C], F32)
            nc.vector.tensor_reduce(
                sqt[:].rearrange("p (slot cc) -> p slot cc", cc=C),
                sem2[:].rearrange("p (slot n cc) -> p slot cc n", n=N, cc=C),
                axis=mybir.AxisListType.X, op=ALU.add)
            lnq = smp.tile([128, NSEM * C], F32)
            nc.scalar.activation(lnq[:], sqt[:], AF.Ln)
            sqq = smp.tile([128, NSEM * C], F32)
            nc.scalar.activation(sqq[:], lnq[:], AF.Exp, scale=0.5)
            up = smp.tile([128, NSEM * C], F32)
            nc.vector.tensor_scalar_add(up[:], sqt[:], 1.0)
            ru = smp.tile([128, NSEM * C], F32)
            nc.vector.reciprocal(ru[:], up[:])
            fq = smp.tile([128, NSEM * C], F32)
            nc.vector.tensor_tensor(fq[:], sqq[:], ru[:], op=ALU.mult)
            sem_sq = sem2  # reuse
            f_ap = fq[:]
            f_b = bass.AP(f_ap.tensor, f_ap.offset,
                          [f_ap.ap[0], [C, NSEM], [0, N], [1, C]])
            nc.vector.tensor_tensor(
                sem_sq[:].rearrange("p (slot n cc) -> p slot n cc", n=N, cc=C),
                sem_own[:].rearrange("p (slot n cc) -> p slot n cc", n=N, cc=C),
                f_b, op=ALU.mult)
            # pair-ordered bf16 copies, pre-masked per c'-group g:
            # sem_pair_g[p, (i,u), nc] = sem_sq[p, slot i+u, nc] * mask[3i+u, g]
            sem_pairs = []
            sq_ap = sem_sq[:]
            gather = bass.AP(sq_ap.tensor, sq_ap.offset,
                             [sq_ap.ap[0], [NC30, BL], [NC30, C], [1, NC30]])
            mk_ap = masks_sb[:]
            for g in range(C):
                spg = smp.tile([128, NPAIR * NC30], BF, name=f"sem_pair_{g}")
                sem_pairs.append(spg)
                mk_b = bass.AP(mk_ap.tensor, mk_ap.offset + g * NPAIR,
                               [mk_ap.ap[0], [3, BL], [1, C], [0, NC30]])
                eng = nc.gpsimd if g == 1 else nc.vector
                eng.tensor_tensor(
                    spg[:].rearrange("p (i u nc) -> p i u nc", i=BL, u=C),
                    gather, mk_b, op=ALU.mult)

            # ---------------- phase 2: priors (bf16, one psum per n) -------
            priors = rp.tile([NPAIR, act_n * S], F32)
            for n in range(act_n):
                ps = pp.tile([NPAIR, S], F32, tag="mm", name=f"ps_pr_{n}")
                k = 0
                for g in range(C):
                    sem_v = sem_pairs[g][:].rearrange(
                        "p (pair nc) -> p nc pair", nc=NC30)
                    rbase = g * (act_n * C * S) + (n * C) * S
                    for cc in range(C):
                        nc.tensor.matmul(
                            ps[:], sem_v[:, n * C + cc, :],
                            rw_sb[:, rbase + cc * S: rbase + (cc + 1) * S],
                            start=(k == 0), stop=(k == 3 * C - 1))
                        k += 1
                nc.scalar.copy(priors[:, n * S:(n + 1) * S], ps[:])

            # ---------------- phase 3: routing (fp32, as baseline) ---------
            vote = rp.tile([NPAIR, S], F32)
            scr = rp.tile([NPAIR, S], F32)
            La = rp.tile([NPAIR, act_n], F32)
            Lb = rp.tile([NPAIR, act_n], F32)
            sqv = rp.tile([NPAIR, 1], F32)
            lv = rp.tile([NPAIR, 1], F32)
            sv = rp.tile([NPAIR, 1], F32)
            uv = rp.tile([NPAIR, 1], F32)
            rv = rp.tile([NPAIR, 1], F32)
            fv = rp.tile([NPAIR, 1], F32)
            outv = rp.tile([NPAIR, S], F32)
            mx = rp.tile([NPAIR, 1], F32)
            mneg = rp.tile([NPAIR, 1], F32)
            ex = rp.tile([NPAIR, act_n], F32)
            es = rp.tile([NPAIR, 1], F32)
            ers = rp.tile([NPAIR, 1], F32)
            probs = rp.tile([NPAIR, act_n], F32)

            scrT = rp.tile([NPAIR, act_n * S], F32)   # [pair, s*act_n + n]

            def vote_from(pr_sc, first_const=None):
                """vote[p,s] = sum_n probs[p,n] * priors[p,n,s] in 2 wide ops:
                scrT[p, s*an+n] = priors[p,n,s]*probs[p,n]; reduce over n."""
                pr_ap = priors[:]
                pr_v = bass.AP(pr_ap.tensor, pr_ap.offset,
                               [pr_ap.ap[0], [S, act_n], [1, S]])
                if first_const is not None:
                    sc_v = None
                else:
                    sc_ap = pr_sc[:]
                    sc_v = bass.AP(sc_ap.tensor, sc_ap.offset,
                                   [sc_ap.ap[0], [1, act_n], [0, S]])
                dstT = bass.AP(scrT[:].tensor, scrT[:].offset,
                               [scrT[:].ap[0], [1, act_n], [act_n, S]])
                if first_const is not None:
                    nc.vector.tensor_scalar_mul(dstT, pr_v, first_const)
                else:
                    nc.vector.tensor_tensor(dstT, pr_v, sc_v, op=ALU.mult)
                nc.vector.tensor_reduce(
                    vote[:].rearrange("p (s one) -> p s one", one=1),
                    scrT[:].rearrange("p (s n) -> p s n", n=act_n),
                    axis=mybir.AxisListType.X, op=ALU.add)

            def squash_vote():
                """fv = sqrt(sq)/(1+sq); the Act Ln/Exp round-trips overlap
                the DVE delta dot-products that don't need fv."""
                nc.vector.tensor_tensor(scr[:], vote[:], vote[:], op=ALU.mult)
                nc.vector.tensor_reduce(sqv[:], scr[:],
                                        axis=mybir.AxisListType.X, op=ALU.add)
                nc.scalar.activation(lv[:], sqv[:], AF.Ln)
                nc.scalar.activation(sv[:], lv[:], AF.Exp, scale=0.5)
                nc.vector.tensor_scalar_add(uv[:], sqv[:], 1.0)
                nc.vector.reciprocal(rv[:], uv[:])
                nc.vector.tensor_tensor(fv[:], sv[:], rv[:], op=ALU.mult)

            scrA = rp.tile([NPAIR, act_n * S], F32)
            Lraw = rp.tile([NPAIR, act_n], F32)

            def deltas(Lprev, Lnew):
                """Lnew = <priors_n, vote>*fv (+ Lprev).  The dot-products
                use the raw vote so they run while Act computes fv."""
                v_ap = vote[:]
                v_b = bass.AP(v_ap.tensor, v_ap.offset,
                              [v_ap.ap[0], [0, act_n], [1, S]])
                nc.vector.tensor_tensor(
                    scrA[:].rearrange("p (n s) -> p n s", n=act_n),
                    priors[:].rearrange("p (n s) -> p n s", n=act_n),
                    v_b, op=ALU.mult)
                nc.vector.tensor_reduce(
                    Lraw[:].rearrange("p (n one) -> p n one", one=1),
                    scrA[:].rearrange("p (n s) -> p n s", n=act_n),
                    axis=mybir.AxisListType.X, op=ALU.add)
                if Lprev is None:
                    nc.vector.tensor_scalar_mul(Lnew[:], Lraw[:], fv[:, 0:1])
                else:
                    nc.vector.scalar_tensor_tensor(
                        Lnew[:], Lraw[:], fv[:, 0:1], Lprev[:],
                        op0=ALU.mult, op1=ALU.add)

            def softmax(L):
                nc.vector.tensor_reduce(mx[:], L[:], axis=mybir.AxisListType.X,
                                        op=ALU.max)
                nc.vector.tensor_scalar_mul(mneg[:], mx[:], -1.0)
                nc.scalar.activation(ex[:], L[:], AF.Exp, bias=mneg[:],
                                     accum_out=es[:])
                nc.vector.reciprocal(ers[:], es[:])
                nc.vector.tensor_scalar_mul(probs[:], ex[:], ers[:])

            vote_from(None, first_const=1.0 / act_n)
            squash_vote()
            deltas(None, La)
            softmax(La)
            vote_from(probs)
            squash_vote()
            deltas(La, Lb)
            softmax(Lb)
            vote_from(probs)

            # vote -> fp8 (*8) -> DRAM -> one gather DMA:
            # vt[u, e*128+s] = votecb[e*384 + 3*s + u]
            # vote -> fp8 -> flat DRAM (one contiguous write).  The capsule
            # coefficient of token t, channel c is flat[3t+c], so vt_sb
            # partition c holds flat[c:] (three contiguous byte-shifted
            # reads) and the m1 matmul reads it with column stride 3.
            vb = rp.tile([NPAIR, S], F8)
            nc.scalar.activation(vb[:], vote[:], AF.Copy, scale=SVT)
            vflat = d_vcb.ap()
            nc.scalar.dma_start(
                bass.AP(vflat.tensor, vflat.offset, [[S, NPAIR], [1, S]]),
                vb[:])
            vdst = bass.AP(vt_sb[:].tensor, vt_sb[:].offset,
                           [[vt_sb[:].ap[0][0], C], [1, NPAIR * S]])
            vsrc2 = bass.AP(vflat.tensor, vflat.offset,
                            [[1, C], [1, NPAIR * S]])
            nc.scalar.dma_start(vdst, vsrc2)

            # ---------------- phase 4: fc1 (fp8 DR, single psum group) -----
            # h8 lives per column-chunk (ring of 2): fc2(cc) consumes it
            # right after fc1(cc) produces it.
            z1p = ap_.tile([128, AK * 512], BF)

            def h8_tile(cc):
                return sp.tile([128, AK * 512], F8, tag="h8", bufs=2,
                               name=f"h8_{cc}")

            def fc1_x_streams(ps, ak, col, last_stop):
                """9 DR matmuls: xh*w1h + xl*w1h + xh*w1l into one group."""
                for hp in range(HK // 2):
                    woff = (2 * hp) * A + ak * 128
                    xoff = (2 * hp) * TOK + col
                    lh_h = pair_ap(w1h_sb[:], woff, A, 128)
                    lh_l = pair_ap(w1l_sb[:], woff, A, 128)
                    rh_h = pair_ap(xh_sb[:], xoff, TOK, 512)
                    rh_l = pair_ap(xl_sb[:], xoff, TOK, 512)
                    st = hp == 0
                    nc.tensor.matmul(ps[:], lh_h, rh_h, start=st, stop=False,
                                     perf_mode=PM.DoubleRow)
                    nc.tensor.matmul(ps[:], lh_h, rh_l, start=False, stop=False,
                                     perf_mode=PM.DoubleRow)
                    nc.tensor.matmul(ps[:], lh_l, rh_h, start=False,
                                     stop=(hp == 2 and last_stop),
                                     perf_mode=PM.DoubleRow)

            def m1_step(ps, ak, col, start):
                lh = pair_ap(m1_sb[:], ak * 128, A, 128)
                vt_ap = vt_sb[:]
                rh = bass.AP(vt_ap.tensor, vt_ap.offset + 3 * col,
                             [vt_ap.ap[0], [0, 2], [3, 512]])
                nc.tensor.matmul(ps[:], lh, rh, start=start, stop=True,
                                 perf_mode=PM.DoubleRow)

            def fc2_chunk(tt, h8c):
                """fc2 for example tt: h8 x (w2h+w2l), then out = x+relu*g2."""
                psa = pp.tile([128, 512], F32, tag="mm", name=f"ps_f2a_{tt}")
                psb = pp.tile([128, 256], F32, tag="mm", name=f"ps_f2b_{tt}")
                for w_sb, first in ((w2h_sb, True), (w2l_sb, False)):
                    for ap8 in range(AK // 2):
                        woff = (2 * ap8) * H
                        hoff = (2 * ap8) * 512 + (tt % 4) * 128
                        lh = pair_ap(h8c[:], hoff, 512, 128)
                        st = first and ap8 == 0
                        sp_ = (not first) and ap8 == AK // 2 - 1
                        nc.tensor.matmul(psa[:], lh, pair_ap(w_sb[:], woff, H, 512),
                                         start=st, stop=(sp_ and not has_b2),
                                         perf_mode=PM.DoubleRow)
                        nc.tensor.matmul(psb[:], lh,
                                         pair_ap(w_sb[:], woff + 512, H, 256),
                                         start=st, stop=(sp_ and not has_b2),
                                         perf_mode=PM.DoubleRow)
                if has_b2:
                    nc.tensor.matmul(psa[:], ones_bf[:], b2_sb[:, 0:512],
                                     start=False, stop=True)
                    nc.tensor.matmul(psb[:], ones_bf[:], b2_sb[:, 512:H],
                                     start=False, stop=True)
                xt = sp.tile([128, H], BF, tag="x", name=f"x_{tt}", bufs=2)
                nc.sync.dma_start(xt[:], d_xown[tt])
                ot = sp.tile([128, H], F32, tag="o", name=f"o_{tt}", bufs=2)
                nc.scalar.activation(ot[:, 0:512], psa[:], AF.Relu)
                nc.scalar.activation(ot[:, 512:H], psb[:], AF.Relu)
                nc.vector.tensor_tensor(ot[:], ot[:], g2rep[:], op=ALU.mult)
                nc.vector.tensor_tensor(ot[:], ot[:], xt[:], op=ALU.add)
                nc.sync.dma_start(d_out[tt], ot[:])

            # cc0: x-only prepass into z1p (no VT dependency) ...
            for ak in range(AK):
                ps = pp.tile([128, 512], F32, tag="mm", name=f"ps_f1p_{ak}")
                fc1_x_streams(ps, ak, 0, last_stop=True)
                nc.scalar.copy(z1p[:, ak * 512:(ak + 1) * 512], ps[:])
            # ... then m1 catch-up once VT lands
            h8c = h8_tile(0)
            for ak in range(AK):
                ps = pp.tile([128, 512], F32, tag="mm", name=f"ps_f1m_{ak}")
                m1_step(ps, ak, 0, start=True)
                nc.vector.tensor_tensor(
                    ps[:], ps[:], z1p[:, ak * 512:(ak + 1) * 512], op=ALU.add)
                nc.scalar.activation(
                    h8c[:, ak * 512:(ak + 1) * 512],
                    ps[:], AF.Relu, scale=H8SC, bias=b1_sb[:, ak:ak + 1])
            for tt in range(4):
                fc2_chunk(tt, h8c)
            # fused fc1 blocks (m1 last in each psum group), fc2 per chunk
            for cc in range(1, 4):
                h8c = h8_tile(cc)
                for ak in range(AK):
                    ps = pp.tile([128, 512], F32, tag="mm", name=f"ps_f1f_{cc}_{ak}")
                    fc1_x_streams(ps, ak, cc * 512, last_stop=False)
                    m1_step(ps, ak, cc * 512, start=False)
                    nc.scalar.activation(
                        h8c[:, ak * 512:(ak + 1) * 512],
                        ps[:], AF.Relu, scale=H8SC, bias=b1_sb[:, ak:ak + 1])
                for tt in range(4 * cc, 4 * cc + 4):
                    fc2_chunk(tt, h8c)

    nc.compile()
    return nc


# ---------------------------------------------------------------------------
# host marshaling
# ---------------------------------------------------------------------------

def _prep_core_inputs(k, xh_t, xl_t, x, shared):
    # own examples b_i = (48k + 43 i) mod 128; sem examples [16k, 16k+18).
    own = np.array([(48 * k + 43 * i) % B for i in range(BL)])
    sem_ex = np.array([(16 * k + j) % B for j in range(NSEM)])

    def slab(xt):
        return np.ascontiguousarray(
            xt[:, sem_ex, :].reshape(H, STOK).reshape(HK, 128, STOK))

    xsem_h = slab(xh_t)
    xsem_l = slab(xl_t)

    def ccmajor(xt):
        a = xt[:, own, :].reshape(HK, 128, TOK)          # [hk, p, tok]
        return np.ascontiguousarray(
            a.reshape(HK, 128, 4, 512).transpose(2, 0, 1, 3))

    xq_h = ccmajor(xh_t)
    xq_l = ccmajor(xl_t)
    x_own = _bf(x[own])

    masks = np.zeros((C, NPAIR), np.float32)
    for i in range(BL):
        for u in range(C):
            t = 3 * int(own[i]) + u
            masks[t // B, 3 * i + u] = 1.0
    masks3 = np.ascontiguousarray(
        np.broadcast_to(masks.reshape(1, C * NPAIR), (128, C * NPAIR)))

    core = {
        "xsem_h": xsem_h, "xsem_l": xsem_l,
        "xq_h": xq_h, "xq_l": xq_l, "x_own": x_own, "masks3": masks3,
    }
    core.update({n: shared[n] for n in (
        "sw_cat", "semb", "rw_pack", "w1h", "w1l", "w2h", "w2l",
        "m1q", "b1s", "g2s", "b2row")})
    return core


_CACHE = {}


def _make_shared(inputs):
    fc1_w = np.asarray(inputs["fc1_w"], np.float32)
    fc1_b = np.asarray(inputs["fc1_b"], np.float32)
    fc2_w = np.asarray(inputs["fc2_w"], np.float32)
    fc2_b = np.asarray(inputs["fc2_b"], np.float32)
    efc1 = np.asarray(inputs["efc1"], np.float32)
    efc2 = np.asarray(inputs["efc2"], np.float32)
    sem_w = np.asarray(inputs["sem_w"], np.float32)
    sem_b = np.asarray(inputs["sem_b"], np.float32)
    route_weights = np.asarray(inputs["route_weights"], np.float32)
    larger_w = np.asarray(inputs["larger_w"], np.float32)
    larger_b = np.asarray(inputs["larger_b"], np.float32)
    elarger = np.asarray(inputs["elarger"], np.float32)
    t = int(np.asarray(inputs["t"]))
    sf = np.float32(int(np.asarray(inputs["s"])))
    act_n = t + 1
    x = np.asarray(inputs["x"], np.float32)

    gfc1 = _sigmoid_f32(sf * efc1[t])
    gfc2 = _sigmoid_f32(sf * efc2[t])
    glarger = _sigmoid_f32(sf * elarger[t])

    lwg = (larger_w * glarger[None, :]).astype(np.float32)
    m1mat = (lwg @ fc1_w).astype(np.float32)
    b1 = ((larger_b * glarger) @ fc1_w + fc1_b).astype(np.float32)
    w2g = (fc2_w * gfc1[:, None]).astype(np.float32)
    rw4 = route_weights.reshape(C, N, S, C, S)

    # transposed x quantization (shared across cores; slabs pick columns)
    xt = np.ascontiguousarray(np.transpose(x, (2, 0, 1)))      # [H, B, S]
    xh_t = _q8(xt, SXH).reshape(H, B, S)
    xl_res = xt - xh_t.astype(np.float32) / SXH
    xl_t = _q8(xl_res, SXH).reshape(H, B, S)                   # lo at same scale

    sw2d = np.transpose(sem_w, (1, 0, 2)).reshape(H, NC30)
    swh = _q8(sw2d, SW)
    swl = _q8(sw2d - swh.astype(np.float32) / SW, SW)
    sw_cat = np.concatenate([swh, swl], axis=1)        # [H, 60]
    w1h = _q8(fc1_w, SW)
    w1l = _q8(fc1_w - w1h.astype(np.float32) / SW, SW)
    w2h = _q8(w2g, SW)
    w2l = _q8(w2g - w2h.astype(np.float32) / SW, SW)

    shared = {
        "sw_cat": np.ascontiguousarray(sw_cat.reshape(HK, 128, 2 * NC30)),
        "semb": np.ascontiguousarray(
            (sem_b.reshape(1, NC30) * 32768.0).astype(np.float32)),
        "rw_pack": np.stack([
            np.ascontiguousarray(np.transpose(rw4[c, :act_n], (1, 0, 2, 3))
                                 .reshape(S, act_n * C * S).astype(BF16))
            for c in range(C)]),
        "w1h": w1h.reshape(HK, 128, A),
        "w1l": w1l.reshape(HK, 128, A),
        "w2h": w2h.reshape(AK, 128, H),
        "w2l": w2l.reshape(AK, 128, H),
        "m1q": _q8(m1mat, SM1),
        "b1s": np.ascontiguousarray(
            (b1 * SH8).astype(np.float32).reshape(AK, 128).T),
        "g2s": np.ascontiguousarray((gfc2 * Z1INV).reshape(1, H)),
        "b2row": _bf(fc2_b.reshape(1, H)),
        "_xh_t": xh_t, "_xl_t": xl_t,
        "_has_semb": bool(np.any(sem_b)), "_has_b2": bool(np.any(fc2_b)),
    }
    return shared, act_n


def kernel(**inputs):
    x = np.asarray(inputs["x"], np.float32)
    shared, act_n = _make_shared(inputs)
    key = (act_n, shared["_has_semb"], shared["_has_b2"])
    if key not in _CACHE:
        _CACHE[key] = _build_program(act_n, shared["_has_semb"], shared["_has_b2"])
    nc = _CACHE[key]

    in_maps = [_prep_core_inputs(k, shared["_xh_t"], shared["_xl_t"], x, shared)
               for k in range(NCORES)]
    res = bass_utils.run_bass_kernel_spmd(nc, in_maps, core_ids=list(range(NCORES)))
    out = np.empty((B, S, H), np.float32)
    for k in range(NCORES):
        own = [(48 * k + 43 * i) % B for i in range(BL)]
        out[own] = res.results[k]["out"]
    return out


# revision 56
# speedup vs baseline: 2.0200x; 1.0819x over previous
"""Trainium2 Bass kernel for nn_BertAdapterCapsuleMask (fp8 DoubleRow version).

Strategy (8 NeuronCores, SPMD — identical program, per-core data):

Sharding (same as proven baseline): core k owns routing pairs
t in [48k, 48k+48) and output examples b_i = (48k + 43 i) mod 128, so the
pairs' sem examples are exactly the 18 consecutive [16k, 16k+18) and the
vote rows each core produces are exactly the ones its own examples'
adapter needs — zero cross-core traffic.

Speed: the adapter GEMMs (fc1 768x2048, fc2 2048x768 per 2048 tokens)
run in fp8-e4m3 with MatmulPerfMode.DoubleRow (256-deep contraction,
half-cycle per output column).  Accuracy is restored with residual
("lo") streams quantized at the SAME dequant scale as the hi streams —
fp8's relative precision is scale-invariant, so all streams of one GEMM
accumulate into a single PSUM group with zero combine cost:

  fc1 psum  = xh*w1h + xl*w1h + xh*w1l + vt8*m1q      (all at 2^15)
  fc2 psum  = h8*w2h + h8*w2l                          (at 2^15)

The capsule path keeps near-fp32 accuracy (routing softmax amplifies
sem errors ~10x): sem runs as four fp8 streams with fine residual
scales (separate psums, combines staged through SBUF), squash/routing
stay fp32, priors run bf16.  The capsule term enters fc1 via the
vt8*m1q rank-3 DoubleRow update, so h_out never materializes.

DMA discipline: every DMA instruction costs ~625ns on the single shared
HWDGE descriptor engine, so all bulk tensors move as ONE instruction
each (3D access patterns); the VT gather is a single 3D
gather-descriptor DMA; x/out staging batches 4 examples per DMA.
"""

import numpy as np
import ml_dtypes

import concourse.bass as bass
import concourse.bacc as bacc
import concourse.mybir as mybir
import concourse.tile as tile
from concourse import bass_utils

E4 = ml_dtypes.float8_e4m3
BF16 = ml_dtypes.bfloat16
F32 = mybir.dt.float32
BF = mybir.dt.bfloat16
F8 = mybir.dt.float8e4
AF = mybir.ActivationFunctionType
ALU = mybir.AluOpType
PM = mybir.MatmulPerfMode

B, S, H, A, C, N = 128, 128, 768, 2048, 3, 10
NUM_ITER = 3
NCORES = 8
BL = B // NCORES          # 16 own examples / core
NPAIR = 3 * B // NCORES   # 48 routing pairs / core
HK = H // 128             # 6
AK = A // 128             # 16
TOK = BL * S              # 2048 tokens / core
NC30 = N * C              # 30
NSEM = 18                 # sem examples per core: [16k, 16k+18) mod 128
STOK = NSEM * S           # 2304 sem tokens / core
NPRE = 1                  # fc1 col-chunks (of 512 tokens) via z1p prepass

# fp8 scales.  All adapter streams share dequant 2^-15; sem streams use
# fine residual scales (routing is sensitive) and combine explicitly.
SXH = 16.0
SW = 2048.0
SVT, SM1 = 8.0, 4096.0         # SVT*SM1 == SXH*SW == 2^15
SH8 = 16.0
SXSL = 512.0                   # xsem lo
SSWL = 32768.0                 # sem_w lo
Z1INV = 1.0 / 32768.0          # 2^-15
H8SC = SH8 * Z1INV             # 2^-11


def _sigmoid_f32(z):
    z = np.asarray(z, np.float64)
    return (1.0 / (1.0 + np.exp(-z))).astype(np.float32)


def _q8(a, scale):
    """fp8-e4m3 quantize: stores clip(a*scale, +-240); dequant is 1/scale."""
    z = np.clip(np.asarray(a, np.float32) * np.float32(scale), -240.0, 240.0)
    return np.ascontiguousarray(z.astype(E4))


def _bf(x):
    return np.ascontiguousarray(np.asarray(x, np.float32).astype(BF16))


# ---------------------------------------------------------------------------
# device program
# ---------------------------------------------------------------------------

def _build_program(act_n, has_semb, has_b2):
    nc = bacc.Bacc("TRN2", target_bir_lowering=False, debug=False,
                   num_devices=NCORES)

    d_xsh = nc.dram_tensor("xsem_h", [HK, 128, STOK], F8, kind="ExternalInput")
    d_xsl = nc.dram_tensor("xsem_l", [HK, 128, STOK], F8, kind="ExternalInput")
    d_swc = nc.dram_tensor("sw_cat", [HK, 128, 2 * NC30], F8, kind="ExternalInput")
    d_semb = nc.dram_tensor("semb", [1, NC30], F32, kind="ExternalInput")
    d_rw = nc.dram_tensor("rw_pack", [C, 128, act_n * C * S], BF, kind="ExternalInput")
    d_masks = nc.dram_tensor("masks3", [128, C * NPAIR], F32, kind="ExternalInput")
    d_xh = nc.dram_tensor("xq_h", [4, HK, 128, 512], F8, kind="ExternalInput")
    d_xl = nc.dram_tensor("xq_l", [4, HK, 128, 512], F8, kind="ExternalInput")
    d_w1h = nc.dram_tensor("w1h", [HK, 128, A], F8, kind="ExternalInput")
    d_w2h = nc.dram_tensor("w2h", [AK, 128, H], F8, kind="ExternalInput")
    d_w2l = nc.dram_tensor("w2l", [AK, 128, H], F8, kind="ExternalInput")
    d_m1q = nc.dram_tensor("m1q", [C, A], F8, kind="ExternalInput")
    d_b1s = nc.dram_tensor("b1s", [128, AK], F32, kind="ExternalInput")
    d_g2s = nc.dram_tensor("g2s", [1, H], F32, kind="ExternalInput")
    d_b2 = nc.dram_tensor("b2row", [1, H], BF, kind="ExternalInput")
    d_xown = nc.dram_tensor("x_own", [BL, S, H], BF, kind="ExternalInput")
    d_vcb = nc.dram_tensor("votecb", [NPAIR * S + 8], F8, kind="Internal")
    d_out = nc.dram_tensor("out", [BL, S, H], F32, kind="ExternalOutput")

    def pair_ap(ap, off, stride, n):
        """[[part], [stride, 2], [1, n]] AP at ap.offset+off (DoubleRow pair)."""
        return bass.AP(ap.tensor, ap.offset + off, [ap.ap[0], [stride, 2], [1, n]])

    def merged_load(queue, dst_tile, dram, inner):
        """One-DMA load of [K, 128, inner] dram into [128, K*inner] sbuf."""
        k = dram.shape[0]
        queue.dma_start(
            dst_tile[:].rearrange("p (k c) -> p k c", k=k),
            dram.ap().rearrange("k p c -> p k c"))

    with tile.TileContext(nc) as tc:
        with (
            tc.tile_pool(name="w", bufs=1) as wp,
            tc.tile_pool(name="semx", bufs=1) as sxp,
            tc.tile_pool(name="sem", bufs=1) as smp,
            tc.tile_pool(name="rt", bufs=1) as rp,
            tc.tile_pool(name="ad", bufs=1) as ap_,
            tc.tile_pool(name="st", bufs=2) as sp,
            tc.tile_pool(name="ps", bufs=8, space="PSUM") as pp,
        ):
            # ---------------- persistent loads (one DMA each) --------------
            # ALL bulk DMA on the SP queue in priority order: a DMA holds
            # its SEQ while waiting for a ring credit, so the Act queue
            # must stay DMA-free for compute.
            xsh_sb = sxp.tile([128, HK * STOK], F8)
            xsl_sb = sxp.tile([128, HK * STOK], F8)
            merged_load(nc.sync, xsh_sb, d_xsh, STOK)
            swc_sb = wp.tile([128, HK * 2 * NC30], F8)
            merged_load(nc.sync, swc_sb, d_swc, 2 * NC30)
            merged_load(nc.sync, xsl_sb, d_xsl, STOK)
            semb_sb = wp.tile([1, NC30], F32)
            nc.sync.dma_start(semb_sb[:], d_semb[:])
            masks_sb = wp.tile([128, C * NPAIR], F32)
            nc.sync.dma_start(masks_sb[:], d_masks[:])
            rw_sb = sxp.tile([128, C * act_n * C * S], BF)
            merged_load(nc.sync, rw_sb, d_rw, act_n * C * S)

            # adapter data; x arrives cc-major so fc1's first column chunk
            # (the z1p prepass) can start before the rest streams in.
            xh_sb = wp.tile([128, HK * TOK], F8, tag="bigx", bufs=1)
            xl_sb = wp.tile([128, HK * TOK], F8, tag="bigx2", bufs=1)
            w1h_sb = wp.tile([128, HK * A], F8)

            def x_cc_load(dst_tile, dram, cc):
                dst = bass.AP(dst_tile[:].tensor,
                              dst_tile[:].offset + cc * 512,
                              [dst_tile[:].ap[0], [TOK, HK], [1, 512]])
                nc.sync.dma_start(dst, dram.ap()[cc].rearrange("hk p c -> p hk c"))

            x_cc_load(xh_sb, d_xh, 0)
            x_cc_load(xl_sb, d_xl, 0)
            merged_load(nc.sync, w1h_sb, d_w1h, A)
            for cc in range(1, 4):
                x_cc_load(xh_sb, d_xh, cc)
                x_cc_load(xl_sb, d_xl, cc)

            # m1 lhsT [128, 2*A]: rows 0-2 of k-tile0 hold m1q, rest zero
            m1_sb = wp.tile([128, 2 * A], F8)
            nc.gpsimd.memset(m1_sb[:], 0.0)
            m1_dst = bass.AP(m1_sb[:].tensor, m1_sb[:].offset,
                             [[m1_sb[:].ap[0][0], C], [1, A]])
            nc.sync.dma_start(m1_dst, d_m1q.ap())
            # vt rhs: partition c (c<3) holds the flat vote buffer shifted
            # by c bytes; other partitions hit zero m1 rows (memset for the
            # simulator's uninitialized-read check).
            vt_sb = ap_.tile([128, NPAIR * S], F8)
            nc.gpsimd.memset(vt_sb[:], 0.0)

            b1_sb = wp.tile([128, AK], F32)
            nc.sync.dma_start(b1_sb[:], d_b1s[:])
            g2rep = wp.tile([128, H], F32)
            g2_src = d_g2s.ap()
            nc.sync.dma_start(
                g2rep[:], bass.AP(g2_src.tensor, g2_src.offset, [[0, 128], [1, H]]))
            w2h_sb = wp.tile([128, AK * H], F8, tag="bigw2", bufs=1)
            w2l_sb = wp.tile([128, AK * H], F8, tag="bigw2l", bufs=1)
            merged_load(nc.sync, w2h_sb, d_w2h, H)
            merged_load(nc.sync, w2l_sb, d_w2l, H)
            ones_f = wp.tile([1, 128], F32)
            nc.gpsimd.memset(ones_f[:], 1.0)
            if has_b2:
                ones_bf = wp.tile([1, 128], BF)
                nc.gpsimd.memset(ones_bf[:], 1.0)
                b2_sb = wp.tile([1, H], BF)
                nc.sync.dma_start(b2_sb[:], d_b2[:])

            # ---------------- phase 1: sem (4 fp8 streams) -----------------
            # swh||swl are concatenated in the rhs, so each DR matmul feeds
            # two streams at once (6 matmuls per slot); 4 slots share one
            # PSUM bank under a single accumulation bracket (start zeroes
            # the whole 2KB zero-region).  hi+lo halves then fold via one
            # Act copy + one strided DVE add per group.
            sem_own = smp.tile([128, NSEM * NC30], F32)
            SEMG = 4
            ngrp = (NSEM + SEMG - 1) // SEMG
            for grp in range(ngrp):
                slots = range(grp * SEMG, min((grp + 1) * SEMG, NSEM))
                nsl = len(slots)
                ps = pp.tile([128, SEMG * 2 * NC30], F32, tag="mm",
                             name=f"ps_sem_{grp}")
                for j, slot in enumerate(slots):
                    for hp in range(HK // 2):
                        off = (2 * hp) * STOK + slot * S
                        lhT_h = pair_ap(xsh_sb[:], off, STOK, 128)
                        lhT_l = pair_ap(xsl_sb[:], off, STOK, 128)
                        rw_c = pair_ap(swc_sb[:], (2 * hp) * 2 * NC30,
                                       2 * NC30, 2 * NC30)
                        dst = ps[:, j * 2 * NC30:(j + 1) * 2 * NC30]
                        st = j == 0 and hp == 0
                        sp_last = (j == nsl - 1 and hp == 2 and not has_semb)
                        nc.tensor.matmul(dst, lhT_h, rw_c, start=st, stop=False,
                                         perf_mode=PM.DoubleRow)
                        nc.tensor.matmul(dst, lhT_l, rw_c, start=False,
                                         stop=sp_last, perf_mode=PM.DoubleRow)
                    if has_semb:
                        nc.tensor.matmul(
                            ps[:, j * 2 * NC30: j * 2 * NC30 + NC30],
                            ones_f[:], semb_sb[:], start=False,
                            stop=(j == nsl - 1))
                # fold hi/lo halves: sem = 2^-15 * (ps[,0:30] + ps[,30:60])
                sc = smp.tile([128, SEMG * 2 * NC30], F32, tag="sc", bufs=2,
                              name=f"sc_{grp}")
                nc.scalar.activation(sc[:, 0:nsl * 2 * NC30],
                                     ps[:, 0:nsl * 2 * NC30],
                                     AF.Copy, scale=Z1INV)
                sc_ap = sc[:]
                hi = bass.AP(sc_ap.tensor, sc_ap.offset,
                             [sc_ap.ap[0], [2 * NC30, nsl], [1, NC30]])
                lo = bass.AP(sc_ap.tensor, sc_ap.offset + NC30,
                             [sc_ap.ap[0], [2 * NC30, nsl], [1, NC30]])
                nc.vector.tensor_tensor(
                    sem_own[:, grp * SEMG * NC30:
                            (grp * SEMG + nsl) * NC30].rearrange(
                        "p (s c) -> p s c", c=NC30),
                    hi, lo, op=ALU.add)

            # ---------------- squash over n (fp32, as baseline) ------------
            sem2 = smp.tile([128, NSEM * NC30], F32)
            nc.vector.tensor_tensor(sem2[:], sem_own[:], sem_own[:], op=ALU.mult)
            sqt = smp.tile([128, NSEM * C], F32)
            nc.vector.tensor_reduce(
                sqt[:].rearrange("p (slot cc) -> p slot cc", cc=C),
                sem2[:].rearrange("p (slot n cc) -> p slot cc n", n=N, cc=C),
                axis=mybir.AxisListType.X, op=ALU.add)
            lnq = smp.tile([128, NSEM * C], F32)
            nc.scalar.activation(lnq[:], sqt[:], AF.Ln)
            sqq = smp.tile([128, NSEM * C], F32)
            nc.scalar.activation(sqq[:], lnq[:], AF.Exp, scale=0.5)
            up = smp.tile([128, NSEM * C], F32)
            nc.vector.tensor_scalar_add(up[:], sqt[:], 1.0)
            ru = smp.tile([128, NSEM * C], F32)
            nc.vector.reciprocal(ru[:], up[:])
            fq = smp.tile([128, NSEM * C], F32)
            nc.vector.tensor_tensor(fq[:], sqq[:], ru[:], op=ALU.mult)
            sem_sq = sem2  # reuse
            f_ap = fq[:]
            f_b = bass.AP(f_ap.tensor, f_ap.offset,
                          [f_ap.ap[0], [C, NSEM], [0, N], [1, C]])
            nc.vector.tensor_tensor(
                sem_sq[:].rearrange("p (slot n cc) -> p slot n cc", n=N, cc=C),
                sem_own[:].rearrange("p (slot n cc) -> p slot n cc", n=N, cc=C),
                f_b, op=ALU.mult)
            # pair-ordered bf16 copies, pre-masked per c'-group g:
            # sem_pair_g[p, (i,u), nc] = sem_sq[p, slot i+u, nc] * mask[3i+u, g]
            sem_pairs = []
            sq_ap = sem_sq[:]
            gather = bass.AP(sq_ap.tensor, sq_ap.offset,
                             [sq_ap.ap[0], [NC30, BL], [NC30, C], [1, NC30]])
            mk_ap = masks_sb[:]
            for g in range(C):
                spg = smp.tile([128, NPAIR * NC30], BF, name=f"sem_pair_{g}")
                sem_pairs.append(spg)
                mk_b = bass.AP(mk_ap.tensor, mk_ap.offset + g * NPAIR,
                               [mk_ap.ap[0], [3, BL], [1, C], [0, NC30]])
                eng = nc.gpsimd if g == 1 else nc.vector
                eng.tensor_tensor(
                    spg[:].rearrange("p (i u nc) -> p i u nc", i=BL, u=C),
                    gather, mk_b, op=ALU.mult)

            # ---------------- phase 2: priors (bf16, one psum per n) -------
            priors = rp.tile([NPAIR, act_n * S], F32)
            for n in range(act_n):
                ps = pp.tile([NPAIR, S], F32, tag="mm", name=f"ps_pr_{n}")
                k = 0
                for g in range(C):
                    sem_v = sem_pairs[g][:].rearrange(
                        "p (pair nc) -> p nc pair", nc=NC30)
                    rbase = g * (act_n * C * S) + (n * C) * S
                    for cc in range(C):
                        nc.tensor.matmul(
                            ps[:], sem_v[:, n * C + cc, :],
                            rw_sb[:, rbase + cc * S: rbase + (cc + 1) * S],
                            start=(k == 0), stop=(k == 3 * C - 1))
                        k += 1
                nc.scalar.copy(priors[:, n * S:(n + 1) * S], ps[:])

            # ---------------- phase 3: routing (fp32, as baseline) ---------
            vote = rp.tile([NPAIR, S], F32)
            scr = rp.tile([NPAIR, S], F32)
            La = rp.tile([NPAIR, act_n], F32)
            Lb = rp.tile([NPAIR, act_n], F32)
            sqv = rp.tile([NPAIR, 1], F32)
            lv = rp.tile([NPAIR, 1], F32)
            sv = rp.tile([NPAIR, 1], F32)
            uv = rp.tile([NPAIR, 1], F32)
            rv = rp.tile([NPAIR, 1], F32)
            fv = rp.tile([NPAIR, 1], F32)
            outv = rp.tile([NPAIR, S], F32)
            mx = rp.tile([NPAIR, 1], F32)
            mneg = rp.tile([NPAIR, 1], F32)
            ex = rp.tile([NPAIR, act_n], F32)
            es = rp.tile([NPAIR, 1], F32)
            ers = rp.tile([NPAIR, 1], F32)
            probs = rp.tile([NPAIR, act_n], F32)

            scrT = rp.tile([NPAIR, act_n * S], F32)   # [pair, s*act_n + n]

            def vote_from(pr_sc, first_const=None):
                """vote[p,s] = sum_n probs[p,n] * priors[p,n,s] in 2 wide ops:
                scrT[p, s*an+n] = priors[p,n,s]*probs[p,n]; reduce over n."""
                pr_ap = priors[:]
                pr_v = bass.AP(pr_ap.tensor, pr_ap.offset,
                               [pr_ap.ap[0], [S, act_n], [1, S]])
                if first_const is not None:
                    sc_v = None
                else:
                    sc_ap = pr_sc[:]
                    sc_v = bass.AP(sc_ap.tensor, sc_ap.offset,
                                   [sc_ap.ap[0], [1, act_n], [0, S]])
                dstT = bass.AP(scrT[:].tensor, scrT[:].offset,
                               [scrT[:].ap[0], [1, act_n], [act_n, S]])
                if first_const is not None:
                    nc.vector.tensor_scalar_mul(dstT, pr_v, first_const)
                else:
                    nc.vector.tensor_tensor(dstT, pr_v, sc_v, op=ALU.mult)
                nc.vector.tensor_reduce(
                    vote[:].rearrange("p (s one) -> p s one", one=1),
                    scrT[:].rearrange("p (s n) -> p s n", n=act_n),
                    axis=mybir.AxisListType.X, op=ALU.add)

            def squash_vote():
                """fv = sqrt(sq)/(1+sq); the Act Ln/Exp round-trips overlap
                the DVE delta dot-products that don't need fv."""
                nc.vector.tensor_tensor(scr[:], vote[:], vote[:], op=ALU.mult)
                nc.vector.tensor_reduce(sqv[:], scr[:],
                                        axis=mybir.AxisListType.X, op=ALU.add)
                nc.scalar.activation(lv[:], sqv[:], AF.Ln)
                nc.scalar.activation(sv[:], lv[:], AF.Exp, scale=0.5)
                nc.vector.tensor_scalar_add(uv[:], sqv[:], 1.0)
                nc.vector.reciprocal(rv[:], uv[:])
                nc.vector.tensor_tensor(fv[:], sv[:], rv[:], op=ALU.mult)

            scrA = rp.tile([NPAIR, act_n * S], F32)
            Lraw = rp.tile([NPAIR, act_n], F32)

            def deltas(Lprev, Lnew):
                """Lnew = <priors_n, vote>*fv (+ Lprev).  The dot-products
                use the raw vote so they run while Act computes fv."""
                v_ap = vote[:]
                v_b = bass.AP(v_ap.tensor, v_ap.offset,
                              [v_ap.ap[0], [0, act_n], [1, S]])
                nc.vector.tensor_tensor(
                    scrA[:].rearrange("p (n s) -> p n s", n=act_n),
                    priors[:].rearrange("p (n s) -> p n s", n=act_n),
                    v_b, op=ALU.mult)
                nc.vector.tensor_reduce(
                    Lraw[:].rearrange("p (n one) -> p n one", one=1),
                    scrA[:].rearrange("p (n s) -> p n s", n=act_n),
                    axis=mybir.AxisListType.X, op=ALU.add)
                if Lprev is None:
                    nc.vector.tensor_scalar_mul(Lnew[:], Lraw[:], fv[:, 0:1])
                else:
                    nc.vector.scalar_tensor_tensor(
                        Lnew[:], Lraw[:], fv[:, 0:1], Lprev[:],
                        op0=ALU.mult, op1=ALU.add)

            def softmax(L):
                nc.vector.tensor_reduce(mx[:], L[:], axis=mybir.AxisListType.X,
                                        op=ALU.max)
                nc.vector.tensor_scalar_mul(mneg[:], mx[:], -1.0)
                nc.scalar.activation(ex[:], L[:], AF.Exp, bias=mneg[:],
                                     accum_out=es[:])
                nc.vector.reciprocal(ers[:], es[:])
                nc.vector.tensor_scalar_mul(probs[:], ex[:], ers[:])

            vote_from(None, first_const=1.0 / act_n)
            squash_vote()
            deltas(None, La)
            softmax(La)
            vote_from(probs)
            squash_vote()
            deltas(La, Lb)
            softmax(Lb)
            vote_from(probs)

            # vote -> fp8 (*8) -> DRAM -> one gather DMA:
            # vt[u, e*128+s] = votecb[e*384 + 3*s + u]
            # vote -> fp8 -> flat DRAM (one contiguous write).  The capsule
            # coefficient of token t, channel c is flat[3t+c], so vt_sb
            # partition c holds flat[c:] (three contiguous byte-shifted
            # reads) and the m1 matmul reads it with column stride 3.
            vb = rp.tile([NPAIR, S], F8)
            nc.scalar.activation(vb[:], vote[:], AF.Copy, scale=SVT)
            vflat = d_vcb.ap()
            nc.scalar.dma_start(
                bass.AP(vflat.tensor, vflat.offset, [[S, NPAIR], [1, S]]),
                vb[:])
            vdst = bass.AP(vt_sb[:].tensor, vt_sb[:].offset,
                           [[vt_sb[:].ap[0][0], C], [1, NPAIR * S]])
            vsrc2 = bass.AP(vflat.tensor, vflat.offset,
                            [[1, C], [1, NPAIR * S]])
            nc.scalar.dma_start(vdst, vsrc2)

            # ---------------- phase 4: fc1 (fp8 DR, single psum group) -----
            # h8 lives per column-chunk (ring of 2): fc2(cc) consumes it
            # right after fc1(cc) produces it.
            z1p = ap_.tile([128, AK * 512], BF)

            def h8_tile(cc):
                return sp.tile([128, AK * 512], F8, tag="h8", bufs=2,
                               name=f"h8_{cc}")

            def fc1_x_streams(ps, ak, col, last_stop):
                """6 DR matmuls: xh*w1h + xl*w1h into one group."""
                for hp in range(HK // 2):
                    woff = (2 * hp) * A + ak * 128
                    xoff = (2 * hp) * TOK + col
                    lh_h = pair_ap(w1h_sb[:], woff, A, 128)
                    rh_h = pair_ap(xh_sb[:], xoff, TOK, 512)
                    rh_l = pair_ap(xl_sb[:], xoff, TOK, 512)
                    st = hp == 0
                    nc.tensor.matmul(ps[:], lh_h, rh_h, start=st, stop=False,
                                     perf_mode=PM.DoubleRow)
                    nc.tensor.matmul(ps[:], lh_h, rh_l,
                                     start=False, stop=(hp == 2 and last_stop),
                                     perf_mode=PM.DoubleRow)

            def m1_step(ps, ak, col, start):
                lh = pair_ap(m1_sb[:], ak * 128, A, 128)
                vt_ap = vt_sb[:]
                rh = bass.AP(vt_ap.tensor, vt_ap.offset + 3 * col,
                             [vt_ap.ap[0], [0, 2], [3, 512]])
                nc.tensor.matmul(ps[:], lh, rh, start=start, stop=True,
                                 perf_mode=PM.DoubleRow)

            def fc2_chunk(tt, h8c):
                """fc2 for example tt: h8 x (w2h+w2l), then out = x+relu*g2."""
                psa = pp.tile([128, 512], F32, tag="mm", name=f"ps_f2a_{tt}")
                psb = pp.tile([128, 256], F32, tag="mm", name=f"ps_f2b_{tt}")
                for w_sb, first in ((w2h_sb, True), (w2l_sb, False)):
                    for ap8 in range(AK // 2):
                        woff = (2 * ap8) * H
                        hoff = (2 * ap8) * 512 + (tt % 4) * 128
                        lh = pair_ap(h8c[:], hoff, 512, 128)
                        st = first and ap8 == 0
                        sp_ = (not first) and ap8 == AK // 2 - 1
                        nc.tensor.matmul(psa[:], lh, pair_ap(w_sb[:], woff, H, 512),
                                         start=st, stop=(sp_ and not has_b2),
                                         perf_mode=PM.DoubleRow)
                        nc.tensor.matmul(psb[:], lh,
                                         pair_ap(w_sb[:], woff + 512, H, 256),
                                         start=st, stop=(sp_ and not has_b2),
                                         perf_mode=PM.DoubleRow)
                if has_b2:
                    nc.tensor.matmul(psa[:], ones_bf[:], b2_sb[:, 0:512],
                                     start=False, stop=True)
                    nc.tensor.matmul(psb[:], ones_bf[:], b2_sb[:, 512:H],
                                     start=False, stop=True)
                xt = sp.tile([128, H], BF, tag="x", name=f"x_{tt}", bufs=2)
                nc.sync.dma_start(xt[:], d_xown[tt])
                ot = sp.tile([128, H], F32, tag="o", name=f"o_{tt}", bufs=2)
                nc.scalar.activation(ot[:, 0:512], psa[:], AF.Relu)
                nc.scalar.activation(ot[:, 512:H], psb[:], AF.Relu)
                nc.vector.tensor_tensor(ot[:], ot[:], g2rep[:], op=ALU.mult)
                nc.vector.tensor_tensor(ot[:], ot[:], xt[:], op=ALU.add)
                nc.sync.dma_start(d_out[tt], ot[:])

            # cc0: x-only prepass into z1p (no VT dependency) ...
            for ak in range(AK):
                ps = pp.tile([128, 512], F32, tag="mm", name=f"ps_f1p_{ak}")
                fc1_x_streams(ps, ak, 0, last_stop=True)
                nc.scalar.copy(z1p[:, ak * 512:(ak + 1) * 512], ps[:])
            # ... then m1 catch-up once VT lands
            h8c = h8_tile(0)
            for ak in range(AK):
                ps = pp.tile([128, 512], F32, tag="mm", name=f"ps_f1m_{ak}")
                m1_step(ps, ak, 0, start=True)
                nc.vector.tensor_tensor(
                    ps[:], ps[:], z1p[:, ak * 512:(ak + 1) * 512], op=ALU.add)
                nc.scalar.activation(
                    h8c[:, ak * 512:(ak + 1) * 512],
                    ps[:], AF.Relu, scale=H8SC, bias=b1_sb[:, ak:ak + 1])
            for tt in range(4):
                fc2_chunk(tt, h8c)
            # fused fc1 blocks (m1 last in each psum group), fc2 per chunk
            for cc in range(1, 4):
                h8c = h8_tile(cc)
                for ak in range(AK):
                    ps = pp.tile([128, 512], F32, tag="mm", name=f"ps_f1f_{cc}_{ak}")
                    fc1_x_streams(ps, ak, cc * 512, last_stop=False)
                    m1_step(ps, ak, cc * 512, start=False)
                    nc.scalar.activation(
                        h8c[:, ak * 512:(ak + 1) * 512],
                        ps[:], AF.Relu, scale=H8SC, bias=b1_sb[:, ak:ak + 1])
                for tt in range(4 * cc, 4 * cc + 4):
                    fc2_chunk(tt, h8c)

    nc.compile()
    return nc


# ---------------------------------------------------------------------------
# host marshaling
# ---------------------------------------------------------------------------

def _prep_core_inputs(k, xh_t, xl_t, x, shared):
    # own examples b_i = (48k + 43 i) mod 128; sem examples [16k, 16k+18).
    own = np.array([(48 * k + 43 * i) % B for i in range(BL)])
    sem_ex = np.array([(16 * k + j) % B for j in range(NSEM)])

    def slab(xt):
        return np.ascontiguousarray(
            xt[:, sem_ex, :].reshape(H, STOK).reshape(HK, 128, STOK))

    xsem_h = slab(xh_t)
    xsem_l = slab(xl_t)

    def ccmajor(xt):
        a = xt[:, own, :].reshape(HK, 128, TOK)          # [hk, p, tok]
        return np.ascontiguousarray(
            a.reshape(HK, 128, 4, 512).transpose(2, 0, 1, 3))

    xq_h = ccmajor(xh_t)
    xq_l = ccmajor(xl_t)
    x_own = _bf(x[own])

    masks = np.zeros((C, NPAIR), np.float32)
    for i in range(BL):
        for u in range(C):
            t = 3 * int(own[i]) + u
            masks[t // B, 3 * i + u] = 1.0
    masks3 = np.ascontiguousarray(
        np.broadcast_to(masks.reshape(1, C * NPAIR), (128, C * NPAIR)))

    core = {
        "xsem_h": xsem_h, "xsem_l": xsem_l,
        "xq_h": xq_h, "xq_l": xq_l, "x_own": x_own, "masks3": masks3,
    }
    core.update({n: shared[n] for n in (
        "sw_cat", "semb", "rw_pack", "w1h", "w2h", "w2l",
        "m1q", "b1s", "g2s", "b2row")})
    return core


_CACHE = {}


def _make_shared(inputs):
    fc1_w = np.asarray(inputs["fc1_w"], np.float32)
    fc1_b = np.asarray(inputs["fc1_b"], np.float32)
    fc2_w = np.asarray(inputs["fc2_w"], np.float32)
    fc2_b = np.asarray(inputs["fc2_b"], np.float32)
    efc1 = np.asarray(inputs["efc1"], np.float32)
    efc2 = np.asarray(inputs["efc2"], np.float32)
    sem_w = np.asarray(inputs["sem_w"], np.float32)
    sem_b = np.asarray(inputs["sem_b"], np.float32)
    route_weights = np.asarray(inputs["route_weights"], np.float32)
    larger_w = np.asarray(inputs["larger_w"], np.float32)
    larger_b = np.asarray(inputs["larger_b"], np.float32)
    elarger = np.asarray(inputs["elarger"], np.float32)
    t = int(np.asarray(inputs["t"]))
    sf = np.float32(int(np.asarray(inputs["s"])))
    act_n = t + 1
    x = np.asarray(inputs["x"], np.float32)

    gfc1 = _sigmoid_f32(sf * efc1[t])
    gfc2 = _sigmoid_f32(sf * efc2[t])
    glarger = _sigmoid_f32(sf * elarger[t])

    lwg = (larger_w * glarger[None, :]).astype(np.float32)
    m1mat = (lwg @ fc1_w).astype(np.float32)
    b1 = ((larger_b * glarger) @ fc1_w + fc1_b).astype(np.float32)
    w2g = (fc2_w * gfc1[:, None]).astype(np.float32)
    rw4 = route_weights.reshape(C, N, S, C, S)

    # transposed x quantization (shared across cores; slabs pick columns)
    xt = np.ascontiguousarray(np.transpose(x, (2, 0, 1)))      # [H, B, S]
    xh_t = _q8(xt, SXH).reshape(H, B, S)
    xl_res = xt - xh_t.astype(np.float32) / SXH
    xl_t = _q8(xl_res, SXH).reshape(H, B, S)                   # lo at same scale

    sw2d = np.transpose(sem_w, (1, 0, 2)).reshape(H, NC30)
    swh = _q8(sw2d, SW)
    swl = _q8(sw2d - swh.astype(np.float32) / SW, SW)
    sw_cat = np.concatenate([swh, swl], axis=1)        # [H, 60]
    w1h = _q8(fc1_w, SW)
    w2h = _q8(w2g, SW)
    w2l = _q8(w2g - w2h.astype(np.float32) / SW, SW)

    shared = {
        "sw_cat": np.ascontiguousarray(sw_cat.reshape(HK, 128, 2 * NC30)),
        "semb": np.ascontiguousarray(
            (sem_b.reshape(1, NC30) * 32768.0).astype(np.float32)),
        "rw_pack": np.stack([
            np.ascontiguousarray(np.transpose(rw4[c, :act_n], (1, 0, 2, 3))
                                 .reshape(S, act_n * C * S).astype(BF16))
            for c in range(C)]),
        "w1h": w1h.reshape(HK, 128, A),
        "w2h": w2h.reshape(AK, 128, H),
        "w2l": w2l.reshape(AK, 128, H),
        "m1q": _q8(m1mat, SM1),
        "b1s": np.ascontiguousarray(
            (b1 * SH8).astype(np.float32).reshape(AK, 128).T),
        "g2s": np.ascontiguousarray((gfc2 * Z1INV).reshape(1, H)),
        "b2row": _bf(fc2_b.reshape(1, H)),
        "_xh_t": xh_t, "_xl_t": xl_t,
        "_has_semb": bool(np.any(sem_b)), "_has_b2": bool(np.any(fc2_b)),
    }
    return shared, act_n


def kernel(**inputs):
    x = np.asarray(inputs["x"], np.float32)
    shared, act_n = _make_shared(inputs)
    key = (act_n, shared["_has_semb"], shared["_has_b2"])
    if key not in _CACHE:
        _CACHE[key] = _build_program(act_n, shared["_has_semb"], shared["_has_b2"])
    nc = _CACHE[key]

    in_maps = [_prep_core_inputs(k, shared["_xh_t"], shared["_xl_t"], x, shared)
               for k in range(NCORES)]
    res = bass_utils.run_bass_kernel_spmd(nc, in_maps, core_ids=list(range(NCORES)))
    out = np.empty((B, S, H), np.float32)
    for k in range(NCORES):
        own = [(48 * k + 43 * i) % B for i in range(BL)]
        out[own] = res.results[k]["out"]
    return out
